# revision 1
# baseline (speedup 1.0000x reference)
"""Trainium2 Bass kernel for Qwen-style GQA attention block (B=2,S=2048,H=16,KV=8,D=128).

Sharding (8 cores): batch(2) x si-stripes(2) x head-half(2).
  core c: b=c>>2, sh=(c>>1)&1, hh=c&1
  - Q proj + attention for 8 q-heads (hh half) on 8 causally-balanced si blocks (sh stripes)
  - K/V proj for 4 kv heads over full S (replicated across the 2 stripe cores)
  - pair AllGather of ctx^T between the two head-half cores, then column-split o_proj.
All matmuls bf16 with fp32 PSUM accumulation. Softmax without max-subtraction
(scores are O(1) after QK RMSNorm); denominator via an appended ones-column on V.
"""
import sys

sys.path.insert(0, '/opt/trn_rl_repo')

import numpy as np

import concourse.bass as bass
import concourse.tile as tile
from concourse import mybir
from concourse.vector_clock import ScopedClock, VectorClock

B, S, HID = 2, 2048, 2048
H, KV, D = 16, 8, 128
EPS = 1e-6
SCALE = D ** -0.5
NBLK = S // 128  # 16
# causally balanced si-block stripes: sum(i+1) = 68 for both
MYBLKS = [[0, 2, 4, 6, 9, 11, 13, 15], [1, 3, 5, 7, 8, 10, 12, 14]]

F32 = mybir.dt.float32
BF16 = mybir.dt.bfloat16
AF = mybir.ActivationFunctionType


# ---------------------------------------------------------------------------
# Workarounds: this walrus supports only ONE sync-wait per instruction.
def _patched_drain_and_barrier(self, tick_clock, wait_clock):
    gc = tick_clock.global_clock
    vec = list(gc)
    nz = [i for i, v in enumerate(vec) if v > 0] or [0]
    for i in nz:
        cvec = [vec[j] if j == i else 0 for j in range(len(vec))]
        inst = self.nc.sync.drain()
        wait_clock.add_sem_waits(inst.ins, ScopedClock({None: VectorClock(cvec)}))
    self.nc.all_engine_barrier()
    assert self.sems is not None
    popped = self.nc._tile_sem_poison_stack.pop()
    assert popped is self._sem_poison
    self.nc.clear_and_free_semaphores(list(self.sems.allocated().values()))
    self.nc.all_engine_barrier()


tile.TileContext._drain_and_barrier = _patched_drain_and_barrier


def split_multi_waits(nc):
    for fn in nc.m.functions:
        for blk in fn.blocks:
            insts = list(blk.instructions)
            out = []
            changed = False
            for inst in insts:
                si = inst.sync_info
                if si is not None and len(si.on_wait) > 1:
                    waits = list(si.on_wait)
                    for k, w in enumerate(waits[:-1]):
                        out.append(mybir.InstNoOp(
                            name=f"{inst.name}.w{k}", engine=inst.engine,
                            sync_info=mybir.SyncInfo(on_wait=[w], on_update=[]),
                            text_hint="waitsplit"))
                    si.on_wait = [waits[-1]]
                    changed = True
                out.append(inst)
            if changed:
                blk.instructions[:] = out


# ---------------------------------------------------------------------------
def build_kernel():
    nc = bass.Bass(trn_type='TRN2')
    hT = nc.dram_tensor('hT', [HID, S], F32, kind='ExternalInput')
    qwT = nc.dram_tensor('qwT', [HID, 1024], F32, kind='ExternalInput')
    kwT = nc.dram_tensor('kwT', [HID, 512], F32, kind='ExternalInput')
    vwT = nc.dram_tensor('vwT', [HID, 512], F32, kind='ExternalInput')
    owT = nc.dram_tensor('owT', [2048, 1024], F32, kind='ExternalInput')
    # host-fused rope tables (cos/sin x norm-weight halves), [rows, 4, 64]
    qtab = nc.dram_tensor('qtab', [1024, 4, 64], F32, kind='ExternalInput')
    ktab = nc.dram_tensor('ktab', [S, 4, 64], F32, kind='ExternalInput')
    tri = nc.dram_tensor('tri', [128, 128], F32, kind='ExternalInput')
    iden = nc.dram_tensor('iden', [128, 128], F32, kind='ExternalInput')
    out_e = nc.dram_tensor('out', [1024, 1024], F32, kind='ExternalOutput')

    from contextlib import ExitStack
    with ExitStack() as ctx:
        tc = ctx.enter_context(tile.TileContext(nc))
        pool = lambda name, bufs, **kw: ctx.enter_context(
            tc.tile_pool(name=name, bufs=bufs, **kw))
        p_wq = pool('wq', 16)
        p_wk = pool('wk', 16)
        p_wv = pool('wv', 16)
        p_ht = pool('ht', 16)
        p_qt = pool('qt', 8)
        p_kt = pool('kt', 4)
        p_va = pool('va', 4)
        p_ctm = pool('ctm', 8)
        p_c = pool('const', 1)
        p_w = pool('work', 2)
        p_s = pool('small', 4)
        p_scl = pool('scl', 1)
        p_exp = pool('expb', 4)
        p_out = pool('outb', 2)
        ps_a = pool('psA', 2, space='PSUM')
        ps_s = pool('psS', 2, space='PSUM')
        ps_c = pool('psC', 2, space='PSUM')
        ps_t = pool('psT', 2, space='PSUM')
        p_d = pool('dram', 1, space='DRAM')
        if True:
            # ---- constants / weights (cast to bf16 on load) ----
            tri_s = p_c.tile([128, 128], BF16)
            nc.gpsimd.dma_start(tri_s[:], tri[:])
            iden_s = p_c.tile([128, 128], BF16)
            nc.gpsimd.dma_start(iden_s[:], iden[:])
            qtab_s = p_c.tile([128, 8, 4, 64], BF16)
            nc.gpsimd.dma_start(qtab_s[:], qtab.rearrange('(n p) t d -> p n t d', p=128))
            ktab_s = p_c.tile([128, 16, 4, 64], BF16)
            nc.gpsimd.dma_start(ktab_s[:], ktab.rearrange('(n p) t d -> p n t d', p=128))

            wq_s = [p_wq.tile([128, 1024], BF16, tag='wq', name='wq') for _ in range(16)]
            wk_s = [p_wk.tile([128, 512], BF16, tag='wk', name='wk') for _ in range(16)]
            wv_s = [p_wv.tile([128, 512], BF16, tag='wv', name='wv') for _ in range(16)]
            for ch in range(16):
                r = bass.ts(ch, 128)
                nc.gpsimd.dma_start(wq_s[ch][:], qwT[r, :])
                nc.gpsimd.dma_start(wk_s[ch][:], kwT[r, :])
                nc.gpsimd.dma_start(wv_s[ch][:], vwT[r, :])

            # persistent activation tiles
            QT = [p_qt.tile([128, 1024], BF16, tag='qt', name='qtl') for _ in range(8)]
            KT = [p_kt.tile([128, 2048], BF16, tag='kt', name='ktl') for _ in range(4)]
            VA = [p_va.tile([128, 16, 132], BF16, tag='va', name='va') for _ in range(4)]
            sclK = p_scl.tile([128, 16, 4], F32)   # SCALE * rstd_k per (sj_blk, kv)
            ctm = [p_ctm.tile([128, 1024], BF16, tag='ctm', name='ctm') for _ in range(8)]

            for kvh in range(4):  # ones column for the softmax denominator
                nc.gpsimd.memset(VA[kvh][:, :, 128:129], 1.0)

            bounds = [max(MYBLKS[0][bi], MYBLKS[1][bi]) for bi in range(8)]
            # per-core diagonal masks: dmask[bi][j] for j in {bounds[bi]-1, bounds[bi]}
            # encoded via a single input: dm [8, 2, 128, 128]
            dm = nc.dram_tensor('dm', [8, 2, 128, 128], F32, kind='ExternalInput')
            dm_s = p_c.tile([128, 8, 2, 128], BF16)
            nc.gpsimd.dma_start(dm_s[:], dm.rearrange('n t p d -> p n t d'))

            # ---- projections, two passes over s-halves ----
            for ph in range(2):
                ht_t = [p_ht.tile([128, 1024], BF16, tag='ht', name='ht') for _ in range(16)]
                for ch in range(16):
                    nc.gpsimd.dma_start(
                        ht_t[ch][:], hT[bass.ts(ch, 128), bass.ts(ph, 1024)])
                for j in range(8):
                    sb = ph * 8 + j
                    sslice = bass.ts(j, 128)
                    # ---- V ----
                    psV = ps_a.tile([128, 512], F32, tag='psA', name='psA')
                    for ch in range(16):
                        nc.tensor.matmul(psV[:], ht_t[ch][:, sslice], wv_s[ch][:],
                                         start=(ch == 0), stop=(ch == 15))
                    for kvh in range(4):
                        nc.scalar.copy(VA[kvh][:, sb, 0:128], psV[:, bass.ts(kvh, 128)])
                    # ---- K ----
                    psK = ps_a.tile([128, 512], F32, tag='psA', name='psA')
                    for ch in range(16):
                        nc.tensor.matmul(psK[:], ht_t[ch][:, sslice], wk_s[ch][:],
                                         start=(ch == 0), stop=(ch == 15))
                    kcp = p_w.tile([128, 512], F32, tag='kcp', name='kcp')
                    nc.scalar.copy(kcp[:], psK[:])
                    scr = p_w.tile([128, 512], F32, tag='scr', name='scr')
                    ss = p_s.tile([128, 4], F32, tag='ss', name='ss')
                    for kvh in range(4):
                        nc.scalar.activation(scr[:, bass.ts(kvh, 128)],
                                             kcp[:, bass.ts(kvh, 128)], AF.Square,
                                             accum_out=ss[:, kvh:kvh + 1])
                    nc.vector.tensor_scalar_add(ss[:], ss[:], float(EPS * D))
                    std = p_s.tile([128, 4], F32, tag='std', name='std')
                    nc.scalar.activation(std[:], ss[:], AF.Sqrt, scale=1.0 / D, bias=0.0)
                    rstd = p_s.tile([128, 4], F32, tag='rstd', name='rstd')
                    nc.vector.reciprocal(rstd[:], std[:])
                    nc.vector.tensor_scalar_mul(sclK[:, sb, :], rstd[:], SCALE)
                    # rope on raw K (w folded into ktab; rstd folded into exp scale)
                    kro = p_w.tile([128, 4, 128], BF16, tag='kro', name='kro')
                    lo = kcp[:].rearrange('p (t d) -> p t d', t=4)[:, :, 0:64]
                    hi = kcp[:].rearrange('p (t d) -> p t d', t=4)[:, :, 64:128]
                    tA = ktab_s[:, sb, :, :][:, 0:1, :]
                    tB = ktab_s[:, sb, :, :][:, 1:2, :]
                    tC = ktab_s[:, sb, :, :][:, 2:3, :]
                    tD = ktab_s[:, sb, :, :][:, 3:4, :]
                    t1 = p_w.tile([128, 4, 64], F32, tag='t1', name='t1')
                    t2 = p_w.tile([128, 4, 64], F32, tag='t2', name='t2')
                    mul_b(nc, t1[:], lo, tA)
                    mul_b(nc, t2[:], hi, tB)
                    nc.vector.tensor_sub(kro[:, :, 0:64], t1[:], t2[:])
                    mul_b(nc, t1[:], hi, tC)
                    mul_b(nc, t2[:], lo, tD)
                    nc.vector.tensor_add(kro[:, :, 64:128], t1[:], t2[:])
                    for kvh in range(4):  # transpose to KT
                        pst = ps_t.tile([128, 128], BF16, tag='psT', name='psT')
                        nc.tensor.transpose(pst[:], kro[:, kvh, :], iden_s[:])
                        nc.scalar.copy(KT[kvh][:, bass.ts(sb, 128)], pst[:])
                del ht_t

            # ---- Q projection from host-gathered hTq (my si rows, local order) ----
            hTq = nc.dram_tensor('hTq', [HID, 1024], F32, kind='ExternalInput')
            htq_t = [p_ht.tile([128, 1024], BF16, tag='ht', name='ht') for _ in range(16)]
            for ch in range(16):
                nc.gpsimd.dma_start(htq_t[ch][:], hTq[bass.ts(ch, 128), :])
            for bi in range(8):
                sslice = bass.ts(bi, 128)
                for qg in range(2):
                    psQ = ps_a.tile([128, 512], F32, tag='psA', name='psA')
                    for ch in range(16):
                        nc.tensor.matmul(psQ[:], htq_t[ch][:, sslice],
                                         wq_s[ch][:, bass.ts(qg, 512)],
                                         start=(ch == 0), stop=(ch == 15))
                    qcp = p_w.tile([128, 512], F32, tag='kcp', name='qcp')
                    nc.scalar.copy(qcp[:], psQ[:])
                    scr = p_w.tile([128, 512], F32, tag='scr', name='scr')
                    ss = p_s.tile([128, 4], F32, tag='ss', name='ss')
                    for hq in range(4):
                        nc.scalar.activation(scr[:, bass.ts(hq, 128)],
                                             qcp[:, bass.ts(hq, 128)], AF.Square,
                                             accum_out=ss[:, hq:hq + 1])
                    nc.vector.tensor_scalar_add(ss[:], ss[:], float(EPS * D))
                    std = p_s.tile([128, 4], F32, tag='std', name='std')
                    nc.scalar.activation(std[:], ss[:], AF.Sqrt, scale=1.0 / D, bias=0.0)
                    rstd = p_s.tile([128, 4], F32, tag='rstd', name='rstd')
                    nc.vector.reciprocal(rstd[:], std[:])
                    qro = p_w.tile([128, 4, 128], BF16, tag='kro', name='kro')
                    lo = qcp[:].rearrange('p (t d) -> p t d', t=4)[:, :, 0:64]
                    hi = qcp[:].rearrange('p (t d) -> p t d', t=4)[:, :, 64:128]
                    tA = qtab_s[:, bi, :, :][:, 0:1, :]
                    tB = qtab_s[:, bi, :, :][:, 1:2, :]
                    tC = qtab_s[:, bi, :, :][:, 2:3, :]
                    tD = qtab_s[:, bi, :, :][:, 3:4, :]
                    t1 = p_w.tile([128, 4, 64], F32, tag='t1', name='t1')
                    t2 = p_w.tile([128, 4, 64], F32, tag='t2', name='t2')
                    mul_b(nc, t1[:], lo, tA)
                    mul_b(nc, t2[:], hi, tB)
                    nc.vector.tensor_sub(qro[:, :, 0:64], t1[:], t2[:])
                    mul_b(nc, t1[:], hi, tC)
                    mul_b(nc, t2[:], lo, tD)
                    nc.vector.tensor_add(qro[:, :, 64:128], t1[:], t2[:])
                    qn = p_w.tile([128, 4, 128], BF16, tag='qn', name='qn')
                    for hq in range(4):
                        nc.vector.tensor_scalar_mul(qn[:, hq, :], qro[:, hq, :],
                                                    rstd[:, hq:hq + 1])
                        pst = ps_t.tile([128, 128], BF16, tag='psT', name='psT')
                        nc.tensor.transpose(pst[:], qn[:, hq, :], iden_s[:])
                        nc.scalar.copy(QT[qg * 4 + hq][:, bass.ts(bi, 128)], pst[:])

            # ---- attention ----
            for h in range(8):
                kvh = h // 2
                for bi in range(8):
                    gi = bounds[bi]
                    psC = ps_c.tile([128, 132], F32, tag='psC', name='psC')
                    for j in range(gi + 1):
                        psS = ps_s.tile([128, 128], F32, tag='psS', name='psS')
                        nc.tensor.matmul(psS[:], KT[kvh][:, bass.ts(j, 128)],
                                         QT[h][:, bass.ts(bi, 128)],
                                         start=True, stop=True)
                        ex = p_exp.tile([128, 128], BF16, tag='expb', name='expb')
                        nc.scalar.activation(ex[:], psS[:], AF.Exp,
                                             scale=sclK[:, j, kvh:kvh + 1])
                        if j >= gi - 1:  # possible diagonal/overhang: apply mask
                            nc.vector.tensor_mul(ex[:], ex[:], dm_s[:, bi, j - (gi - 1), :])
                        nc.tensor.matmul(psC[:, 0:129], ex[:], VA[kvh][:, j, 0:129],
                                         start=(j == 0), stop=(j == gi))
                    rd = p_s.tile([128, 1], F32, tag='rd', name='rd')
                    nc.vector.reciprocal(rd[:], psC[:, 128:129])
                    cn = p_w.tile([128, 128], BF16, tag='cn', name='cn')
                    nc.vector.tensor_scalar_mul(cn[:], psC[:, 0:128], rd[:])
                    pst = ps_t.tile([128, 128], BF16, tag='psT', name='psT')
                    nc.tensor.transpose(pst[:], cn[:], iden_s[:])
                    nc.scalar.copy(ctm[h][:, bass.ts(bi, 128)], pst[:])

            wo_s = [p_ht.tile([128, 1024], BF16, tag='ht', name='wo') for _ in range(16)]
            for ch in range(16):
                nc.gpsimd.dma_start(wo_s[ch][:], owT[bass.ts(ch, 128), :])

            # ---- pair AllGather of ctx^T ----
            cc_in = p_d.tile([1024, 1024], BF16)
            cc_out = p_d.tile([2048, 1024], BF16)
            for h in range(8):
                nc.sync.dma_start(cc_in[bass.ts(h, 128), :], ctm[h][:])
            nc.gpsimd.collective_compute(
                'AllGather', mybir.AluOpType.bypass,
                replica_groups=[[0, 1], [2, 3], [4, 5], [6, 7]],
                ins=[cc_in.opt()], outs=[cc_out.opt()])
            ctf = [p_wq.tile([128, 1024], BF16, tag='wq', name='ctf') for _ in range(16)]
            for ch in range(16):
                nc.sync.dma_start(ctf[ch][:], cc_out[bass.ts(ch, 128), :])

            # ---- o_proj (my ho half columns) ----
            for bi in range(8):
                for nt in range(2):
                    psO = ps_a.tile([128, 512], F32, tag='psA', name='psA')
                    for ch in range(16):
                        nc.tensor.matmul(psO[:], ctf[ch][:, bass.ts(bi, 128)],
                                         wo_s[ch][:, bass.ts(nt, 512)],
                                         start=(ch == 0), stop=(ch == 15))
                    ob = p_out.tile([128, 512], F32, tag='outb', name='outb')
                    nc.scalar.copy(ob[:], psO[:])
                    nc.sync.dma_start(out_e[bass.ts(bi, 128), bass.ts(nt, 512)], ob[:])

    split_multi_waits(nc)
    return nc


def mul_b(nc, out, a, b):
    """tensor_tensor multiply with free-dim broadcast of b over dim 1."""
    a2, b2 = bass.broadcast_tensor_aps(a, b)
    nc.vector.tensor_mul(out, a2, b2)


# ---------------------------------------------------------------------------
_NC_CACHE = None


def _get_nc():
    global _NC_CACHE
    if _NC_CACHE is None:
        _NC_CACHE = build_kernel()
    return _NC_CACHE


def kernel(hidden_states, cos, sin, q_w, k_w, v_w, o_w, q_norm_w, k_norm_w):
    from concourse.bass_utils import run_bass_kernel_spmd

    hidden_states = np.asarray(hidden_states, np.float32)
    cos = np.asarray(cos, np.float32)
    sin = np.asarray(sin, np.float32)
    q_w = np.asarray(q_w, np.float32)
    k_w = np.asarray(k_w, np.float32)
    v_w = np.asarray(v_w, np.float32)
    o_w = np.asarray(o_w, np.float32)
    q_norm_w = np.asarray(q_norm_w, np.float32)
    k_norm_w = np.asarray(k_norm_w, np.float32)

    tri_np = np.triu(np.ones((128, 128), np.float32))  # [sj,si]: valid sj<=si
    iden_np = np.eye(128, dtype=np.float32)

    def rope_tabs(c, s_, w):
        # tables [rows, 4, 64]: A=c_lo*w_lo, B=s_lo*w_hi, C=c_lo*w_hi, D=s_lo*w_lo
        cl, sl = c[:, 0:64], s_[:, 0:64]
        wl, wh = w[0:64], w[64:128]
        return np.stack([cl * wl, sl * wh, cl * wh, sl * wl], axis=1).astype(np.float32)

    bounds = [max(MYBLKS[0][bi], MYBLKS[1][bi]) for bi in range(8)]

    in_maps = []
    for c in range(8):
        b, sh, hh = c >> 2, (c >> 1) & 1, c & 1
        blks = MYBLKS[sh]
        rows = np.concatenate([np.arange(g * 128, (g + 1) * 128) for g in blks])
        hT = np.ascontiguousarray(hidden_states[b].T)
        hTq = np.ascontiguousarray(hidden_states[b][rows].T)
        qwT = np.ascontiguousarray(q_w[hh * 1024:(hh + 1) * 1024].T)
        kwT = np.ascontiguousarray(k_w[hh * 512:(hh + 1) * 512].T)
        vwT = np.ascontiguousarray(v_w[hh * 512:(hh + 1) * 512].T)
        owT = np.ascontiguousarray(o_w[hh * 1024:(hh + 1) * 1024].T)
        qtab = rope_tabs(cos[b][rows], sin[b][rows], q_norm_w)
        ktab = rope_tabs(cos[b], sin[b], k_norm_w)
        # diagonal masks dm[bi, t]: t=0 -> sj block gi-1, t=1 -> sj block gi
        # my true causal diagonal is at block g=blks[bi] (<= bounds[bi]).
        dm = np.zeros((8, 2, 128, 128), np.float32)
        for bi in range(8):
            g, gb = blks[bi], bounds[bi]
            for t, j in enumerate((gb - 1, gb)):
                if j < 0:
                    continue
                if j < g:
                    dm[bi, t] = 1.0
                elif j == g:
                    dm[bi, t] = tri_np
                # j > g: stays 0 (block fully masked)
        in_maps.append(dict(
            hT=hT, hTq=hTq, qwT=qwT, kwT=kwT, vwT=vwT, owT=owT,
            qtab=qtab, ktab=ktab, tri=tri_np, iden=iden_np, dm=dm))

    nc = _get_nc()
    res = run_bass_kernel_spmd(nc, in_maps, core_ids=list(range(8)))

    out = np.zeros((B, S, HID), np.float32)
    for c in range(8):
        b, sh, hh = c >> 2, (c >> 1) & 1, c & 1
        o = res.results[c]['out']  # [1024, 1024]
        for bi, g in enumerate(MYBLKS[sh]):
            out[b, g * 128:(g + 1) * 128, hh * 1024:(hh + 1) * 1024] = \
                o[bi * 128:(bi + 1) * 128]
    return out


if __name__ == '__main__':
    sys.path.insert(0, '/root/problem')
    import reference
    inputs = {k: np.asarray(v) for k, v in reference.setup_inputs().items()}
    exp = np.asarray(reference.reference(**inputs))
    act = kernel(**inputs)
    err = np.abs(act - exp)
    rel = np.linalg.norm(act - exp) / np.linalg.norm(exp)
    print('Relative error:', rel, 'max abs err:', err.max())



# revision 3
# speedup vs baseline: 1.1853x; 1.1853x over previous
"""Trainium2 Bass kernel for Qwen-style GQA attention block (B=2,S=2048,H=16,KV=8,D=128).

Sharding (8 cores): batch(2) x si-stripes(2) x head-half(2).
  core c: b=c>>2, sh=(c>>1)&1, hh=c&1
  - Q proj + attention for 8 q-heads (hh half) on 8 causally-balanced si blocks (sh stripes)
  - K/V proj for 4 kv heads over full S (replicated across the 2 stripe cores)
  - pair AllGather of ctx^T between the two head-half cores, then column-split o_proj.
All matmuls bf16 with fp32 PSUM accumulation. Softmax without max-subtraction
(scores are O(1) after QK RMSNorm); denominator via an appended ones-column on V.
"""
import sys

sys.path.insert(0, '/opt/trn_rl_repo')

import numpy as np

import concourse.bass as bass
import concourse.tile as tile
from concourse import mybir
from concourse.vector_clock import ScopedClock, VectorClock

B, S, HID = 2, 2048, 2048
H, KV, D = 16, 8, 128
EPS = 1e-6
SCALE = D ** -0.5
NBLK = S // 128  # 16
# causally balanced si-block stripes: sum(i+1) = 68 for both
MYBLKS = [[0, 2, 4, 6, 9, 11, 13, 15], [1, 3, 5, 7, 8, 10, 12, 14]]

F32 = mybir.dt.float32
BF16 = mybir.dt.bfloat16
AF = mybir.ActivationFunctionType


# ---------------------------------------------------------------------------
# Workarounds: this walrus supports only ONE sync-wait per instruction.
def _patched_drain_and_barrier(self, tick_clock, wait_clock):
    gc = tick_clock.global_clock
    vec = list(gc)
    nz = [i for i, v in enumerate(vec) if v > 0] or [0]
    for i in nz:
        cvec = [vec[j] if j == i else 0 for j in range(len(vec))]
        inst = self.nc.sync.drain()
        wait_clock.add_sem_waits(inst.ins, ScopedClock({None: VectorClock(cvec)}))
    self.nc.all_engine_barrier()
    assert self.sems is not None
    popped = self.nc._tile_sem_poison_stack.pop()
    assert popped is self._sem_poison
    self.nc.clear_and_free_semaphores(list(self.sems.allocated().values()))
    self.nc.all_engine_barrier()


tile.TileContext._drain_and_barrier = _patched_drain_and_barrier


def split_multi_waits(nc):
    for fn in nc.m.functions:
        for blk in fn.blocks:
            insts = list(blk.instructions)
            out = []
            changed = False
            for inst in insts:
                si = inst.sync_info
                if si is not None and len(si.on_wait) > 1:
                    waits = list(si.on_wait)
                    for k, w in enumerate(waits[:-1]):
                        out.append(mybir.InstNoOp(
                            name=f"{inst.name}.w{k}", engine=inst.engine,
                            sync_info=mybir.SyncInfo(on_wait=[w], on_update=[]),
                            text_hint="waitsplit"))
                    si.on_wait = [waits[-1]]
                    changed = True
                out.append(inst)
            if changed:
                blk.instructions[:] = out


# ---------------------------------------------------------------------------
def build_kernel():
    nc = bass.Bass(trn_type='TRN2')
    hT = nc.dram_tensor('hT', [HID, S], F32, kind='ExternalInput')
    qwT = nc.dram_tensor('qwT', [HID, 1024], F32, kind='ExternalInput')
    kwT = nc.dram_tensor('kwT', [HID, 512], F32, kind='ExternalInput')
    vwT = nc.dram_tensor('vwT', [HID, 512], F32, kind='ExternalInput')
    owT = nc.dram_tensor('owT', [2048, 1024], F32, kind='ExternalInput')
    # host-fused rope tables (cos/sin x norm-weight halves), [rows, 4, 64]
    qtab = nc.dram_tensor('qtab', [1024, 4, 64], F32, kind='ExternalInput')
    ktab = nc.dram_tensor('ktab', [S, 4, 64], F32, kind='ExternalInput')
    tri = nc.dram_tensor('tri', [128, 128], F32, kind='ExternalInput')
    iden = nc.dram_tensor('iden', [128, 128], F32, kind='ExternalInput')
    out_e = nc.dram_tensor('out', [1024, 1024], F32, kind='ExternalOutput')

    from contextlib import ExitStack
    with ExitStack() as ctx:
        tc = ctx.enter_context(tile.TileContext(nc))
        pool = lambda name, bufs, **kw: ctx.enter_context(
            tc.tile_pool(name=name, bufs=bufs, **kw))
        p_wq = pool('wq', 16)
        p_wk = pool('wk', 16)
        p_wv = pool('wv', 16)
        p_ht = pool('ht', 16)
        p_qt = pool('qt', 8)
        p_kt = pool('kt', 4)
        p_va = pool('va', 4)
        p_ctm = pool('ctm', 8)
        p_c = pool('const', 1)
        p_w = pool('work', 2)
        p_s = pool('small', 4)
        p_scl = pool('scl', 1)
        p_exp = pool('expb', 4)
        p_out = pool('outb', 2)
        ps_a = pool('psA', 2, space='PSUM')
        ps_s = pool('psS', 2, space='PSUM')
        ps_c = pool('psC', 2, space='PSUM')
        ps_t = pool('psT', 2, space='PSUM')
        p_d = pool('dram', 1, space='DRAM')
        if True:
            # ---- constants / weights (cast to bf16 on load) ----
            tri_s = p_c.tile([128, 128], BF16)
            nc.gpsimd.dma_start(tri_s[:], tri[:])
            iden_s = p_c.tile([128, 128], BF16)
            nc.gpsimd.dma_start(iden_s[:], iden[:])
            qtab_s = p_c.tile([128, 8, 4, 64], BF16)
            nc.gpsimd.dma_start(qtab_s[:], qtab.rearrange('(n p) t d -> p n t d', p=128))
            ktab_s = p_c.tile([128, 16, 4, 64], BF16)
            nc.gpsimd.dma_start(ktab_s[:], ktab.rearrange('(n p) t d -> p n t d', p=128))

            wq_s = [p_wq.tile([128, 1024], BF16, tag='wq', name='wq') for _ in range(16)]
            wk_s = [p_wk.tile([128, 512], BF16, tag='wk', name='wk') for _ in range(16)]
            wv_s = [p_wv.tile([128, 512], BF16, tag='wv', name='wv') for _ in range(16)]
            for ch in range(16):
                r = bass.ts(ch, 128)
                nc.gpsimd.dma_start(wq_s[ch][:], qwT[r, :])
                nc.gpsimd.dma_start(wk_s[ch][:], kwT[r, :])
                nc.gpsimd.dma_start(wv_s[ch][:], vwT[r, :])

            # persistent activation tiles
            QT = [p_qt.tile([128, 1024], BF16, tag='qt', name='qtl') for _ in range(8)]
            KT = [p_kt.tile([128, 2048], BF16, tag='kt', name='ktl') for _ in range(4)]
            VA = [p_va.tile([128, 16, 132], BF16, tag='va', name='va') for _ in range(4)]
            sclK = p_scl.tile([128, 16, 4], F32)   # SCALE * rstd_k per (sj_blk, kv)
            ctm = [p_ctm.tile([128, 1024], BF16, tag='ctm', name='ctm') for _ in range(8)]

            for kvh in range(4):  # ones column for the softmax denominator
                nc.gpsimd.memset(VA[kvh][:, :, 128:129], 1.0)

            bounds = [max(MYBLKS[0][bi], MYBLKS[1][bi]) for bi in range(8)]
            # per-core diagonal masks: dmask[bi][j] for j in {bounds[bi]-1, bounds[bi]}
            # encoded via a single input: dm [8, 2, 128, 128]
            dm = nc.dram_tensor('dm', [8, 2, 128, 128], F32, kind='ExternalInput')
            dm_s = p_c.tile([128, 8, 2, 128], BF16)
            nc.gpsimd.dma_start(dm_s[:], dm.rearrange('n t p d -> p n t d'))

            # ---- projections, two passes over s-halves ----
            for ph in range(2):
                ht_t = [p_ht.tile([128, 1024], BF16, tag='ht', name='ht') for _ in range(16)]
                for ch in range(16):
                    nc.gpsimd.dma_start(
                        ht_t[ch][:], hT[bass.ts(ch, 128), bass.ts(ph, 1024)])
                for j in range(8):
                    sb = ph * 8 + j
                    sslice = bass.ts(j, 128)
                    # ---- V ----
                    psV = ps_a.tile([128, 512], F32, tag='psA', name='psA')
                    for ch in range(16):
                        nc.tensor.matmul(psV[:], ht_t[ch][:, sslice], wv_s[ch][:],
                                         start=(ch == 0), stop=(ch == 15))
                    for kvh in range(4):
                        nc.scalar.copy(VA[kvh][:, sb, 0:128], psV[:, bass.ts(kvh, 128)])
                    # ---- K ----
                    psK = ps_a.tile([128, 512], F32, tag='psA', name='psA')
                    for ch in range(16):
                        nc.tensor.matmul(psK[:], ht_t[ch][:, sslice], wk_s[ch][:],
                                         start=(ch == 0), stop=(ch == 15))
                    kcp = p_w.tile([128, 512], F32, tag='kcp', name='kcp')
                    nc.scalar.copy(kcp[:], psK[:])
                    scr = p_w.tile([128, 512], F32, tag='scr', name='scr')
                    ss = p_s.tile([128, 4], F32, tag='ss', name='ss')
                    for kvh in range(4):
                        nc.scalar.activation(scr[:, bass.ts(kvh, 128)],
                                             kcp[:, bass.ts(kvh, 128)], AF.Square,
                                             accum_out=ss[:, kvh:kvh + 1])
                    nc.vector.tensor_scalar_add(ss[:], ss[:], float(EPS * D))
                    std = p_s.tile([128, 4], F32, tag='std', name='std')
                    nc.scalar.activation(std[:], ss[:], AF.Sqrt, scale=1.0 / D, bias=0.0)
                    rstd = p_s.tile([128, 4], F32, tag='rstd', name='rstd')
                    nc.vector.reciprocal(rstd[:], std[:])
                    nc.vector.tensor_scalar_mul(sclK[:, sb, :], rstd[:], SCALE)
                    # rope on raw K (w folded into ktab; rstd folded into exp scale)
                    kro = p_w.tile([128, 4, 128], BF16, tag='kro', name='kro')
                    lo = kcp[:].rearrange('p (t d) -> p t d', t=4)[:, :, 0:64]
                    hi = kcp[:].rearrange('p (t d) -> p t d', t=4)[:, :, 64:128]
                    tA = ktab_s[:, sb, :, :][:, 0:1, :]
                    tB = ktab_s[:, sb, :, :][:, 1:2, :]
                    tC = ktab_s[:, sb, :, :][:, 2:3, :]
                    tD = ktab_s[:, sb, :, :][:, 3:4, :]
                    t1 = p_w.tile([128, 4, 64], F32, tag='t1', name='t1')
                    t2 = p_w.tile([128, 4, 64], F32, tag='t2', name='t2')
                    mul_b(nc, t1[:], lo, tA)
                    mul_b(nc, t2[:], hi, tB)
                    nc.vector.tensor_sub(kro[:, :, 0:64], t1[:], t2[:])
                    mul_b(nc, t1[:], hi, tC)
                    mul_b(nc, t2[:], lo, tD)
                    nc.vector.tensor_add(kro[:, :, 64:128], t1[:], t2[:])
                    for kvh in range(4):  # transpose to KT
                        pst = ps_t.tile([128, 128], BF16, tag='psT', name='psT')
                        nc.tensor.transpose(pst[:], kro[:, kvh, :], iden_s[:])
                        nc.scalar.copy(KT[kvh][:, bass.ts(sb, 128)], pst[:])
                del ht_t

            # ---- Q projection from host-gathered hTq (my si rows, local order) ----
            hTq = nc.dram_tensor('hTq', [HID, 1024], F32, kind='ExternalInput')
            htq_t = [p_ht.tile([128, 1024], BF16, tag='ht', name='ht') for _ in range(16)]
            for ch in range(16):
                nc.gpsimd.dma_start(htq_t[ch][:], hTq[bass.ts(ch, 128), :])
            for bi in range(8):
                sslice = bass.ts(bi, 128)
                for qg in range(2):
                    psQ = ps_a.tile([128, 512], F32, tag='psA', name='psA')
                    for ch in range(16):
                        nc.tensor.matmul(psQ[:], htq_t[ch][:, sslice],
                                         wq_s[ch][:, bass.ts(qg, 512)],
                                         start=(ch == 0), stop=(ch == 15))
                    qcp = p_w.tile([128, 512], F32, tag='kcp', name='qcp')
                    nc.scalar.copy(qcp[:], psQ[:])
                    scr = p_w.tile([128, 512], F32, tag='scr', name='scr')
                    ss = p_s.tile([128, 4], F32, tag='ss', name='ss')
                    for hq in range(4):
                        nc.scalar.activation(scr[:, bass.ts(hq, 128)],
                                             qcp[:, bass.ts(hq, 128)], AF.Square,
                                             accum_out=ss[:, hq:hq + 1])
                    nc.vector.tensor_scalar_add(ss[:], ss[:], float(EPS * D))
                    std = p_s.tile([128, 4], F32, tag='std', name='std')
                    nc.scalar.activation(std[:], ss[:], AF.Sqrt, scale=1.0 / D, bias=0.0)
                    rstd = p_s.tile([128, 4], F32, tag='rstd', name='rstd')
                    nc.vector.reciprocal(rstd[:], std[:])
                    qro = p_w.tile([128, 4, 128], BF16, tag='kro', name='kro')
                    lo = qcp[:].rearrange('p (t d) -> p t d', t=4)[:, :, 0:64]
                    hi = qcp[:].rearrange('p (t d) -> p t d', t=4)[:, :, 64:128]
                    tA = qtab_s[:, bi, :, :][:, 0:1, :]
                    tB = qtab_s[:, bi, :, :][:, 1:2, :]
                    tC = qtab_s[:, bi, :, :][:, 2:3, :]
                    tD = qtab_s[:, bi, :, :][:, 3:4, :]
                    t1 = p_w.tile([128, 4, 64], F32, tag='t1', name='t1')
                    t2 = p_w.tile([128, 4, 64], F32, tag='t2', name='t2')
                    mul_b(nc, t1[:], lo, tA)
                    mul_b(nc, t2[:], hi, tB)
                    nc.vector.tensor_sub(qro[:, :, 0:64], t1[:], t2[:])
                    mul_b(nc, t1[:], hi, tC)
                    mul_b(nc, t2[:], lo, tD)
                    nc.vector.tensor_add(qro[:, :, 64:128], t1[:], t2[:])
                    qn = p_w.tile([128, 4, 128], BF16, tag='qn', name='qn')
                    for hq in range(4):
                        nc.vector.tensor_scalar_mul(qn[:, hq, :], qro[:, hq, :],
                                                    rstd[:, hq:hq + 1])
                        pst = ps_t.tile([128, 128], BF16, tag='psT', name='psT')
                        nc.tensor.transpose(pst[:], qn[:, hq, :], iden_s[:])
                        nc.scalar.copy(QT[qg * 4 + hq][:, bass.ts(bi, 128)], pst[:])

            # ---- attention ----
            for h in range(8):
                kvh = h // 2
                for bi in range(8):
                    gi = bounds[bi]
                    psC = ps_c.tile([128, 132], F32, tag='psC', name='psC')
                    for j in range(gi + 1):
                        psS = ps_s.tile([128, 128], F32, tag='psS', name='psS')
                        nc.tensor.matmul(psS[:], KT[kvh][:, bass.ts(j, 128)],
                                         QT[h][:, bass.ts(bi, 128)],
                                         start=True, stop=True)
                        ex = p_exp.tile([128, 128], BF16, tag='expb', name='expb')
                        nc.scalar.activation(ex[:], psS[:], AF.Exp,
                                             scale=sclK[:, j, kvh:kvh + 1])
                        if j >= gi - 1:  # possible diagonal/overhang: apply mask
                            nc.vector.tensor_mul(ex[:], ex[:], dm_s[:, bi, j - (gi - 1), :])
                        nc.tensor.matmul(psC[:, 0:129], ex[:], VA[kvh][:, j, 0:129],
                                         start=(j == 0), stop=(j == gi))
                    rd = p_s.tile([128, 1], F32, tag='rd', name='rd')
                    nc.vector.reciprocal(rd[:], psC[:, 128:129])
                    cn = p_w.tile([128, 128], BF16, tag='cn', name='cn')
                    nc.vector.tensor_scalar_mul(cn[:], psC[:, 0:128], rd[:])
                    pst = ps_t.tile([128, 128], BF16, tag='psT', name='psT')
                    nc.tensor.transpose(pst[:], cn[:], iden_s[:])
                    nc.scalar.copy(ctm[h][:, bass.ts(bi, 128)], pst[:])

            wo_s = [p_ht.tile([128, 1024], BF16, tag='ht', name='wo') for _ in range(16)]
            for ch in range(16):
                nc.gpsimd.dma_start(wo_s[ch][:], owT[bass.ts(ch, 128), :])

            # ---- pair AllGather of ctx^T ----
            cc_in = p_d.tile([1024, 1024], BF16)
            cc_out = p_d.tile([2048, 1024], BF16)
            for h in range(8):
                nc.sync.dma_start(cc_in[bass.ts(h, 128), :], ctm[h][:])
            nc.gpsimd.collective_compute(
                'AllGather', mybir.AluOpType.bypass,
                replica_groups=[[0, 1], [2, 3], [4, 5], [6, 7]],
                ins=[cc_in.opt()], outs=[cc_out.opt()])
            ctf = [p_wq.tile([128, 1024], BF16, tag='wq', name='ctf') for _ in range(16)]
            for ch in range(16):
                nc.sync.dma_start(ctf[ch][:], cc_out[bass.ts(ch, 128), :])

            # ---- o_proj (my ho half columns) ----
            for bi in range(8):
                for nt in range(2):
                    psO = ps_a.tile([128, 512], F32, tag='psA', name='psA')
                    for ch in range(16):
                        nc.tensor.matmul(psO[:], ctf[ch][:, bass.ts(bi, 128)],
                                         wo_s[ch][:, bass.ts(nt, 512)],
                                         start=(ch == 0), stop=(ch == 15))
                    ob = p_out.tile([128, 512], F32, tag='outb', name='outb')
                    nc.scalar.copy(ob[:], psO[:])
                    nc.sync.dma_start(out_e[bass.ts(bi, 128), bass.ts(nt, 512)], ob[:])

    split_multi_waits(nc)
    return nc


def mul_b(nc, out, a, b):
    """tensor_tensor multiply with free-dim broadcast of b over dim 1."""
    a2, b2 = bass.broadcast_tensor_aps(a, b)
    nc.vector.tensor_mul(out, a2, b2)


# ---------------------------------------------------------------------------
_NC_CACHE = None
_LAST_IN_MAPS = None


def _get_nc():
    global _NC_CACHE
    if _NC_CACHE is None:
        _NC_CACHE = build_kernel()
    return _NC_CACHE


def kernel(hidden_states, cos, sin, q_w, k_w, v_w, o_w, q_norm_w, k_norm_w):
    from concourse.bass_utils import run_bass_kernel_spmd

    hidden_states = np.asarray(hidden_states, np.float32)
    cos = np.asarray(cos, np.float32)
    sin = np.asarray(sin, np.float32)
    q_w = np.asarray(q_w, np.float32)
    k_w = np.asarray(k_w, np.float32)
    v_w = np.asarray(v_w, np.float32)
    o_w = np.asarray(o_w, np.float32)
    q_norm_w = np.asarray(q_norm_w, np.float32)
    k_norm_w = np.asarray(k_norm_w, np.float32)

    tri_np = np.triu(np.ones((128, 128), np.float32))  # [sj,si]: valid sj<=si
    iden_np = np.eye(128, dtype=np.float32)

    def rope_tabs(c, s_, w):
        # tables [rows, 4, 64]: A=c_lo*w_lo, B=s_lo*w_hi, C=c_lo*w_hi, D=s_lo*w_lo
        cl, sl = c[:, 0:64], s_[:, 0:64]
        wl, wh = w[0:64], w[64:128]
        return np.stack([cl * wl, sl * wh, cl * wh, sl * wl], axis=1).astype(np.float32)

    bounds = [max(MYBLKS[0][bi], MYBLKS[1][bi]) for bi in range(8)]

    in_maps = []
    for c in range(8):
        b, sh, hh = c >> 2, (c >> 1) & 1, c & 1
        blks = MYBLKS[sh]
        rows = np.concatenate([np.arange(g * 128, (g + 1) * 128) for g in blks])
        hT = np.ascontiguousarray(hidden_states[b].T)
        hTq = np.ascontiguousarray(hidden_states[b][rows].T)
        qwT = np.ascontiguousarray(q_w[hh * 1024:(hh + 1) * 1024].T)
        kwT = np.ascontiguousarray(k_w[hh * 512:(hh + 1) * 512].T)
        vwT = np.ascontiguousarray(v_w[hh * 512:(hh + 1) * 512].T)
        owT = np.ascontiguousarray(o_w[hh * 1024:(hh + 1) * 1024].T)
        qtab = rope_tabs(cos[b][rows], sin[b][rows], q_norm_w)
        ktab = rope_tabs(cos[b], sin[b], k_norm_w)
        # diagonal masks dm[bi, t]: t=0 -> sj block gi-1, t=1 -> sj block gi
        # my true causal diagonal is at block g=blks[bi] (<= bounds[bi]).
        dm = np.zeros((8, 2, 128, 128), np.float32)
        for bi in range(8):
            g, gb = blks[bi], bounds[bi]
            for t, j in enumerate((gb - 1, gb)):
                if j < 0:
                    continue
                if j < g:
                    dm[bi, t] = 1.0
                elif j == g:
                    dm[bi, t] = tri_np
                # j > g: stays 0 (block fully masked)
        in_maps.append(dict(
            hT=hT, hTq=hTq, qwT=qwT, kwT=kwT, vwT=vwT, owT=owT,
            qtab=qtab, ktab=ktab, tri=tri_np, iden=iden_np, dm=dm))

    global _LAST_IN_MAPS
    _LAST_IN_MAPS = in_maps
    nc = _get_nc()
    res = run_bass_kernel_spmd(nc, in_maps, core_ids=list(range(8)))

    out = np.zeros((B, S, HID), np.float32)
    for c in range(8):
        b, sh, hh = c >> 2, (c >> 1) & 1, c & 1
        o = res.results[c]['out']  # [1024, 1024]
        for bi, g in enumerate(MYBLKS[sh]):
            out[b, g * 128:(g + 1) * 128, hh * 1024:(hh + 1) * 1024] = \
                o[bi * 128:(bi + 1) * 128]
    return out


if __name__ == '__main__':
    sys.path.insert(0, '/root/problem')
    import reference
    inputs = {k: np.asarray(v) for k, v in reference.setup_inputs().items()}
    exp = np.asarray(reference.reference(**inputs))
    act = kernel(**inputs)
    err = np.abs(act - exp)
    rel = np.linalg.norm(act - exp) / np.linalg.norm(exp)
    print('Relative error:', rel, 'max abs err:', err.max())



# revision 11
# speedup vs baseline: 93146.2374x; 78584.3879x over previous
"""Trainium2 Bass kernel for Qwen-style GQA attention (B=2,S=2048,H=16,KV=8,D=128).

Sharding (8 cores): batch(2) x si-stripes(2) x head-half(2), uniform SPMD
program (all per-core variation flows through host-prepared inputs).
  core c: b=c>>2, sh=(c>>1)&1, hh=c&1
  stripes: MYBLKS[sh] — causally balanced interleaved si blocks.

vs the original baseline:
  - attention processes si in two groups of 4 local blocks with [128,512]-wide
    exp instructions (uniform j bounds 0..7 / 0..15; host masks zero the
    causal overhang), cutting Act-engine time ~30%
  - PSUM->SBUF copies move off the Act engine to Pool/DVE (bf16 2x modes)
  - RMSNorm squares read PSUM directly; sqrt batched into two act-table eras
    so exp/sqrt table thrash is bounded at 4 loads
  - all big inputs arrive bf16 (half the HBM traffic of f32)
  - ctx exchange is two staged pair-AllGathers (heads 0-3 overlap the
    attention of heads 4-7); o_proj pipelines cc1-half / cc2-half chunks
  - o_proj results DMA straight from PSUM
"""
import sys

sys.path.insert(0, '/opt/trn_rl_repo')

import numpy as np

import concourse.bass as bass
import concourse.tile as tile
from concourse import mybir
from concourse.vector_clock import ScopedClock, VectorClock

B, S, HID = 2, 2048, 2048
H, KV, D = 16, 8, 128
EPS = 1e-6
NBLK = S // 16  # noqa
# causally balanced si-block stripes: sum(blk+1) = 68 for both
MYBLKS = [[0, 2, 4, 6, 9, 11, 13, 15], [1, 3, 5, 7, 8, 10, 12, 14]]
JMAX = [7, 15]  # uniform j bound per si group (max over stripes)

F32 = mybir.dt.float32
BF16 = mybir.dt.bfloat16
AF = mybir.ActivationFunctionType


# ---------------------------------------------------------------------------
# Workarounds: this walrus supports only ONE sync-wait per instruction.
def _patched_drain_and_barrier(self, tick_clock, wait_clock):
    gc = tick_clock.global_clock
    vec = list(gc)
    nz = [i for i, v in enumerate(vec) if v > 0] or [0]
    for i in nz:
        cvec = [vec[j] if j == i else 0 for j in range(len(vec))]
        inst = self.nc.sync.drain()
        wait_clock.add_sem_waits(inst.ins, ScopedClock({None: VectorClock(cvec)}))
    self.nc.all_engine_barrier()
    assert self.sems is not None
    popped = self.nc._tile_sem_poison_stack.pop()
    assert popped is self._sem_poison
    self.nc.clear_and_free_semaphores(list(self.sems.allocated().values()))
    self.nc.all_engine_barrier()


tile.TileContext._drain_and_barrier = _patched_drain_and_barrier


def split_multi_waits(nc):
    for fn in nc.m.functions:
        for blk in fn.blocks:
            insts = list(blk.instructions)
            out = []
            changed = False
            for inst in insts:
                si = inst.sync_info
                if si is not None and len(si.on_wait) > 1:
                    waits = list(si.on_wait)
                    for k, w in enumerate(waits[:-1]):
                        out.append(mybir.InstNoOp(
                            name=f"{inst.name}.w{k}", engine=inst.engine,
                            sync_info=mybir.SyncInfo(on_wait=[w], on_update=[]),
                            text_hint="waitsplit"))
                    si.on_wait = [waits[-1]]
                    changed = True
                out.append(inst)
            if changed:
                blk.instructions[:] = out


def mul_b(eng, out, a, b):
    a2, b2 = bass.broadcast_tensor_aps(a, b)
    eng.tensor_mul(out, a2, b2)


# ---------------------------------------------------------------------------
def build_kernel():
    nc = bass.Bass(trn_type='TRN2')
    hT = nc.dram_tensor('hT', [HID, S], BF16, kind='ExternalInput')
    hTq = nc.dram_tensor('hTq', [HID, 1024], BF16, kind='ExternalInput')
    qwT = nc.dram_tensor('qwT', [HID, 1024], BF16, kind='ExternalInput')
    kwT = nc.dram_tensor('kwT', [HID, 512], BF16, kind='ExternalInput')
    vwT = nc.dram_tensor('vwT', [HID, 512], BF16, kind='ExternalInput')
    owT = nc.dram_tensor('owT', [2048, 1024], BF16, kind='ExternalInput')
    qtab = nc.dram_tensor('qtab', [1024, 4, 64], BF16, kind='ExternalInput')
    ktab = nc.dram_tensor('ktab', [S, 4, 64], BF16, kind='ExternalInput')
    mskin = nc.dram_tensor('mskin', [16, 128, 512], BF16, kind='ExternalInput')
    iden = nc.dram_tensor('iden', [128, 128], BF16, kind='ExternalInput')
    myrows = nc.dram_tensor('myrows', [8, 1], mybir.dt.uint32,
                            kind='ExternalInput')
    prows = nc.dram_tensor('prows', [8, 1], mybir.dt.uint32,
                           kind='ExternalInput')
    out_e = nc.dram_tensor('out', [1024, 1024], F32, kind='ExternalOutput')
    # ctx^T exchange buffer in the pair-shared HBM domain: rows hh*1024+h*128
    shctx = nc.dram_tensor('shctx', [2048, 1024], BF16, kind='Internal',
                           addr_space='Shared')
    bar_in = nc.dram_tensor('bar_in', [1, 1], BF16, kind='Internal')
    bar_out = nc.dram_tensor('bar_out', [2, 1], BF16, kind='Internal')
    bar_in2 = nc.dram_tensor('bar_in2', [1, 1], BF16, kind='Internal')
    bar_out2 = nc.dram_tensor('bar_out2', [2, 1], BF16, kind='Internal')

    from contextlib import ExitStack
    with ExitStack() as ctx:
        tc = ctx.enter_context(tile.TileContext(nc))
        pool = lambda name, bufs, **kw: ctx.enter_context(
            tc.tile_pool(name=name, bufs=bufs, **kw))
        p_wq = pool('wq', 16)     # wq tiles; ctf chunks reuse
        p_wk = pool('wk', 16)
        p_wv = pool('wv', 16)
        p_ht = pool('ht', 16)     # ht pass0 -> hTq -> ht pass1 -> wo
        p_big = pool('big', 1)    # KT/VA/QT/ctm/masks persistent
        p_c = pool('const', 1)
        p_tab = pool('tab', 4)    # streamed rope tables
        p_hq = pool('hq', 24)     # streamed hTq chunks (16 per si block)
        p_ex = pool('expb', 3)
        p_cp = pool('cpb', 4)
        p_ro = pool('rope', 12)   # rope outputs (8 can be stashed)
        p_t12 = pool('t12', 4)
        p_ob = pool('outb', 2)
        p_s = pool('small', 20)
        p_scl = pool('scl', 1)
        p_rd = pool('rd', 2)
        p_d = pool('dump', 1)
        ps_a = pool('psA', 2, space='PSUM')
        ps_s = pool('psS', 2, space='PSUM')
        ps_c = pool('psC', 4, space='PSUM')
        p_dram = pool('dram', 8, space='DRAM')

        # ---- constants ----
        p_msk = pool('msk', 1)
        iden_s = p_c.tile([128, 128], BF16)
        nc.gpsimd.dma_start(iden_s[:], iden[:])

        # persistent activations
        KT = p_big.tile([128, 4, 2048], BF16, name='KT')     # [d, kvh, sj]
        VA = p_big.tile([128, 4, 16, 132], BF16, name='VA')  # [sj, kvh, sb, d|1]
        QT = p_big.tile([128, 8, 1024], BF16, name='QT')     # [d, h, si-local]
        ctm = p_big.tile([128, 8, 1024], BF16, name='ctm')   # [d, h, si-local]
        sclK = p_scl.tile([128, 16, 4], F32, tag='sclK', name='sclK')
        nc.gpsimd.memset(VA[:, :, :, 128:129], 1.0)
        epsK = p_c.tile([128, 1], F32)
        nc.gpsimd.memset(epsK[:], float(D * EPS))
        epsQ = p_c.tile([128, 1], F32)
        nc.gpsimd.memset(epsQ[:], float(EPS))

        # ---- weights + first ht pass (pass-0 critical loads first) ----
        wq_s = [p_wq.tile([128, 1024], BF16, tag='wq', name='wq') for _ in range(16)]
        wk_s = [p_wk.tile([128, 512], BF16, tag='wk', name='wk') for _ in range(16)]
        wv_s = [p_wv.tile([128, 512], BF16, tag='wv', name='wv') for _ in range(16)]
        ht_t = [p_ht.tile([128, 1024], BF16, tag='ht', name='ht0')
                for _ in range(16)]
        for ch in range(16):
            r = bass.ts(ch, 128)
            nc.gpsimd.dma_start(ht_t[ch][:], hT[r, 0:1024])
            nc.gpsimd.dma_start(wv_s[ch][:], vwT[r, :])
            nc.gpsimd.dma_start(wk_s[ch][:], kwT[r, :])
        for ch in range(16):
            nc.gpsimd.dma_start(wq_s[ch][:], qwT[bass.ts(ch, 128), :])

        dump = p_d.tile([128, 128], F32, name='dump')
        rows_s = p_c.tile([8, 2], mybir.dt.uint32, name='rows_s')
        nc.gpsimd.dma_start(rows_s[:, 0:1], myrows[:])
        nc.gpsimd.dma_start(rows_s[:, 1:2], prows[:])

        # ------- helpers -------
        def rope4(cpb, tab):
            """RoPE for 4 heads packed [128, 4*128] bf16 -> new bf16 tile."""
            lo = cpb[:].rearrange('p (t d) -> p t d', t=4)[:, :, 0:64]
            hi = cpb[:].rearrange('p (t d) -> p t d', t=4)[:, :, 64:128]
            ro = p_ro.tile([128, 512], BF16, tag='ro', name='ro')
            rov = ro[:].rearrange('p (t d) -> p t d', t=4)
            t1 = p_t12.tile([128, 4, 64], BF16, tag='t12', name='t1')
            t2 = p_t12.tile([128, 4, 64], BF16, tag='t12', name='t2')
            mul_b(nc.vector, t1[:], lo, tab[:, 0:1, :])
            mul_b(nc.vector, t2[:], hi, tab[:, 1:2, :])
            nc.vector.tensor_sub(rov[:, :, 0:64], t1[:], t2[:])
            mul_b(nc.vector, t1[:], hi, tab[:, 2:3, :])
            mul_b(nc.vector, t2[:], lo, tab[:, 3:4, :])
            nc.vector.tensor_add(rov[:, :, 64:128], t1[:], t2[:])
            return ro

        def kv_unit(sb, ht_t, col, do_scl):
            """V+K projection for global sj block sb from pass tiles ht_t
            (col = 128-col offset in the pass window)."""
            sslice = bass.ts(col, 128)
            psV = ps_a.tile([128, 512], F32, tag='psA', name='psV')
            for ch in range(16):
                nc.tensor.matmul(psV[:], ht_t[ch][:, sslice], wv_s[ch][:],
                                 start=(ch == 0), stop=(ch == 15))
            nc.scalar.copy(VA[:, :, sb, 0:128],
                           psV[:].rearrange('p (t d) -> p t d', t=4))
            psK = ps_a.tile([128, 512], F32, tag='psA', name='psK')
            for ch in range(16):
                nc.tensor.matmul(psK[:], ht_t[ch][:, sslice], wk_s[ch][:],
                                 start=(ch == 0), stop=(ch == 15))
            kcpb = p_cp.tile([128, 512], BF16, tag='cp', name='kcpb')
            nc.vector.tensor_copy(kcpb[:], psK[:])
            ss = p_s.tile([128, 4], F32, tag='ss', name='ssk')
            for kvh in range(4):
                nc.scalar.activation(dump[:], kcpb[:, bass.ts(kvh, 128)],
                                     AF.Square, accum_out=ss[:, kvh:kvh + 1])
            ktb = p_tab.tile([128, 4, 64], BF16, tag='ktab', name='ktb')
            nc.gpsimd.dma_start(ktb[:], ktab[sb * 128:(sb + 1) * 128])
            kro = rope4(kcpb, ktb)
            krov = kro[:].rearrange('p (t d) -> p t d', t=4)
            for kvh in range(4):
                pst = ps_s.tile([128, 128], BF16, tag='psS', name='psT')
                nc.tensor.transpose(pst[:], krov[:, kvh, :], iden_s[:])
                nc.scalar.copy(KT[:, kvh, bass.ts(sb, 128)], pst[:])
            if do_scl:
                scl_finish(sb, ss)
            return ss

        def scl_finish(sb, ss):
            # SCALE*rstd folded exactly: 1/sqrt(ss + D*eps)
            std = p_s.tile([128, 4], F32, tag='std', name='std')
            nc.scalar.activation(std[:], ss[:], AF.Sqrt, bias=epsK[:])
            nc.vector.reciprocal(sclK[:, sb, :], std[:])

        def load_hq(l):
            hq = [p_hq.tile([128, 128], BF16, tag='hq', name='hq')
                  for _ in range(16)]
            for ch in range(16):
                nc.gpsimd.dma_start(hq[ch][:],
                                    hTq[bass.ts(ch, 128), bass.ts(l, 128)])
            return hq

        def q_unit(l, qg, hq):
            """Q proj+square+rope for local block l, head group qg."""
            psQ = ps_a.tile([128, 512], F32, tag='psA', name='psQ')
            for ch in range(16):
                nc.tensor.matmul(psQ[:], hq[ch][:],
                                 wq_s[ch][:, bass.ts(qg, 512)],
                                 start=(ch == 0), stop=(ch == 15))
            qcpb = p_cp.tile([128, 512], BF16, tag='cp', name='qcpb')
            nc.vector.tensor_copy(qcpb[:], psQ[:])
            ss = p_s.tile([128, 4], F32, tag='ss', name='ssq')
            for hq in range(4):
                nc.scalar.activation(dump[:], qcpb[:, bass.ts(hq, 128)],
                                     AF.Square, accum_out=ss[:, hq:hq + 1])
            qtb = p_tab.tile([128, 4, 64], BF16, tag='qtab', name='qtb')
            nc.gpsimd.dma_start(qtb[:], qtab[l * 128:(l + 1) * 128])
            qro = rope4(qcpb, qtb)
            return ss, qro

        def q_finish(l, qg, ss, qro):
            """rstd -> scale -> transpose into QT."""
            std = p_s.tile([128, 4], F32, tag='std', name='stdq')
            nc.scalar.activation(std[:], ss[:], AF.Sqrt, scale=1.0 / D,
                                 bias=epsQ[:])
            rstd = p_s.tile([128, 4], F32, tag='rstd', name='rstdq')
            nc.vector.reciprocal(rstd[:], std[:])
            qrov = qro[:].rearrange('p (t d) -> p t d', t=4)
            qn = p_ro.tile([128, 512], BF16, tag='ro', name='qn')
            qnv = qn[:].rearrange('p (t d) -> p t d', t=4)
            for hq in range(4):
                nc.vector.tensor_scalar_mul(qnv[:, hq, :], qrov[:, hq, :],
                                            rstd[:, hq:hq + 1])
                pst = ps_s.tile([128, 128], BF16, tag='psS', name='psT')
                nc.tensor.transpose(pst[:], qnv[:, hq, :], iden_s[:])
                nc.vector.tensor_copy(QT[:, qg * 4 + hq, bass.ts(l, 128)],
                                      pst[:])

        def attn_group(h, g, msk_t):
            """Attention for head h on si group g (local blocks 4g..4g+3).
            In G1 the proj pools are idle, so QK tiles also rotate through
            psA for a 4-deep pipeline."""
            kvh = h // 2
            jmax = JMAX[g]
            silo = bass.ts(g, 512)
            psCs = [ps_c.tile([128, 132], F32, tag='psC', name='psC')
                    for _ in range(4)]
            for j in range(jmax + 1):
                if g == 1 and j % 2 == 1:
                    psS = ps_a.tile([128, 512], F32, tag='psA', name='psS')
                else:
                    psS = ps_s.tile([128, 512], F32, tag='psS', name='psS')
                nc.tensor.matmul(psS[:], KT[:, kvh, bass.ts(j, 128)],
                                 QT[:, h, silo], start=True, stop=True)
                ex = p_ex.tile([128, 512], BF16, tag='ex', name='ex')
                nc.scalar.activation(ex[:], psS[:], AF.Exp,
                                     scale=sclK[:, j, kvh:kvh + 1])
                if g == 0 or j >= 8:  # host mask handles diagonal + overhang
                    nc.vector.tensor_mul(ex[:], ex[:], msk_t[:, j - 8 * g, :])
                for s in range(4):
                    nc.tensor.matmul(psCs[s][:, 0:129],
                                     ex[:, bass.ts(s, 128)],
                                     VA[:, kvh, j, 0:129],
                                     start=(j == 0), stop=(j == jmax))
            for s in range(4):
                rd = p_rd.tile([128, 1], F32, tag='rd', name='rd')
                nc.vector.reciprocal(rd[:], psCs[s][:, 128:129])
                cn = p_ex.tile([128, 128], BF16, tag='cn', name='cn')
                nc.vector.tensor_scalar_mul(cn[:], psCs[s][:, 0:128], rd[:])
                pst = ps_s.tile([128, 128], BF16, tag='psS', name='psT')
                nc.tensor.transpose(pst[:], cn[:], iden_s[:])
                nc.vector.tensor_copy(ctm[:, h, bass.ts(4 * g + s, 128)],
                                      pst[:])

        # ------- pass 0: K/V proj for global sj blocks 0..7 -------
        for sb in range(8):
            kv_unit(sb, ht_t, sb, do_scl=True)
        msk_s0 = p_msk.tile([128, 8, 512], BF16, tag='msk', name='msk0')
        nc.gpsimd.dma_start(
            msk_s0[:], mskin[0:8].rearrange('t p d -> p t d'))

        # ------- Q proj for local blocks 0..3 (needed by G0) -------
        for l in range(4):
            hq = load_hq(l)
            for qg in range(2):
                ss, qro = q_unit(l, qg, hq)
                q_finish(l, qg, ss, qro)

        # ------- G0 attention interleaved with pass 1 K/V proj ------
        ht_t2 = [p_ht.tile([128, 1024], BF16, tag='ht', name='ht1')
                 for _ in range(16)]
        for ch in range(16):
            nc.gpsimd.dma_start(ht_t2[ch][:], hT[bass.ts(ch, 128), 1024:2048])
        p1_ss = {}
        q_stash = {}
        hq_cur = [None]
        for h in range(8):
            attn_group(h, 0, msk_s0)
            sb = 8 + h
            p1_ss[sb] = kv_unit(sb, ht_t2, sb - 8, do_scl=False)
            # Q proj for local blocks 4..7 (squares now, sqrts batched later)
            l, qg = 4 + h // 2, h % 2
            if qg == 0:
                hq_cur[0] = load_hq(l)
            q_stash[(l, qg)] = q_unit(l, qg, hq_cur[0])

        # ------- batched pass-1 norms (single sqrt-table era) -------
        for sb in range(8, 16):
            scl_finish(sb, p1_ss[sb])
        for (l, qg), (ss, qro) in q_stash.items():
            q_finish(l, qg, ss, qro)

        # ------- wo loads (reuse ht pool) -------
        wo_s = [p_ht.tile([128, 1024], BF16, tag='ht', name='wo')
                for _ in range(16)]
        for ch in range(16):
            nc.gpsimd.dma_start(wo_s[ch][:], owT[bass.ts(ch, 128), :])

        # ------- G1 attention + shared-HBM ctx export -------
        msk_s1 = p_msk.tile([128, 8, 512], BF16, tag='msk', name='msk1')
        nc.gpsimd.dma_start(
            msk_s1[:], mskin[8:16].rearrange('t p d -> p t d'))
        for h in range(8):
            attn_group(h, 1, msk_s1)
            rtmp = nc.gpsimd.alloc_register(f'myrow{h}')
            nc.gpsimd.reg_load(rtmp, rows_s[h:h + 1, 0:1])
            rrow = nc.gpsimd.snap(rtmp, donate=True, min_val=0, max_val=1920)
            nc.gpsimd.dma_start(shctx[bass.ds(rrow, 128), :], ctm[:, h, :])
            if h == 6:  # barrier 1: peer heads 0..6
                nc.gpsimd.dma_start(bar_in[0:1, 0:1], shctx[0:1, 0:1])
                nc.gpsimd.collective_compute(
                    'AllGather', mybir.AluOpType.bypass,
                    replica_groups=[[0, 1], [2, 3], [4, 5], [6, 7]],
                    ins=[bar_in[:].opt()], outs=[bar_out[:].opt()])

        # barrier 2 covers head 7 (barrier 1 was issued inside the G1 loop
        # after head 6's export; pokes read shctx so the RAW deps order each
        # barrier after the exports emitted before it).
        nc.gpsimd.dma_start(bar_in2[0:1, 0:1], shctx[0:1, 0:1])
        nc.gpsimd.collective_compute(
            'AllGather', mybir.AluOpType.bypass,
            replica_groups=[[0, 1], [2, 3], [4, 5], [6, 7]],
            ins=[bar_in2[:].opt()], outs=[bar_out2[:].opt()])
        ctf = [p_wq.tile([128, 1024], BF16, tag='wq', name='ctf')
               for _ in range(8)]
        for i in range(8):
            # corner poke: WAW dep orders the peer read after its barrier
            bo = bar_out if i < 7 else bar_out2
            nc.sync.dma_start(ctf[i][0:1, 0:1], bo[0:1, 0:1])
            ptmp = nc.sync.alloc_register(f'prow{i}')
            nc.sync.reg_load(ptmp, rows_s[i:i + 1, 1:2])
            prow = nc.sync.snap(ptmp, donate=True, min_val=0, max_val=1920)
            nc.sync.dma_start(ctf[i][:], shctx[bass.ds(prow, 128), :])

        # ------- o_proj: local-chunk halves lead, peer halves pipelined ----
        # owT is host-permuted to local-first chunk order, so rows 0..7 pair
        # with ctm heads and 8..15 with peer ctf chunks — uniform program.
        units = [(bi, nt) for bi in range(8) for nt in range(2)]

        oslots = [(ps_a, 'psA'), (ps_a, 'psA'), (ps_s, 'psS'), (ps_s, 'psS')]

        def o_first(u):
            bi, nt = units[u]
            pool_u, tag_u = oslots[u % 4]
            psO = pool_u.tile([128, 512], F32, tag=tag_u, name='psO')
            for i in range(8):
                nc.tensor.matmul(psO[:], ctm[:, i, bass.ts(bi, 128)],
                                 wo_s[i][:, bass.ts(nt, 512)],
                                 start=(i == 0), stop=False)
            return psO

        def o_second(u, psO):
            bi, nt = units[u]
            for i in range(8):
                nc.tensor.matmul(psO[:], ctf[i][:, bass.ts(bi, 128)],
                                 wo_s[8 + i][:, bass.ts(nt, 512)],
                                 start=False, stop=(i == 7))
            ob = p_ob.tile([128, 512], F32, tag='ob', name='ob')
            nc.vector.tensor_copy(ob[:], psO[:])
            nc.gpsimd.dma_start(
                out_e[bass.ts(bi, 128), bass.ts(nt, 512)], ob[:])

        live = []
        for u in range(16):
            live.append((u, o_first(u)))
            if len(live) == 4:
                v, psO = live.pop(0)
                o_second(v, psO)
        for v, psO in live:
            o_second(v, psO)

    split_multi_waits(nc)
    return nc


# ---------------------------------------------------------------------------
_NC_CACHE = None
_LAST_IN_MAPS = None


def _get_nc():
    global _NC_CACHE
    if _NC_CACHE is None:
        _NC_CACHE = build_kernel()
    return _NC_CACHE


def make_in_maps(hidden_states, cos, sin, q_w, k_w, v_w, o_w, q_norm_w, k_norm_w):
    import ml_dtypes
    bf16 = ml_dtypes.bfloat16

    hidden_states = np.asarray(hidden_states, np.float32)
    cos = np.asarray(cos, np.float32)
    sin = np.asarray(sin, np.float32)
    q_w = np.asarray(q_w, np.float32)
    k_w = np.asarray(k_w, np.float32)
    v_w = np.asarray(v_w, np.float32)
    o_w = np.asarray(o_w, np.float32)
    q_norm_w = np.asarray(q_norm_w, np.float32)
    k_norm_w = np.asarray(k_norm_w, np.float32)

    tri_np = np.triu(np.ones((128, 128), np.float32))  # [sj,si]: valid sj<=si
    iden_np = np.eye(128, dtype=np.float32).astype(bf16)

    def rope_tabs(c, s_, w):
        cl, sl = c[:, 0:64], s_[:, 0:64]
        wl, wh = w[0:64], w[64:128]
        return np.stack([cl * wl, sl * wh, cl * wh, sl * wl], axis=1).astype(bf16)

    in_maps = []
    for c in range(8):
        b, sh, hh = c >> 2, (c >> 1) & 1, c & 1
        blks = MYBLKS[sh]
        rows = np.concatenate([np.arange(g * 128, (g + 1) * 128) for g in blks])
        # o_w contraction rows permuted local-first: my hh half then peer half
        operm = np.concatenate([
            np.arange(hh * 1024, (hh + 1) * 1024),
            np.arange((1 - hh) * 1024, (2 - hh) * 1024)])
        myrows = ((hh * 8 + np.arange(8)) * 128).astype(np.uint32)[:, None]
        prows = (((1 - hh) * 8 + np.arange(8)) * 128).astype(np.uint32)[:, None]
        # masks indexed by global sj block j: j<8 -> si group 0 (locals 0..3),
        # j>=8 -> group 1 (locals 4..7). ones below diag, tri on diag, zero
        # above.
        msk = np.zeros((16, 128, 512), np.float32)
        for j in range(16):
            loc = range(4) if j < 8 else range(4, 8)
            for s_i, l in enumerate(loc):
                g_s = blks[l]
                if j < g_s:
                    msk[j, :, s_i * 128:(s_i + 1) * 128] = 1.0
                elif j == g_s:
                    msk[j, :, s_i * 128:(s_i + 1) * 128] = tri_np
        in_maps.append(dict(
            hT=np.ascontiguousarray(hidden_states[b].T).astype(bf16),
            hTq=np.ascontiguousarray(hidden_states[b][rows].T).astype(bf16),
            qwT=np.ascontiguousarray(q_w[hh * 1024:(hh + 1) * 1024].T).astype(bf16),
            kwT=np.ascontiguousarray(k_w[hh * 512:(hh + 1) * 512].T).astype(bf16),
            vwT=np.ascontiguousarray(v_w[hh * 512:(hh + 1) * 512].T).astype(bf16),
            owT=np.ascontiguousarray(
                o_w[hh * 1024:(hh + 1) * 1024].T[operm]).astype(bf16),
            qtab=rope_tabs(cos[b][rows], sin[b][rows], q_norm_w),
            ktab=rope_tabs(cos[b], sin[b], k_norm_w),
            mskin=msk.astype(bf16), iden=iden_np,
            myrows=myrows, prows=prows))
    return in_maps


def gather_out(outs):
    """outs: list of 8 per-core 'out' arrays -> full [B,S,HID]."""
    out = np.zeros((B, S, HID), np.float32)
    for c in range(8):
        b, sh, hh = c >> 2, (c >> 1) & 1, c & 1
        o = np.asarray(outs[c], np.float32)  # [1024, 1024]
        for l, g in enumerate(MYBLKS[sh]):
            out[b, g * 128:(g + 1) * 128, hh * 1024:(hh + 1) * 1024] = \
                o[l * 128:(l + 1) * 128]
    return out


def kernel(hidden_states, cos, sin, q_w, k_w, v_w, o_w, q_norm_w, k_norm_w):
    from concourse.bass_utils import run_bass_kernel_spmd

    in_maps = make_in_maps(hidden_states, cos, sin, q_w, k_w, v_w, o_w,
                           q_norm_w, k_norm_w)
    global _LAST_IN_MAPS
    _LAST_IN_MAPS = in_maps
    nc = _get_nc()
    res = run_bass_kernel_spmd(nc, in_maps, core_ids=list(range(8)))
    return gather_out([res.results[c]['out'] for c in range(8)])


if __name__ == '__main__':
    sys.path.insert(0, '/root/problem')
    import reference
    inputs = {k: np.asarray(v) for k, v in reference.setup_inputs().items()}
    exp = np.asarray(reference.reference(**inputs))
    act = kernel(**inputs)
    rel = np.linalg.norm(act - exp) / np.linalg.norm(exp)
    print('Relative error:', rel)


# revision 19
# speedup vs baseline: 93422.7465x; 1.0030x over previous
"""Trainium2 Bass kernel for Qwen-style GQA attention (B=2,S=2048,H=16,KV=8,D=128).

Sharding (8 cores): batch(2) x si-stripes(2) x head-half(2), uniform SPMD
program (all per-core variation flows through host-prepared inputs).
  core c: b=c>>2, sh=(c>>1)&1, hh=c&1
  stripes: MYBLKS[sh] — causally balanced interleaved si blocks.

vs the original baseline:
  - attention processes si in two groups of 4 local blocks with [128,512]-wide
    exp instructions (uniform j bounds 0..7 / 0..15; host masks zero the
    causal overhang), cutting Act-engine time ~30%
  - PSUM->SBUF copies move off the Act engine to Pool/DVE (bf16 2x modes)
  - RMSNorm squares read PSUM directly; sqrt batched into two act-table eras
    so exp/sqrt table thrash is bounded at 4 loads
  - all big inputs arrive bf16 (half the HBM traffic of f32)
  - ctx exchange is two staged pair-AllGathers (heads 0-3 overlap the
    attention of heads 4-7); o_proj pipelines cc1-half / cc2-half chunks
  - o_proj results DMA straight from PSUM
"""
import sys

sys.path.insert(0, '/opt/trn_rl_repo')

import numpy as np

import concourse.bass as bass
import concourse.tile as tile
from concourse import mybir
from concourse.vector_clock import ScopedClock, VectorClock

B, S, HID = 2, 2048, 2048
H, KV, D = 16, 8, 128
EPS = 1e-6
NBLK = S // 16  # noqa
# causally balanced si-block stripes: sum(blk+1) = 68 for both
MYBLKS = [[0, 2, 4, 6, 9, 11, 13, 15], [1, 3, 5, 7, 8, 10, 12, 14]]
JMAX = [7, 15]  # uniform j bound per si group (max over stripes)

F32 = mybir.dt.float32
BF16 = mybir.dt.bfloat16
AF = mybir.ActivationFunctionType


# ---------------------------------------------------------------------------
# Workarounds: this walrus supports only ONE sync-wait per instruction.
def _patched_drain_and_barrier(self, tick_clock, wait_clock):
    gc = tick_clock.global_clock
    vec = list(gc)
    nz = [i for i, v in enumerate(vec) if v > 0] or [0]
    for i in nz:
        cvec = [vec[j] if j == i else 0 for j in range(len(vec))]
        inst = self.nc.sync.drain()
        wait_clock.add_sem_waits(inst.ins, ScopedClock({None: VectorClock(cvec)}))
    self.nc.all_engine_barrier()
    assert self.sems is not None
    popped = self.nc._tile_sem_poison_stack.pop()
    assert popped is self._sem_poison
    self.nc.clear_and_free_semaphores(list(self.sems.allocated().values()))
    self.nc.all_engine_barrier()


tile.TileContext._drain_and_barrier = _patched_drain_and_barrier


def split_multi_waits(nc):
    for fn in nc.m.functions:
        for blk in fn.blocks:
            insts = list(blk.instructions)
            out = []
            changed = False
            for inst in insts:
                si = inst.sync_info
                if si is not None and len(si.on_wait) > 1:
                    waits = list(si.on_wait)
                    for k, w in enumerate(waits[:-1]):
                        out.append(mybir.InstNoOp(
                            name=f"{inst.name}.w{k}", engine=inst.engine,
                            sync_info=mybir.SyncInfo(on_wait=[w], on_update=[]),
                            text_hint="waitsplit"))
                    si.on_wait = [waits[-1]]
                    changed = True
                out.append(inst)
            if changed:
                blk.instructions[:] = out


def mul_b(eng, out, a, b):
    a2, b2 = bass.broadcast_tensor_aps(a, b)
    eng.tensor_mul(out, a2, b2)


# ---------------------------------------------------------------------------
def build_kernel():
    nc = bass.Bass(trn_type='TRN2')
    hT = nc.dram_tensor('hT', [HID, S], BF16, kind='ExternalInput')
    hTq = nc.dram_tensor('hTq', [HID, 1024], BF16, kind='ExternalInput')
    qwT = nc.dram_tensor('qwT', [HID, 1024], BF16, kind='ExternalInput')
    kwT = nc.dram_tensor('kwT', [HID, 512], BF16, kind='ExternalInput')
    vwT = nc.dram_tensor('vwT', [HID, 512], BF16, kind='ExternalInput')
    owT = nc.dram_tensor('owT', [2048, 1024], BF16, kind='ExternalInput')
    qtab = nc.dram_tensor('qtab', [1024, 4, 64], BF16, kind='ExternalInput')
    ktab = nc.dram_tensor('ktab', [S, 4, 64], BF16, kind='ExternalInput')
    mskin = nc.dram_tensor('mskin', [16, 128, 512], BF16, kind='ExternalInput')
    iden = nc.dram_tensor('iden', [128, 128], BF16, kind='ExternalInput')
    myrows = nc.dram_tensor('myrows', [8, 1], mybir.dt.uint32,
                            kind='ExternalInput')
    prows = nc.dram_tensor('prows', [8, 1], mybir.dt.uint32,
                           kind='ExternalInput')
    out_e = nc.dram_tensor('out', [1024, 1024], F32, kind='ExternalOutput')
    # ctx^T exchange buffer in the pair-shared HBM domain: rows hh*1024+h*128
    shctx = nc.dram_tensor('shctx', [2048, 1024], BF16, kind='Internal',
                           addr_space='Shared')
    bar_in = nc.dram_tensor('bar_in', [1, 1], BF16, kind='Internal')
    bar_out = nc.dram_tensor('bar_out', [2, 1], BF16, kind='Internal')
    bar_in2 = nc.dram_tensor('bar_in2', [1, 1], BF16, kind='Internal')
    bar_out2 = nc.dram_tensor('bar_out2', [2, 1], BF16, kind='Internal')

    from contextlib import ExitStack
    with ExitStack() as ctx:
        tc = ctx.enter_context(tile.TileContext(nc))
        pool = lambda name, bufs, **kw: ctx.enter_context(
            tc.tile_pool(name=name, bufs=bufs, **kw))
        p_wq = pool('wq', 16)     # wq tiles; ctf chunks reuse
        p_wk = pool('wk', 16)
        p_wv = pool('wv', 16)
        p_ht = pool('ht', 16)     # ht pass0 -> hTq -> ht pass1 -> wo
        p_big = pool('big', 1)    # KT/VA/QT/ctm/masks persistent
        p_c = pool('const', 1)
        p_tab = pool('tab', 4)    # streamed rope tables
        p_hq = pool('hq', 24)     # streamed hTq chunks (16 per si block)
        p_ex = pool('expb', 3)
        p_cp = pool('cpb', 4)
        p_ro = pool('rope', 12)   # rope outputs (8 can be stashed)
        p_t12 = pool('t12', 4)
        p_ob = pool('outb', 2)
        p_s = pool('small', 20)
        p_scl = pool('scl', 1)
        p_rd = pool('rd', 2)
        p_d = pool('dump', 1)
        ps_a = pool('psA', 2, space='PSUM')
        ps_s = pool('psS', 2, space='PSUM')
        ps_c = pool('psC', 4, space='PSUM')
        p_dram = pool('dram', 8, space='DRAM')

        # ---- constants ----
        p_msk = pool('msk', 1)
        iden_s = p_c.tile([128, 128], BF16)
        nc.gpsimd.dma_start(iden_s[:], iden[:])

        # persistent activations
        KT = p_big.tile([128, 4, 2048], BF16, name='KT')     # [d, kvh, sj]
        VA = p_big.tile([128, 4, 16, 132], BF16, name='VA')  # [sj, kvh, sb, d|1]
        QT = p_big.tile([128, 8, 1024], BF16, name='QT')     # [d, h, si-local]
        ctm = p_big.tile([128, 8, 1024], BF16, name='ctm')   # [d, h, si-local]
        sclK = p_scl.tile([128, 16, 4], F32, tag='sclK', name='sclK')
        nc.gpsimd.memset(VA[:, :, :, 128:129], 1.0)
        epsK = p_c.tile([128, 1], F32)
        nc.gpsimd.memset(epsK[:], float(D * EPS))
        epsQ = p_c.tile([128, 1], F32)
        nc.gpsimd.memset(epsQ[:], float(EPS))

        # ---- weights + first ht pass (pass-0 critical loads first) ----
        wq_s = [p_wq.tile([128, 1024], BF16, tag='wq', name='wq') for _ in range(16)]
        wk_s = [p_wk.tile([128, 512], BF16, tag='wk', name='wk') for _ in range(16)]
        wv_s = [p_wv.tile([128, 512], BF16, tag='wv', name='wv') for _ in range(16)]
        ht_t = [p_ht.tile([128, 1024], BF16, tag='ht', name='ht0')
                for _ in range(16)]
        for ch in range(16):
            r = bass.ts(ch, 128)
            nc.gpsimd.dma_start(ht_t[ch][:], hT[r, 0:1024])
            nc.gpsimd.dma_start(wv_s[ch][:], vwT[r, :])
            nc.gpsimd.dma_start(wk_s[ch][:], kwT[r, :])
        for ch in range(16):
            nc.gpsimd.dma_start(wq_s[ch][:], qwT[bass.ts(ch, 128), :])

        dump = p_d.tile([128, 128], F32, name='dump')
        rows_s = p_c.tile([8, 2], mybir.dt.uint32, name='rows_s')
        nc.gpsimd.dma_start(rows_s[:, 0:1], myrows[:])
        nc.gpsimd.dma_start(rows_s[:, 1:2], prows[:])

        # ------- helpers -------
        def rope4(cpb, tab):
            """RoPE for 4 heads packed [128, 4*128] bf16 -> new bf16 tile."""
            lo = cpb[:].rearrange('p (t d) -> p t d', t=4)[:, :, 0:64]
            hi = cpb[:].rearrange('p (t d) -> p t d', t=4)[:, :, 64:128]
            ro = p_ro.tile([128, 512], BF16, tag='ro', name='ro')
            rov = ro[:].rearrange('p (t d) -> p t d', t=4)
            t1 = p_t12.tile([128, 4, 64], BF16, tag='t12', name='t1')
            t2 = p_t12.tile([128, 4, 64], BF16, tag='t12', name='t2')
            mul_b(nc.vector, t1[:], lo, tab[:, 0:1, :])
            mul_b(nc.vector, t2[:], hi, tab[:, 1:2, :])
            nc.vector.tensor_sub(rov[:, :, 0:64], t1[:], t2[:])
            mul_b(nc.vector, t1[:], hi, tab[:, 2:3, :])
            mul_b(nc.vector, t2[:], lo, tab[:, 3:4, :])
            nc.vector.tensor_add(rov[:, :, 64:128], t1[:], t2[:])
            return ro

        def kv_unit(sb, ht_t, col, do_scl):
            """V+K projection for global sj block sb from pass tiles ht_t
            (col = 128-col offset in the pass window). During pass 0 the
            attention pools are idle, so psV borrows psS to decouple the
            V/K PSUM rotation."""
            sslice = bass.ts(col, 128)
            if sb < 8:
                psV = ps_s.tile([128, 512], F32, tag='psS', name='psV')
            else:
                psV = ps_a.tile([128, 512], F32, tag='psA', name='psV')
            for ch in range(16):
                nc.tensor.matmul(psV[:], ht_t[ch][:, sslice], wv_s[ch][:],
                                 start=(ch == 0), stop=(ch == 15))
            nc.scalar.copy(VA[:, :, sb, 0:128],
                           psV[:].rearrange('p (t d) -> p t d', t=4))
            psK = ps_a.tile([128, 512], F32, tag='psA', name='psK')
            for ch in range(16):
                nc.tensor.matmul(psK[:], ht_t[ch][:, sslice], wk_s[ch][:],
                                 start=(ch == 0), stop=(ch == 15))
            kcpb = p_cp.tile([128, 512], BF16, tag='cp', name='kcpb')
            nc.vector.tensor_copy(kcpb[:], psK[:])
            ss = p_s.tile([128, 4], F32, tag='ss', name='ssk')
            for kvh in range(4):
                nc.scalar.activation(dump[:], kcpb[:, bass.ts(kvh, 128)],
                                     AF.Square, accum_out=ss[:, kvh:kvh + 1])
            ktb = p_tab.tile([128, 4, 64], BF16, tag='ktab', name='ktb')
            nc.gpsimd.dma_start(ktb[:], ktab[sb * 128:(sb + 1) * 128])
            kro = rope4(kcpb, ktb)
            krov = kro[:].rearrange('p (t d) -> p t d', t=4)
            for kvh in range(4):
                pst = ps_s.tile([128, 128], BF16, tag='psS', name='psT')
                nc.tensor.transpose(pst[:], krov[:, kvh, :], iden_s[:])
                nc.scalar.copy(KT[:, kvh, bass.ts(sb, 128)], pst[:])
            if do_scl:
                scl_finish(sb, ss)
            return ss

        def scl_finish(sb, ss):
            # SCALE*rstd folded exactly: 1/sqrt(ss + D*eps)
            std = p_s.tile([128, 4], F32, tag='std', name='std')
            nc.scalar.activation(std[:], ss[:], AF.Sqrt, bias=epsK[:])
            nc.vector.reciprocal(sclK[:, sb, :], std[:])

        def load_hq(l):
            hq = [p_hq.tile([128, 128], BF16, tag='hq', name='hq')
                  for _ in range(16)]
            for ch in range(16):
                nc.gpsimd.dma_start(hq[ch][:],
                                    hTq[bass.ts(ch, 128), bass.ts(l, 128)])
            return hq

        def q_unit(l, qg, hq):
            """Q proj+square+rope for local block l, head group qg."""
            psQ = ps_a.tile([128, 512], F32, tag='psA', name='psQ')
            for ch in range(16):
                nc.tensor.matmul(psQ[:], hq[ch][:],
                                 wq_s[ch][:, bass.ts(qg, 512)],
                                 start=(ch == 0), stop=(ch == 15))
            qcpb = p_cp.tile([128, 512], BF16, tag='cp', name='qcpb')
            nc.vector.tensor_copy(qcpb[:], psQ[:])
            ss = p_s.tile([128, 4], F32, tag='ss', name='ssq')
            for hq in range(4):
                nc.scalar.activation(dump[:], qcpb[:, bass.ts(hq, 128)],
                                     AF.Square, accum_out=ss[:, hq:hq + 1])
            qtb = p_tab.tile([128, 4, 64], BF16, tag='qtab', name='qtb')
            nc.gpsimd.dma_start(qtb[:], qtab[l * 128:(l + 1) * 128])
            qro = rope4(qcpb, qtb)
            return ss, qro

        def q_finish(l, qg, ss, qro):
            """rstd -> scale -> transpose into QT."""
            std = p_s.tile([128, 4], F32, tag='std', name='stdq')
            nc.scalar.activation(std[:], ss[:], AF.Sqrt, scale=1.0 / D,
                                 bias=epsQ[:])
            rstd = p_s.tile([128, 4], F32, tag='rstd', name='rstdq')
            nc.vector.reciprocal(rstd[:], std[:])
            qrov = qro[:].rearrange('p (t d) -> p t d', t=4)
            qn = p_ro.tile([128, 512], BF16, tag='ro', name='qn')
            qnv = qn[:].rearrange('p (t d) -> p t d', t=4)
            for hq in range(4):
                nc.vector.tensor_scalar_mul(qnv[:, hq, :], qrov[:, hq, :],
                                            rstd[:, hq:hq + 1])
                pst = ps_s.tile([128, 128], BF16, tag='psS', name='psT')
                nc.tensor.transpose(pst[:], qnv[:, hq, :], iden_s[:])
                nc.vector.tensor_copy(QT[:, qg * 4 + hq, bass.ts(l, 128)],
                                      pst[:])

        def attn_group(h, g, msk_t):
            """Attention for head h on si group g (local blocks 4g..4g+3).
            In G1 the proj pools are idle, so QK tiles also rotate through
            psA for a 4-deep pipeline."""
            kvh = h // 2
            jmax = JMAX[g]
            silo = bass.ts(g, 512)
            psCs = [ps_c.tile([128, 132], F32, tag='psC', name='psC')
                    for _ in range(4)]
            # software-pipelined QK: emit QK(j+ahead) before PV(j) so the PE
            # stream never blocks the Act exp stream on a full round trip.
            ahead = 2 if g == 1 else 1
            psSs = {}

            def do_qk(j):
                if g == 1 and j % 2 == 1:
                    psS = ps_a.tile([128, 512], F32, tag='psA', name='psS')
                else:
                    psS = ps_s.tile([128, 512], F32, tag='psS', name='psS')
                nc.tensor.matmul(psS[:], KT[:, kvh, bass.ts(j, 128)],
                                 QT[:, h, silo], start=True, stop=True)
                psSs[j] = psS

            for j in range(min(ahead, jmax + 1)):
                do_qk(j)
            for j in range(jmax + 1):
                psS = psSs.pop(j)
                ex = p_ex.tile([128, 512], BF16, tag='ex', name='ex')
                nc.scalar.activation(ex[:], psS[:], AF.Exp,
                                     scale=sclK[:, j, kvh:kvh + 1])
                if g == 0 or j >= 8:  # host mask handles diagonal + overhang
                    nc.vector.tensor_mul(ex[:], ex[:], msk_t[:, j - 8 * g, :])
                if j + ahead <= jmax:
                    do_qk(j + ahead)
                for s in range(4):
                    nc.tensor.matmul(psCs[s][:, 0:129],
                                     ex[:, bass.ts(s, 128)],
                                     VA[:, kvh, j, 0:129],
                                     start=(j == 0), stop=(j == jmax))
            for s in range(4):
                rd = p_rd.tile([128, 1], F32, tag='rd', name='rd')
                nc.vector.reciprocal(rd[:], psCs[s][:, 128:129])
                cn = p_ex.tile([128, 128], BF16, tag='cn', name='cn')
                nc.vector.tensor_scalar_mul(cn[:], psCs[s][:, 0:128], rd[:])
                pst = ps_s.tile([128, 128], BF16, tag='psS', name='psT')
                nc.tensor.transpose(pst[:], cn[:], iden_s[:])
                nc.vector.tensor_copy(ctm[:, h, bass.ts(4 * g + s, 128)],
                                      pst[:])

        # ------- pass 0: K/V proj for global sj blocks 0..7 -------
        for sb in range(8):
            kv_unit(sb, ht_t, sb, do_scl=True)
        msk_s0 = p_msk.tile([128, 8, 512], BF16, tag='msk', name='msk0')
        nc.gpsimd.dma_start(
            msk_s0[:], mskin[0:8].rearrange('t p d -> p t d'))

        # ------- Q proj for local blocks 0..3 (needed by G0) -------
        for l in range(4):
            hq = load_hq(l)
            for qg in range(2):
                ss, qro = q_unit(l, qg, hq)
                q_finish(l, qg, ss, qro)

        # ------- G0 attention interleaved with pass 1 K/V proj ------
        ht_t2 = [p_ht.tile([128, 1024], BF16, tag='ht', name='ht1')
                 for _ in range(16)]
        for ch in range(16):
            nc.gpsimd.dma_start(ht_t2[ch][:], hT[bass.ts(ch, 128), 1024:2048])
        p1_ss = {}
        q_stash = {}
        hq_cur = [None]
        for h in range(8):
            attn_group(h, 0, msk_s0)
            sb = 8 + h
            p1_ss[sb] = kv_unit(sb, ht_t2, sb - 8, do_scl=False)
            # Q proj for local blocks 4..7 (squares now, sqrts batched later)
            l, qg = 4 + h // 2, h % 2
            if qg == 0:
                hq_cur[0] = load_hq(l)
            q_stash[(l, qg)] = q_unit(l, qg, hq_cur[0])

        # ------- batched pass-1 norms (single sqrt-table era) -------
        for sb in range(8, 16):
            scl_finish(sb, p1_ss[sb])
        for (l, qg), (ss, qro) in q_stash.items():
            q_finish(l, qg, ss, qro)

        # ------- wo loads (reuse ht pool) -------
        wo_s = [p_ht.tile([128, 1024], BF16, tag='ht', name='wo')
                for _ in range(16)]
        for ch in range(16):
            nc.gpsimd.dma_start(wo_s[ch][:], owT[bass.ts(ch, 128), :])

        # ------- G1 attention + shared-HBM ctx export -------
        msk_s1 = p_msk.tile([128, 8, 512], BF16, tag='msk', name='msk1')
        nc.gpsimd.dma_start(
            msk_s1[:], mskin[8:16].rearrange('t p d -> p t d'))
        for h in range(8):
            attn_group(h, 1, msk_s1)
            rtmp = nc.gpsimd.alloc_register(f'myrow{h}')
            nc.gpsimd.reg_load(rtmp, rows_s[h:h + 1, 0:1])
            rrow = nc.gpsimd.snap(rtmp, donate=True, min_val=0, max_val=1920)
            nc.gpsimd.dma_start(shctx[bass.ds(rrow, 128), :], ctm[:, h, :])
            if h == 6:  # barrier 1: peer heads 0..6
                nc.gpsimd.dma_start(bar_in[0:1, 0:1], shctx[0:1, 0:1])
                nc.gpsimd.collective_compute(
                    'AllGather', mybir.AluOpType.bypass,
                    replica_groups=[[0, 1], [2, 3], [4, 5], [6, 7]],
                    ins=[bar_in[:].opt()], outs=[bar_out[:].opt()])

        # barrier 2 covers head 7 (barrier 1 was issued inside the G1 loop
        # after head 6's export; pokes read shctx so the RAW deps order each
        # barrier after the exports emitted before it).
        nc.gpsimd.dma_start(bar_in2[0:1, 0:1], shctx[0:1, 0:1])
        nc.gpsimd.collective_compute(
            'AllGather', mybir.AluOpType.bypass,
            replica_groups=[[0, 1], [2, 3], [4, 5], [6, 7]],
            ins=[bar_in2[:].opt()], outs=[bar_out2[:].opt()])
        ctf = [p_wq.tile([128, 1024], BF16, tag='wq', name='ctf')
               for _ in range(8)]
        for i in range(8):
            # corner poke: WAW dep orders the peer read after its barrier
            bo = bar_out if i < 7 else bar_out2
            nc.sync.dma_start(ctf[i][0:1, 0:1], bo[0:1, 0:1])
            ptmp = nc.sync.alloc_register(f'prow{i}')
            nc.sync.reg_load(ptmp, rows_s[i:i + 1, 1:2])
            prow = nc.sync.snap(ptmp, donate=True, min_val=0, max_val=1920)
            nc.sync.dma_start(ctf[i][:], shctx[bass.ds(prow, 128), :])

        # ------- o_proj: local-chunk halves lead, peer halves pipelined ----
        # owT is host-permuted to local-first chunk order, so rows 0..7 pair
        # with ctm heads and 8..15 with peer ctf chunks — uniform program.
        units = [(bi, nt) for bi in range(8) for nt in range(2)]

        oslots = [(ps_a, 'psA'), (ps_a, 'psA'), (ps_s, 'psS'), (ps_s, 'psS')]

        def o_first(u):
            bi, nt = units[u]
            pool_u, tag_u = oslots[u % 4]
            psO = pool_u.tile([128, 512], F32, tag=tag_u, name='psO')
            for i in range(8):
                nc.tensor.matmul(psO[:], ctm[:, i, bass.ts(bi, 128)],
                                 wo_s[i][:, bass.ts(nt, 512)],
                                 start=(i == 0), stop=False)
            return psO

        def o_second(u, psO):
            bi, nt = units[u]
            for i in range(8):
                nc.tensor.matmul(psO[:], ctf[i][:, bass.ts(bi, 128)],
                                 wo_s[8 + i][:, bass.ts(nt, 512)],
                                 start=False, stop=(i == 7))
            ob = p_ob.tile([128, 512], F32, tag='ob', name='ob')
            nc.vector.tensor_copy(ob[:], psO[:])
            nc.gpsimd.dma_start(
                out_e[bass.ts(bi, 128), bass.ts(nt, 512)], ob[:])

        live = []
        for u in range(16):
            live.append((u, o_first(u)))
            if len(live) == 4:
                v, psO = live.pop(0)
                o_second(v, psO)
        for v, psO in live:
            o_second(v, psO)

    split_multi_waits(nc)
    return nc


# ---------------------------------------------------------------------------
_NC_CACHE = None
_LAST_IN_MAPS = None


def _get_nc():
    global _NC_CACHE
    if _NC_CACHE is None:
        _NC_CACHE = build_kernel()
    return _NC_CACHE


def make_in_maps(hidden_states, cos, sin, q_w, k_w, v_w, o_w, q_norm_w, k_norm_w):
    import ml_dtypes
    bf16 = ml_dtypes.bfloat16

    hidden_states = np.asarray(hidden_states, np.float32)
    cos = np.asarray(cos, np.float32)
    sin = np.asarray(sin, np.float32)
    q_w = np.asarray(q_w, np.float32)
    k_w = np.asarray(k_w, np.float32)
    v_w = np.asarray(v_w, np.float32)
    o_w = np.asarray(o_w, np.float32)
    q_norm_w = np.asarray(q_norm_w, np.float32)
    k_norm_w = np.asarray(k_norm_w, np.float32)

    tri_np = np.triu(np.ones((128, 128), np.float32))  # [sj,si]: valid sj<=si
    iden_np = np.eye(128, dtype=np.float32).astype(bf16)

    def rope_tabs(c, s_, w):
        cl, sl = c[:, 0:64], s_[:, 0:64]
        wl, wh = w[0:64], w[64:128]
        return np.stack([cl * wl, sl * wh, cl * wh, sl * wl], axis=1).astype(bf16)

    in_maps = []
    for c in range(8):
        b, sh, hh = c >> 2, (c >> 1) & 1, c & 1
        blks = MYBLKS[sh]
        rows = np.concatenate([np.arange(g * 128, (g + 1) * 128) for g in blks])
        # o_w contraction rows permuted local-first: my hh half then peer half
        operm = np.concatenate([
            np.arange(hh * 1024, (hh + 1) * 1024),
            np.arange((1 - hh) * 1024, (2 - hh) * 1024)])
        myrows = ((hh * 8 + np.arange(8)) * 128).astype(np.uint32)[:, None]
        prows = (((1 - hh) * 8 + np.arange(8)) * 128).astype(np.uint32)[:, None]
        # masks indexed by global sj block j: j<8 -> si group 0 (locals 0..3),
        # j>=8 -> group 1 (locals 4..7). ones below diag, tri on diag, zero
        # above.
        msk = np.zeros((16, 128, 512), np.float32)
        for j in range(16):
            loc = range(4) if j < 8 else range(4, 8)
            for s_i, l in enumerate(loc):
                g_s = blks[l]
                if j < g_s:
                    msk[j, :, s_i * 128:(s_i + 1) * 128] = 1.0
                elif j == g_s:
                    msk[j, :, s_i * 128:(s_i + 1) * 128] = tri_np
        in_maps.append(dict(
            hT=np.ascontiguousarray(hidden_states[b].T).astype(bf16),
            hTq=np.ascontiguousarray(hidden_states[b][rows].T).astype(bf16),
            qwT=np.ascontiguousarray(q_w[hh * 1024:(hh + 1) * 1024].T).astype(bf16),
            kwT=np.ascontiguousarray(k_w[hh * 512:(hh + 1) * 512].T).astype(bf16),
            vwT=np.ascontiguousarray(v_w[hh * 512:(hh + 1) * 512].T).astype(bf16),
            owT=np.ascontiguousarray(
                o_w[hh * 1024:(hh + 1) * 1024].T[operm]).astype(bf16),
            qtab=rope_tabs(cos[b][rows], sin[b][rows], q_norm_w),
            ktab=rope_tabs(cos[b], sin[b], k_norm_w),
            mskin=msk.astype(bf16), iden=iden_np,
            myrows=myrows, prows=prows))
    return in_maps


def gather_out(outs):
    """outs: list of 8 per-core 'out' arrays -> full [B,S,HID]."""
    out = np.zeros((B, S, HID), np.float32)
    for c in range(8):
        b, sh, hh = c >> 2, (c >> 1) & 1, c & 1
        o = np.asarray(outs[c], np.float32)  # [1024, 1024]
        for l, g in enumerate(MYBLKS[sh]):
            out[b, g * 128:(g + 1) * 128, hh * 1024:(hh + 1) * 1024] = \
                o[l * 128:(l + 1) * 128]
    return out


def kernel(hidden_states, cos, sin, q_w, k_w, v_w, o_w, q_norm_w, k_norm_w):
    from concourse.bass_utils import run_bass_kernel_spmd

    in_maps = make_in_maps(hidden_states, cos, sin, q_w, k_w, v_w, o_w,
                           q_norm_w, k_norm_w)
    global _LAST_IN_MAPS
    _LAST_IN_MAPS = in_maps
    nc = _get_nc()
    res = run_bass_kernel_spmd(nc, in_maps, core_ids=list(range(8)))
    return gather_out([res.results[c]['out'] for c in range(8)])


if __name__ == '__main__':
    sys.path.insert(0, '/root/problem')
    import reference
    inputs = {k: np.asarray(v) for k, v in reference.setup_inputs().items()}
    exp = np.asarray(reference.reference(**inputs))
    act = kernel(**inputs)
    rel = np.linalg.norm(act - exp) / np.linalg.norm(exp)
    print('Relative error:', rel)


# revision 20
# speedup vs baseline: 95006.6063x; 1.0170x over previous
"""Trainium2 Bass kernel for Qwen-style GQA attention (B=2,S=2048,H=16,KV=8,D=128).

Sharding (8 cores): batch(2) x si-stripes(2) x head-half(2), uniform SPMD
program (all per-core variation flows through host-prepared inputs).
  core c: b=c>>2, sh=(c>>1)&1, hh=c&1
  stripes: MYBLKS[sh] — causally balanced interleaved si blocks.

vs the original baseline:
  - attention processes si in two groups of 4 local blocks with [128,512]-wide
    exp instructions (uniform j bounds 0..7 / 0..15; host masks zero the
    causal overhang), cutting Act-engine time ~30%
  - PSUM->SBUF copies move off the Act engine to Pool/DVE (bf16 2x modes)
  - RMSNorm squares read PSUM directly; sqrt batched into two act-table eras
    so exp/sqrt table thrash is bounded at 4 loads
  - all big inputs arrive bf16 (half the HBM traffic of f32)
  - ctx exchange is two staged pair-AllGathers (heads 0-3 overlap the
    attention of heads 4-7); o_proj pipelines cc1-half / cc2-half chunks
  - o_proj results DMA straight from PSUM
"""
import sys

sys.path.insert(0, '/opt/trn_rl_repo')

import numpy as np

import concourse.bass as bass
import concourse.tile as tile
from concourse import mybir
from concourse.vector_clock import ScopedClock, VectorClock

B, S, HID = 2, 2048, 2048
H, KV, D = 16, 8, 128
EPS = 1e-6
NBLK = S // 16  # noqa
# causally balanced si-block stripes: sum(blk+1) = 68 for both
MYBLKS = [[0, 2, 4, 6, 9, 11, 13, 15], [1, 3, 5, 7, 8, 10, 12, 14]]
JMAX = [7, 15]  # uniform j bound per si group (max over stripes)

F32 = mybir.dt.float32
BF16 = mybir.dt.bfloat16
AF = mybir.ActivationFunctionType


# ---------------------------------------------------------------------------
# Workarounds: this walrus supports only ONE sync-wait per instruction.
def _patched_drain_and_barrier(self, tick_clock, wait_clock):
    gc = tick_clock.global_clock
    vec = list(gc)
    nz = [i for i, v in enumerate(vec) if v > 0] or [0]
    for i in nz:
        cvec = [vec[j] if j == i else 0 for j in range(len(vec))]
        inst = self.nc.sync.drain()
        wait_clock.add_sem_waits(inst.ins, ScopedClock({None: VectorClock(cvec)}))
    self.nc.all_engine_barrier()
    assert self.sems is not None
    popped = self.nc._tile_sem_poison_stack.pop()
    assert popped is self._sem_poison
    self.nc.clear_and_free_semaphores(list(self.sems.allocated().values()))
    self.nc.all_engine_barrier()


tile.TileContext._drain_and_barrier = _patched_drain_and_barrier


def split_multi_waits(nc):
    for fn in nc.m.functions:
        for blk in fn.blocks:
            insts = list(blk.instructions)
            out = []
            changed = False
            for inst in insts:
                si = inst.sync_info
                if si is not None and len(si.on_wait) > 1:
                    waits = list(si.on_wait)
                    for k, w in enumerate(waits[:-1]):
                        out.append(mybir.InstNoOp(
                            name=f"{inst.name}.w{k}", engine=inst.engine,
                            sync_info=mybir.SyncInfo(on_wait=[w], on_update=[]),
                            text_hint="waitsplit"))
                    si.on_wait = [waits[-1]]
                    changed = True
                out.append(inst)
            if changed:
                blk.instructions[:] = out


def mul_b(eng, out, a, b):
    a2, b2 = bass.broadcast_tensor_aps(a, b)
    eng.tensor_mul(out, a2, b2)


# ---------------------------------------------------------------------------
def build_kernel():
    nc = bass.Bass(trn_type='TRN2')
    hT = nc.dram_tensor('hT', [HID, S], BF16, kind='ExternalInput')
    hTq = nc.dram_tensor('hTq', [HID, 1024], BF16, kind='ExternalInput')
    qwT = nc.dram_tensor('qwT', [HID, 1024], BF16, kind='ExternalInput')
    kwT = nc.dram_tensor('kwT', [HID, 512], BF16, kind='ExternalInput')
    vwT = nc.dram_tensor('vwT', [HID, 512], BF16, kind='ExternalInput')
    owT = nc.dram_tensor('owT', [2048, 1024], BF16, kind='ExternalInput')
    qtab = nc.dram_tensor('qtab', [1024, 4, 64], BF16, kind='ExternalInput')
    ktab = nc.dram_tensor('ktab', [S, 4, 64], BF16, kind='ExternalInput')
    mskin = nc.dram_tensor('mskin', [16, 128, 512], BF16, kind='ExternalInput')
    iden = nc.dram_tensor('iden', [128, 128], BF16, kind='ExternalInput')
    myrows = nc.dram_tensor('myrows', [8, 1], mybir.dt.uint32,
                            kind='ExternalInput')
    prows = nc.dram_tensor('prows', [8, 1], mybir.dt.uint32,
                           kind='ExternalInput')
    out_e = nc.dram_tensor('out', [1024, 1024], F32, kind='ExternalOutput')
    # ctx^T exchange buffer in the pair-shared HBM domain: rows hh*1024+h*128
    shctx = nc.dram_tensor('shctx', [2048, 1024], BF16, kind='Internal',
                           addr_space='Shared')
    bar_in = nc.dram_tensor('bar_in', [1, 1], BF16, kind='Internal')
    bar_out = nc.dram_tensor('bar_out', [2, 1], BF16, kind='Internal')
    bar_in2 = nc.dram_tensor('bar_in2', [1, 1], BF16, kind='Internal')
    bar_out2 = nc.dram_tensor('bar_out2', [2, 1], BF16, kind='Internal')

    from contextlib import ExitStack
    with ExitStack() as ctx:
        tc = ctx.enter_context(tile.TileContext(nc))
        pool = lambda name, bufs, **kw: ctx.enter_context(
            tc.tile_pool(name=name, bufs=bufs, **kw))
        p_wq = pool('wq', 16)     # wq tiles; ctf chunks reuse
        p_wk = pool('wk', 16)
        p_wv = pool('wv', 16)
        p_ht = pool('ht', 16)     # ht pass0 -> hTq -> ht pass1 -> wo
        p_big = pool('big', 1)    # KT/VA/QT/ctm/masks persistent
        p_c = pool('const', 1)
        p_tab = pool('tab', 4)    # streamed rope tables
        p_hq = pool('hq', 24)     # streamed hTq chunks (16 per si block)
        p_ex = pool('expb', 3)
        p_cp = pool('cpb', 4)
        p_ro = pool('rope', 12)   # rope outputs (8 can be stashed)
        p_t12 = pool('t12', 4)
        p_ob = pool('outb', 2)
        p_s = pool('small', 20)
        p_scl = pool('scl', 1)
        p_rd = pool('rd', 2)
        p_d = pool('dump', 1)
        ps_a = pool('psA', 2, space='PSUM')
        ps_s = pool('psS', 2, space='PSUM')
        ps_c = pool('psC', 4, space='PSUM')
        p_dram = pool('dram', 8, space='DRAM')

        # ---- constants ----
        p_msk = pool('msk', 1)
        iden_s = p_c.tile([128, 128], BF16)
        nc.gpsimd.dma_start(iden_s[:], iden[:])

        # persistent activations
        KT = p_big.tile([128, 4, 2048], BF16, name='KT')     # [d, kvh, sj]
        VA = p_big.tile([128, 4, 16, 132], BF16, name='VA')  # [sj, kvh, sb, d|1]
        QT = p_big.tile([128, 8, 1024], BF16, name='QT')     # [d, h, si-local]
        ctm = p_big.tile([128, 8, 1024], BF16, name='ctm')   # [d, h, si-local]
        sclK = p_scl.tile([128, 16, 4], F32, tag='sclK', name='sclK')
        nc.gpsimd.memset(VA[:, :, :, 128:129], 1.0)
        epsK = p_c.tile([128, 1], F32)
        nc.gpsimd.memset(epsK[:], float(D * EPS))
        epsQ = p_c.tile([128, 1], F32)
        nc.gpsimd.memset(epsQ[:], float(EPS))

        # ---- weights + first ht pass (pass-0 critical loads first) ----
        wq_s = [p_wq.tile([128, 1024], BF16, tag='wq', name='wq') for _ in range(16)]
        wk_s = [p_wk.tile([128, 512], BF16, tag='wk', name='wk') for _ in range(16)]
        wv_s = [p_wv.tile([128, 512], BF16, tag='wv', name='wv') for _ in range(16)]
        ht_t = [p_ht.tile([128, 1024], BF16, tag='ht', name='ht0')
                for _ in range(16)]
        for ch in range(16):
            r = bass.ts(ch, 128)
            nc.gpsimd.dma_start(ht_t[ch][:], hT[r, 0:1024])
            nc.gpsimd.dma_start(wv_s[ch][:], vwT[r, :])
            nc.gpsimd.dma_start(wk_s[ch][:], kwT[r, :])
        for ch in range(16):
            nc.gpsimd.dma_start(wq_s[ch][:], qwT[bass.ts(ch, 128), :])

        dump = p_d.tile([128, 128], F32, name='dump')
        rows_s = p_c.tile([8, 2], mybir.dt.uint32, name='rows_s')
        nc.gpsimd.dma_start(rows_s[:, 0:1], myrows[:])
        nc.gpsimd.dma_start(rows_s[:, 1:2], prows[:])

        # ------- helpers -------
        def rope4(cpb, tab):
            """RoPE for 4 heads packed [128, 4*128] bf16 -> new bf16 tile."""
            lo = cpb[:].rearrange('p (t d) -> p t d', t=4)[:, :, 0:64]
            hi = cpb[:].rearrange('p (t d) -> p t d', t=4)[:, :, 64:128]
            ro = p_ro.tile([128, 512], BF16, tag='ro', name='ro')
            rov = ro[:].rearrange('p (t d) -> p t d', t=4)
            t1 = p_t12.tile([128, 4, 64], BF16, tag='t12', name='t1')
            t2 = p_t12.tile([128, 4, 64], BF16, tag='t12', name='t2')
            mul_b(nc.vector, t1[:], lo, tab[:, 0:1, :])
            mul_b(nc.vector, t2[:], hi, tab[:, 1:2, :])
            nc.vector.tensor_sub(rov[:, :, 0:64], t1[:], t2[:])
            mul_b(nc.vector, t1[:], hi, tab[:, 2:3, :])
            mul_b(nc.vector, t2[:], lo, tab[:, 3:4, :])
            nc.vector.tensor_add(rov[:, :, 64:128], t1[:], t2[:])
            return ro

        def kv_unit(sb, ht_t, col, do_scl):
            """V+K projection for global sj block sb from pass tiles ht_t
            (col = 128-col offset in the pass window). During pass 0 the
            attention pools are idle, so psV borrows psS to decouple the
            V/K PSUM rotation."""
            sslice = bass.ts(col, 128)
            if sb < 8:
                psV = ps_s.tile([128, 512], F32, tag='psS', name='psV')
            else:
                psV = ps_a.tile([128, 512], F32, tag='psA', name='psV')
            for ch in range(16):
                nc.tensor.matmul(psV[:], ht_t[ch][:, sslice], wv_s[ch][:],
                                 start=(ch == 0), stop=(ch == 15))
            nc.scalar.copy(VA[:, :, sb, 0:128],
                           psV[:].rearrange('p (t d) -> p t d', t=4))
            psK = ps_a.tile([128, 512], F32, tag='psA', name='psK')
            for ch in range(16):
                nc.tensor.matmul(psK[:], ht_t[ch][:, sslice], wk_s[ch][:],
                                 start=(ch == 0), stop=(ch == 15))
            kcpb = p_cp.tile([128, 512], BF16, tag='cp', name='kcpb')
            nc.vector.tensor_copy(kcpb[:], psK[:])
            ss = p_s.tile([128, 4], F32, tag='ss', name='ssk')
            for kvh in range(4):
                nc.scalar.activation(dump[:], kcpb[:, bass.ts(kvh, 128)],
                                     AF.Square, accum_out=ss[:, kvh:kvh + 1])
            ktb = p_tab.tile([128, 4, 64], BF16, tag='ktab', name='ktb')
            nc.gpsimd.dma_start(ktb[:], ktab[sb * 128:(sb + 1) * 128])
            kro = rope4(kcpb, ktb)
            krov = kro[:].rearrange('p (t d) -> p t d', t=4)
            for kvh in range(4):
                pst = ps_s.tile([128, 128], BF16, tag='psS', name='psT')
                nc.tensor.transpose(pst[:], krov[:, kvh, :], iden_s[:])
                nc.scalar.copy(KT[:, kvh, bass.ts(sb, 128)], pst[:])
            if do_scl:
                scl_finish(sb, ss)
            return ss

        def scl_finish(sb, ss):
            # SCALE*rstd folded exactly: 1/sqrt(ss + D*eps) = exp(-ln(.)/2);
            # ln/exp/square/copy share one act table -> no table thrash.
            lt = p_s.tile([128, 4], F32, tag='std', name='lt')
            nc.scalar.activation(lt[:], ss[:], AF.Ln, bias=epsK[:])
            nc.scalar.activation(sclK[:, sb, :], lt[:], AF.Exp, scale=-0.5)

        def load_hq(l):
            hq = [p_hq.tile([128, 128], BF16, tag='hq', name='hq')
                  for _ in range(16)]
            for ch in range(16):
                nc.gpsimd.dma_start(hq[ch][:],
                                    hTq[bass.ts(ch, 128), bass.ts(l, 128)])
            return hq

        def q_unit(l, qg, hq):
            """Q proj+square+rope for local block l, head group qg."""
            psQ = ps_a.tile([128, 512], F32, tag='psA', name='psQ')
            for ch in range(16):
                nc.tensor.matmul(psQ[:], hq[ch][:],
                                 wq_s[ch][:, bass.ts(qg, 512)],
                                 start=(ch == 0), stop=(ch == 15))
            qcpb = p_cp.tile([128, 512], BF16, tag='cp', name='qcpb')
            nc.vector.tensor_copy(qcpb[:], psQ[:])
            ss = p_s.tile([128, 4], F32, tag='ss', name='ssq')
            for hq in range(4):
                nc.scalar.activation(dump[:], qcpb[:, bass.ts(hq, 128)],
                                     AF.Square, accum_out=ss[:, hq:hq + 1])
            qtb = p_tab.tile([128, 4, 64], BF16, tag='qtab', name='qtb')
            nc.gpsimd.dma_start(qtb[:], qtab[l * 128:(l + 1) * 128])
            qro = rope4(qcpb, qtb)
            return ss, qro

        def q_finish(l, qg, ss, qro):
            """rstd -> scale -> transpose into QT."""
            lt = p_s.tile([128, 4], F32, tag='std', name='ltq')
            nc.scalar.activation(lt[:], ss[:], AF.Ln, scale=1.0 / D,
                                 bias=epsQ[:])
            rstd = p_s.tile([128, 4], F32, tag='rstd', name='rstdq')
            nc.scalar.activation(rstd[:], lt[:], AF.Exp, scale=-0.5)
            qrov = qro[:].rearrange('p (t d) -> p t d', t=4)
            qn = p_ro.tile([128, 512], BF16, tag='ro', name='qn')
            qnv = qn[:].rearrange('p (t d) -> p t d', t=4)
            for hq in range(4):
                nc.vector.tensor_scalar_mul(qnv[:, hq, :], qrov[:, hq, :],
                                            rstd[:, hq:hq + 1])
                pst = ps_s.tile([128, 128], BF16, tag='psS', name='psT')
                nc.tensor.transpose(pst[:], qnv[:, hq, :], iden_s[:])
                nc.vector.tensor_copy(QT[:, qg * 4 + hq, bass.ts(l, 128)],
                                      pst[:])

        def attn_group(h, g, msk_t):
            """Attention for head h on si group g (local blocks 4g..4g+3).
            In G1 the proj pools are idle, so QK tiles also rotate through
            psA for a 4-deep pipeline."""
            kvh = h // 2
            jmax = JMAX[g]
            silo = bass.ts(g, 512)
            psCs = [ps_c.tile([128, 132], F32, tag='psC', name='psC')
                    for _ in range(4)]
            # software-pipelined QK: emit QK(j+ahead) before PV(j) so the PE
            # stream never blocks the Act exp stream on a full round trip.
            ahead = 2 if g == 1 else 1
            psSs = {}

            def do_qk(j):
                if g == 1 and j % 2 == 1:
                    psS = ps_a.tile([128, 512], F32, tag='psA', name='psS')
                else:
                    psS = ps_s.tile([128, 512], F32, tag='psS', name='psS')
                nc.tensor.matmul(psS[:], KT[:, kvh, bass.ts(j, 128)],
                                 QT[:, h, silo], start=True, stop=True)
                psSs[j] = psS

            for j in range(min(ahead, jmax + 1)):
                do_qk(j)
            for j in range(jmax + 1):
                psS = psSs.pop(j)
                ex = p_ex.tile([128, 512], BF16, tag='ex', name='ex')
                nc.scalar.activation(ex[:], psS[:], AF.Exp,
                                     scale=sclK[:, j, kvh:kvh + 1])
                if g == 0 or j >= 8:  # host mask handles diagonal + overhang
                    nc.vector.tensor_mul(ex[:], ex[:], msk_t[:, j - 8 * g, :])
                if j + ahead <= jmax:
                    do_qk(j + ahead)
                for s in range(4):
                    nc.tensor.matmul(psCs[s][:, 0:129],
                                     ex[:, bass.ts(s, 128)],
                                     VA[:, kvh, j, 0:129],
                                     start=(j == 0), stop=(j == jmax))
            for s in range(4):
                rd = p_rd.tile([128, 1], F32, tag='rd', name='rd')
                nc.vector.reciprocal(rd[:], psCs[s][:, 128:129])
                cn = p_ex.tile([128, 128], BF16, tag='cn', name='cn')
                nc.vector.tensor_scalar_mul(cn[:], psCs[s][:, 0:128], rd[:])
                pst = ps_s.tile([128, 128], BF16, tag='psS', name='psT')
                nc.tensor.transpose(pst[:], cn[:], iden_s[:])
                nc.vector.tensor_copy(ctm[:, h, bass.ts(4 * g + s, 128)],
                                      pst[:])

        # ------- pass 0: K/V proj for global sj blocks 0..7 -------
        for sb in range(8):
            kv_unit(sb, ht_t, sb, do_scl=True)
        msk_s0 = p_msk.tile([128, 8, 512], BF16, tag='msk', name='msk0')
        nc.gpsimd.dma_start(
            msk_s0[:], mskin[0:8].rearrange('t p d -> p t d'))

        # ------- Q proj for local blocks 0..3 (needed by G0) -------
        for l in range(4):
            hq = load_hq(l)
            for qg in range(2):
                ss, qro = q_unit(l, qg, hq)
                q_finish(l, qg, ss, qro)

        # ------- G0 attention interleaved with pass 1 K/V proj ------
        ht_t2 = [p_ht.tile([128, 1024], BF16, tag='ht', name='ht1')
                 for _ in range(16)]
        for ch in range(16):
            nc.gpsimd.dma_start(ht_t2[ch][:], hT[bass.ts(ch, 128), 1024:2048])
        hq_cur = [None]
        for h in range(8):
            attn_group(h, 0, msk_s0)
            sb = 8 + h
            kv_unit(sb, ht_t2, sb - 8, do_scl=True)
            l, qg = 4 + h // 2, h % 2
            if qg == 0:
                hq_cur[0] = load_hq(l)
            ss, qro = q_unit(l, qg, hq_cur[0])
            q_finish(l, qg, ss, qro)

        # ------- wo loads (reuse ht pool) -------
        wo_s = [p_ht.tile([128, 1024], BF16, tag='ht', name='wo')
                for _ in range(16)]
        for ch in range(16):
            nc.gpsimd.dma_start(wo_s[ch][:], owT[bass.ts(ch, 128), :])

        # ------- G1 attention + shared-HBM ctx export -------
        msk_s1 = p_msk.tile([128, 8, 512], BF16, tag='msk', name='msk1')
        nc.gpsimd.dma_start(
            msk_s1[:], mskin[8:16].rearrange('t p d -> p t d'))
        for h in range(8):
            attn_group(h, 1, msk_s1)
            rtmp = nc.gpsimd.alloc_register(f'myrow{h}')
            nc.gpsimd.reg_load(rtmp, rows_s[h:h + 1, 0:1])
            rrow = nc.gpsimd.snap(rtmp, donate=True, min_val=0, max_val=1920)
            nc.gpsimd.dma_start(shctx[bass.ds(rrow, 128), :], ctm[:, h, :])
            if h == 6:  # barrier 1: peer heads 0..6
                nc.gpsimd.dma_start(bar_in[0:1, 0:1], shctx[0:1, 0:1])
                nc.gpsimd.collective_compute(
                    'AllGather', mybir.AluOpType.bypass,
                    replica_groups=[[0, 1], [2, 3], [4, 5], [6, 7]],
                    ins=[bar_in[:].opt()], outs=[bar_out[:].opt()])

        # barrier 2 covers head 7 (barrier 1 was issued inside the G1 loop
        # after head 6's export; pokes read shctx so the RAW deps order each
        # barrier after the exports emitted before it).
        nc.gpsimd.dma_start(bar_in2[0:1, 0:1], shctx[0:1, 0:1])
        nc.gpsimd.collective_compute(
            'AllGather', mybir.AluOpType.bypass,
            replica_groups=[[0, 1], [2, 3], [4, 5], [6, 7]],
            ins=[bar_in2[:].opt()], outs=[bar_out2[:].opt()])
        ctf = [p_wq.tile([128, 1024], BF16, tag='wq', name='ctf')
               for _ in range(8)]
        for i in range(8):
            # corner poke: WAW dep orders the peer read after its barrier
            bo = bar_out if i < 7 else bar_out2
            nc.sync.dma_start(ctf[i][0:1, 0:1], bo[0:1, 0:1])
            ptmp = nc.sync.alloc_register(f'prow{i}')
            nc.sync.reg_load(ptmp, rows_s[i:i + 1, 1:2])
            prow = nc.sync.snap(ptmp, donate=True, min_val=0, max_val=1920)
            nc.sync.dma_start(ctf[i][:], shctx[bass.ds(prow, 128), :])

        # ------- o_proj: local-chunk halves lead, peer halves pipelined ----
        # owT is host-permuted to local-first chunk order, so rows 0..7 pair
        # with ctm heads and 8..15 with peer ctf chunks — uniform program.
        units = [(bi, nt) for bi in range(8) for nt in range(2)]

        oslots = [(ps_a, 'psA'), (ps_a, 'psA'), (ps_s, 'psS'), (ps_s, 'psS')]

        def o_first(u):
            bi, nt = units[u]
            pool_u, tag_u = oslots[u % 4]
            psO = pool_u.tile([128, 512], F32, tag=tag_u, name='psO')
            for i in range(8):
                nc.tensor.matmul(psO[:], ctm[:, i, bass.ts(bi, 128)],
                                 wo_s[i][:, bass.ts(nt, 512)],
                                 start=(i == 0), stop=False)
            return psO

        def o_second(u, psO):
            bi, nt = units[u]
            for i in range(8):
                nc.tensor.matmul(psO[:], ctf[i][:, bass.ts(bi, 128)],
                                 wo_s[8 + i][:, bass.ts(nt, 512)],
                                 start=False, stop=(i == 7))
            ob = p_ob.tile([128, 512], F32, tag='ob', name='ob')
            nc.vector.tensor_copy(ob[:], psO[:])
            nc.gpsimd.dma_start(
                out_e[bass.ts(bi, 128), bass.ts(nt, 512)], ob[:])

        live = []
        for u in range(16):
            live.append((u, o_first(u)))
            if len(live) == 4:
                v, psO = live.pop(0)
                o_second(v, psO)
        for v, psO in live:
            o_second(v, psO)

    split_multi_waits(nc)
    return nc


# ---------------------------------------------------------------------------
_NC_CACHE = None
_LAST_IN_MAPS = None


def _get_nc():
    global _NC_CACHE
    if _NC_CACHE is None:
        _NC_CACHE = build_kernel()
    return _NC_CACHE


def make_in_maps(hidden_states, cos, sin, q_w, k_w, v_w, o_w, q_norm_w, k_norm_w):
    import ml_dtypes
    bf16 = ml_dtypes.bfloat16

    hidden_states = np.asarray(hidden_states, np.float32)
    cos = np.asarray(cos, np.float32)
    sin = np.asarray(sin, np.float32)
    q_w = np.asarray(q_w, np.float32)
    k_w = np.asarray(k_w, np.float32)
    v_w = np.asarray(v_w, np.float32)
    o_w = np.asarray(o_w, np.float32)
    q_norm_w = np.asarray(q_norm_w, np.float32)
    k_norm_w = np.asarray(k_norm_w, np.float32)

    tri_np = np.triu(np.ones((128, 128), np.float32))  # [sj,si]: valid sj<=si
    iden_np = np.eye(128, dtype=np.float32).astype(bf16)

    def rope_tabs(c, s_, w):
        cl, sl = c[:, 0:64], s_[:, 0:64]
        wl, wh = w[0:64], w[64:128]
        return np.stack([cl * wl, sl * wh, cl * wh, sl * wl], axis=1).astype(bf16)

    in_maps = []
    for c in range(8):
        b, sh, hh = c >> 2, (c >> 1) & 1, c & 1
        blks = MYBLKS[sh]
        rows = np.concatenate([np.arange(g * 128, (g + 1) * 128) for g in blks])
        # o_w contraction rows permuted local-first: my hh half then peer half
        operm = np.concatenate([
            np.arange(hh * 1024, (hh + 1) * 1024),
            np.arange((1 - hh) * 1024, (2 - hh) * 1024)])
        myrows = ((hh * 8 + np.arange(8)) * 128).astype(np.uint32)[:, None]
        prows = (((1 - hh) * 8 + np.arange(8)) * 128).astype(np.uint32)[:, None]
        # masks indexed by global sj block j: j<8 -> si group 0 (locals 0..3),
        # j>=8 -> group 1 (locals 4..7). ones below diag, tri on diag, zero
        # above.
        msk = np.zeros((16, 128, 512), np.float32)
        for j in range(16):
            loc = range(4) if j < 8 else range(4, 8)
            for s_i, l in enumerate(loc):
                g_s = blks[l]
                if j < g_s:
                    msk[j, :, s_i * 128:(s_i + 1) * 128] = 1.0
                elif j == g_s:
                    msk[j, :, s_i * 128:(s_i + 1) * 128] = tri_np
        in_maps.append(dict(
            hT=np.ascontiguousarray(hidden_states[b].T).astype(bf16),
            hTq=np.ascontiguousarray(hidden_states[b][rows].T).astype(bf16),
            qwT=np.ascontiguousarray(q_w[hh * 1024:(hh + 1) * 1024].T).astype(bf16),
            kwT=np.ascontiguousarray(k_w[hh * 512:(hh + 1) * 512].T).astype(bf16),
            vwT=np.ascontiguousarray(v_w[hh * 512:(hh + 1) * 512].T).astype(bf16),
            owT=np.ascontiguousarray(
                o_w[hh * 1024:(hh + 1) * 1024].T[operm]).astype(bf16),
            qtab=rope_tabs(cos[b][rows], sin[b][rows], q_norm_w),
            ktab=rope_tabs(cos[b], sin[b], k_norm_w),
            mskin=msk.astype(bf16), iden=iden_np,
            myrows=myrows, prows=prows))
    return in_maps


def gather_out(outs):
    """outs: list of 8 per-core 'out' arrays -> full [B,S,HID]."""
    out = np.zeros((B, S, HID), np.float32)
    for c in range(8):
        b, sh, hh = c >> 2, (c >> 1) & 1, c & 1
        o = np.asarray(outs[c], np.float32)  # [1024, 1024]
        for l, g in enumerate(MYBLKS[sh]):
            out[b, g * 128:(g + 1) * 128, hh * 1024:(hh + 1) * 1024] = \
                o[l * 128:(l + 1) * 128]
    return out


def kernel(hidden_states, cos, sin, q_w, k_w, v_w, o_w, q_norm_w, k_norm_w):
    from concourse.bass_utils import run_bass_kernel_spmd

    in_maps = make_in_maps(hidden_states, cos, sin, q_w, k_w, v_w, o_w,
                           q_norm_w, k_norm_w)
    global _LAST_IN_MAPS
    _LAST_IN_MAPS = in_maps
    nc = _get_nc()
    res = run_bass_kernel_spmd(nc, in_maps, core_ids=list(range(8)))
    return gather_out([res.results[c]['out'] for c in range(8)])


if __name__ == '__main__':
    sys.path.insert(0, '/root/problem')
    import reference
    inputs = {k: np.asarray(v) for k, v in reference.setup_inputs().items()}
    exp = np.asarray(reference.reference(**inputs))
    act = kernel(**inputs)
    rel = np.linalg.norm(act - exp) / np.linalg.norm(exp)
    print('Relative error:', rel)


# revision 21
# speedup vs baseline: 95101.4346x; 1.0010x over previous
"""Trainium2 Bass kernel for Qwen-style GQA attention (B=2,S=2048,H=16,KV=8,D=128).

Sharding (8 cores): batch(2) x si-stripes(2) x head-half(2), uniform SPMD
program (all per-core variation flows through host-prepared inputs).
  core c: b=c>>2, sh=(c>>1)&1, hh=c&1
  stripes: MYBLKS[sh] — causally balanced interleaved si blocks.

vs the original baseline:
  - attention processes si in two groups of 4 local blocks with [128,512]-wide
    exp instructions (uniform j bounds 0..7 / 0..15; host masks zero the
    causal overhang), cutting Act-engine time ~30%
  - PSUM->SBUF copies move off the Act engine to Pool/DVE (bf16 2x modes)
  - RMSNorm squares read PSUM directly; sqrt batched into two act-table eras
    so exp/sqrt table thrash is bounded at 4 loads
  - all big inputs arrive bf16 (half the HBM traffic of f32)
  - ctx exchange is two staged pair-AllGathers (heads 0-3 overlap the
    attention of heads 4-7); o_proj pipelines cc1-half / cc2-half chunks
  - o_proj results DMA straight from PSUM
"""
import sys

sys.path.insert(0, '/opt/trn_rl_repo')

import numpy as np

import concourse.bass as bass
import concourse.tile as tile
from concourse import mybir
from concourse.vector_clock import ScopedClock, VectorClock

B, S, HID = 2, 2048, 2048
H, KV, D = 16, 8, 128
EPS = 1e-6
NBLK = S // 16  # noqa
# causally balanced si-block stripes: sum(blk+1) = 68 for both
MYBLKS = [[0, 2, 4, 6, 9, 11, 13, 15], [1, 3, 5, 7, 8, 10, 12, 14]]
JMAX = [7, 15]  # uniform j bound per si group (max over stripes)

F32 = mybir.dt.float32
BF16 = mybir.dt.bfloat16
AF = mybir.ActivationFunctionType


# ---------------------------------------------------------------------------
# Workarounds: this walrus supports only ONE sync-wait per instruction.
def _patched_drain_and_barrier(self, tick_clock, wait_clock):
    gc = tick_clock.global_clock
    vec = list(gc)
    nz = [i for i, v in enumerate(vec) if v > 0] or [0]
    for i in nz:
        cvec = [vec[j] if j == i else 0 for j in range(len(vec))]
        inst = self.nc.sync.drain()
        wait_clock.add_sem_waits(inst.ins, ScopedClock({None: VectorClock(cvec)}))
    self.nc.all_engine_barrier()
    assert self.sems is not None
    popped = self.nc._tile_sem_poison_stack.pop()
    assert popped is self._sem_poison
    self.nc.clear_and_free_semaphores(list(self.sems.allocated().values()))
    self.nc.all_engine_barrier()


tile.TileContext._drain_and_barrier = _patched_drain_and_barrier


def split_multi_waits(nc):
    for fn in nc.m.functions:
        for blk in fn.blocks:
            insts = list(blk.instructions)
            out = []
            changed = False
            for inst in insts:
                si = inst.sync_info
                if si is not None and len(si.on_wait) > 1:
                    waits = list(si.on_wait)
                    for k, w in enumerate(waits[:-1]):
                        out.append(mybir.InstNoOp(
                            name=f"{inst.name}.w{k}", engine=inst.engine,
                            sync_info=mybir.SyncInfo(on_wait=[w], on_update=[]),
                            text_hint="waitsplit"))
                    si.on_wait = [waits[-1]]
                    changed = True
                out.append(inst)
            if changed:
                blk.instructions[:] = out


def mul_b(eng, out, a, b):
    a2, b2 = bass.broadcast_tensor_aps(a, b)
    eng.tensor_mul(out, a2, b2)


# ---------------------------------------------------------------------------
def build_kernel():
    nc = bass.Bass(trn_type='TRN2')
    hT = nc.dram_tensor('hT', [HID, S], BF16, kind='ExternalInput')
    hTq = nc.dram_tensor('hTq', [HID, 1024], BF16, kind='ExternalInput')
    qwT = nc.dram_tensor('qwT', [HID, 1024], BF16, kind='ExternalInput')
    kwT = nc.dram_tensor('kwT', [HID, 512], BF16, kind='ExternalInput')
    vwT = nc.dram_tensor('vwT', [HID, 512], BF16, kind='ExternalInput')
    owT = nc.dram_tensor('owT', [2048, 1024], BF16, kind='ExternalInput')
    qtab = nc.dram_tensor('qtab', [1024, 4, 64], BF16, kind='ExternalInput')
    ktab = nc.dram_tensor('ktab', [S, 4, 64], BF16, kind='ExternalInput')
    mskin = nc.dram_tensor('mskin', [16, 128, 512], BF16, kind='ExternalInput')
    iden = nc.dram_tensor('iden', [128, 128], BF16, kind='ExternalInput')
    myrows = nc.dram_tensor('myrows', [8, 1], mybir.dt.uint32,
                            kind='ExternalInput')
    prows = nc.dram_tensor('prows', [8, 1], mybir.dt.uint32,
                           kind='ExternalInput')
    out_e = nc.dram_tensor('out', [1024, 1024], F32, kind='ExternalOutput')
    # ctx^T exchange buffer in the pair-shared HBM domain: rows hh*1024+h*128
    shctx = nc.dram_tensor('shctx', [2048, 1024], BF16, kind='Internal',
                           addr_space='Shared')
    bar_in = nc.dram_tensor('bar_in', [1, 1], BF16, kind='Internal')
    bar_out = nc.dram_tensor('bar_out', [2, 1], BF16, kind='Internal')
    bar_in2 = nc.dram_tensor('bar_in2', [1, 1], BF16, kind='Internal')
    bar_out2 = nc.dram_tensor('bar_out2', [2, 1], BF16, kind='Internal')

    from contextlib import ExitStack
    with ExitStack() as ctx:
        tc = ctx.enter_context(tile.TileContext(nc))
        pool = lambda name, bufs, **kw: ctx.enter_context(
            tc.tile_pool(name=name, bufs=bufs, **kw))
        p_wq = pool('wq', 16)     # wq tiles; ctf chunks reuse
        p_wk = pool('wk', 16)
        p_wv = pool('wv', 16)
        p_ht = pool('ht', 16)     # ht pass0 -> hTq -> ht pass1 -> wo
        p_big = pool('big', 1)    # KT/VA/QT/ctm/masks persistent
        p_c = pool('const', 1)
        p_tab = pool('tab', 4)    # streamed rope tables
        p_hq = pool('hq', 24)     # streamed hTq chunks (16 per si block)
        p_ex = pool('expb', 5)
        p_cp = pool('cpb', 4)
        p_ro = pool('rope', 6)    # rope outputs (finished inline now)
        p_t12 = pool('t12', 4)
        p_ob = pool('outb', 2)
        p_s = pool('small', 20)
        p_scl = pool('scl', 1)
        p_rd = pool('rd', 2)
        p_d = pool('dump', 1)
        ps_a = pool('psA', 2, space='PSUM')
        ps_s = pool('psS', 2, space='PSUM')
        ps_c = pool('psC', 4, space='PSUM')
        p_dram = pool('dram', 8, space='DRAM')

        # ---- constants ----
        p_msk = pool('msk', 1)
        iden_s = p_c.tile([128, 128], BF16)
        nc.gpsimd.dma_start(iden_s[:], iden[:])

        # persistent activations
        KT = p_big.tile([128, 4, 2048], BF16, name='KT')     # [d, kvh, sj]
        VA = p_big.tile([128, 4, 16, 132], BF16, name='VA')  # [sj, kvh, sb, d|1]
        QT = p_big.tile([128, 8, 1024], BF16, name='QT')     # [d, h, si-local]
        ctm = p_big.tile([128, 8, 1024], BF16, name='ctm')   # [d, h, si-local]
        sclK = p_scl.tile([128, 16, 4], F32, tag='sclK', name='sclK')
        nc.gpsimd.memset(VA[:, :, :, 128:129], 1.0)
        epsK = p_c.tile([128, 1], F32)
        nc.gpsimd.memset(epsK[:], float(D * EPS))
        epsQ = p_c.tile([128, 1], F32)
        nc.gpsimd.memset(epsQ[:], float(EPS))

        # ---- weights + first ht pass (pass-0 critical loads first) ----
        wq_s = [p_wq.tile([128, 1024], BF16, tag='wq', name='wq') for _ in range(16)]
        wk_s = [p_wk.tile([128, 512], BF16, tag='wk', name='wk') for _ in range(16)]
        wv_s = [p_wv.tile([128, 512], BF16, tag='wv', name='wv') for _ in range(16)]
        ht_t = [p_ht.tile([128, 1024], BF16, tag='ht', name='ht0')
                for _ in range(16)]
        for ch in range(16):
            r = bass.ts(ch, 128)
            nc.gpsimd.dma_start(ht_t[ch][:], hT[r, 0:1024])
            nc.gpsimd.dma_start(wv_s[ch][:], vwT[r, :])
            nc.gpsimd.dma_start(wk_s[ch][:], kwT[r, :])
        for ch in range(16):
            nc.gpsimd.dma_start(wq_s[ch][:], qwT[bass.ts(ch, 128), :])

        dump = p_d.tile([128, 128], F32, name='dump')
        rows_s = p_c.tile([8, 2], mybir.dt.uint32, name='rows_s')
        nc.gpsimd.dma_start(rows_s[:, 0:1], myrows[:])
        nc.gpsimd.dma_start(rows_s[:, 1:2], prows[:])

        # ------- helpers -------
        def rope4(cpb, tab):
            """RoPE for 4 heads packed [128, 4*128] bf16 -> new bf16 tile."""
            lo = cpb[:].rearrange('p (t d) -> p t d', t=4)[:, :, 0:64]
            hi = cpb[:].rearrange('p (t d) -> p t d', t=4)[:, :, 64:128]
            ro = p_ro.tile([128, 512], BF16, tag='ro', name='ro')
            rov = ro[:].rearrange('p (t d) -> p t d', t=4)
            t1 = p_t12.tile([128, 4, 64], BF16, tag='t12', name='t1')
            t2 = p_t12.tile([128, 4, 64], BF16, tag='t12', name='t2')
            mul_b(nc.vector, t1[:], lo, tab[:, 0:1, :])
            mul_b(nc.vector, t2[:], hi, tab[:, 1:2, :])
            nc.vector.tensor_sub(rov[:, :, 0:64], t1[:], t2[:])
            mul_b(nc.vector, t1[:], hi, tab[:, 2:3, :])
            mul_b(nc.vector, t2[:], lo, tab[:, 3:4, :])
            nc.vector.tensor_add(rov[:, :, 64:128], t1[:], t2[:])
            return ro

        def kv_unit(sb, ht_t, col, do_scl):
            """V+K projection for global sj block sb from pass tiles ht_t
            (col = 128-col offset in the pass window). During pass 0 the
            attention pools are idle, so psV borrows psS to decouple the
            V/K PSUM rotation."""
            sslice = bass.ts(col, 128)
            if sb < 8:
                psV = ps_s.tile([128, 512], F32, tag='psS', name='psV')
            else:
                psV = ps_a.tile([128, 512], F32, tag='psA', name='psV')
            for ch in range(16):
                nc.tensor.matmul(psV[:], ht_t[ch][:, sslice], wv_s[ch][:],
                                 start=(ch == 0), stop=(ch == 15))
            nc.scalar.copy(VA[:, :, sb, 0:128],
                           psV[:].rearrange('p (t d) -> p t d', t=4))
            psK = ps_a.tile([128, 512], F32, tag='psA', name='psK')
            for ch in range(16):
                nc.tensor.matmul(psK[:], ht_t[ch][:, sslice], wk_s[ch][:],
                                 start=(ch == 0), stop=(ch == 15))
            kcpb = p_cp.tile([128, 512], BF16, tag='cp', name='kcpb')
            nc.vector.tensor_copy(kcpb[:], psK[:])
            ss = p_s.tile([128, 4], F32, tag='ss', name='ssk')
            for kvh in range(4):
                nc.scalar.activation(dump[:], kcpb[:, bass.ts(kvh, 128)],
                                     AF.Square, accum_out=ss[:, kvh:kvh + 1])
            ktb = p_tab.tile([128, 4, 64], BF16, tag='ktab', name='ktb')
            nc.gpsimd.dma_start(ktb[:], ktab[sb * 128:(sb + 1) * 128])
            kro = rope4(kcpb, ktb)
            krov = kro[:].rearrange('p (t d) -> p t d', t=4)
            for kvh in range(4):
                pst = ps_s.tile([128, 128], BF16, tag='psS', name='psT')
                nc.tensor.transpose(pst[:], krov[:, kvh, :], iden_s[:])
                nc.scalar.copy(KT[:, kvh, bass.ts(sb, 128)], pst[:])
            if do_scl:
                scl_finish(sb, ss)
            return ss

        def scl_finish(sb, ss):
            # SCALE*rstd folded exactly: 1/sqrt(ss + D*eps) = exp(-ln(.)/2);
            # ln/exp/square/copy share one act table -> no table thrash.
            lt = p_s.tile([128, 4], F32, tag='std', name='lt')
            nc.scalar.activation(lt[:], ss[:], AF.Ln, bias=epsK[:])
            nc.scalar.activation(sclK[:, sb, :], lt[:], AF.Exp, scale=-0.5)

        def load_hq(l):
            hq = [p_hq.tile([128, 128], BF16, tag='hq', name='hq')
                  for _ in range(16)]
            for ch in range(16):
                nc.gpsimd.dma_start(hq[ch][:],
                                    hTq[bass.ts(ch, 128), bass.ts(l, 128)])
            return hq

        def q_unit(l, qg, hq):
            """Q proj+square+rope for local block l, head group qg."""
            psQ = ps_a.tile([128, 512], F32, tag='psA', name='psQ')
            for ch in range(16):
                nc.tensor.matmul(psQ[:], hq[ch][:],
                                 wq_s[ch][:, bass.ts(qg, 512)],
                                 start=(ch == 0), stop=(ch == 15))
            qcpb = p_cp.tile([128, 512], BF16, tag='cp', name='qcpb')
            nc.vector.tensor_copy(qcpb[:], psQ[:])
            ss = p_s.tile([128, 4], F32, tag='ss', name='ssq')
            for hq in range(4):
                nc.scalar.activation(dump[:], qcpb[:, bass.ts(hq, 128)],
                                     AF.Square, accum_out=ss[:, hq:hq + 1])
            qtb = p_tab.tile([128, 4, 64], BF16, tag='qtab', name='qtb')
            nc.gpsimd.dma_start(qtb[:], qtab[l * 128:(l + 1) * 128])
            qro = rope4(qcpb, qtb)
            return ss, qro

        def q_finish(l, qg, ss, qro):
            """rstd -> scale -> transpose into QT."""
            lt = p_s.tile([128, 4], F32, tag='std', name='ltq')
            nc.scalar.activation(lt[:], ss[:], AF.Ln, scale=1.0 / D,
                                 bias=epsQ[:])
            rstd = p_s.tile([128, 4], F32, tag='rstd', name='rstdq')
            nc.scalar.activation(rstd[:], lt[:], AF.Exp, scale=-0.5)
            qrov = qro[:].rearrange('p (t d) -> p t d', t=4)
            qn = p_ro.tile([128, 512], BF16, tag='ro', name='qn')
            qnv = qn[:].rearrange('p (t d) -> p t d', t=4)
            for hq in range(4):
                nc.vector.tensor_scalar_mul(qnv[:, hq, :], qrov[:, hq, :],
                                            rstd[:, hq:hq + 1])
                pst = ps_s.tile([128, 128], BF16, tag='psS', name='psT')
                nc.tensor.transpose(pst[:], qnv[:, hq, :], iden_s[:])
                nc.vector.tensor_copy(QT[:, qg * 4 + hq, bass.ts(l, 128)],
                                      pst[:])

        def attn_group(h, g, msk_t):
            """Attention for head h on si group g (local blocks 4g..4g+3).
            In G1 the proj pools are idle, so QK tiles also rotate through
            psA for a 4-deep pipeline."""
            kvh = h // 2
            jmax = JMAX[g]
            silo = bass.ts(g, 512)
            psCs = [ps_c.tile([128, 132], F32, tag='psC', name='psC')
                    for _ in range(4)]
            # software-pipelined QK: emit QK(j+ahead) before PV(j) so the PE
            # stream never blocks the Act exp stream on a full round trip.
            ahead = 2 if g == 1 else 1
            psSs = {}

            def do_qk(j):
                if g == 1 and j % 2 == 1:
                    psS = ps_a.tile([128, 512], F32, tag='psA', name='psS')
                else:
                    psS = ps_s.tile([128, 512], F32, tag='psS', name='psS')
                nc.tensor.matmul(psS[:], KT[:, kvh, bass.ts(j, 128)],
                                 QT[:, h, silo], start=True, stop=True)
                psSs[j] = psS

            for j in range(min(ahead, jmax + 1)):
                do_qk(j)
            for j in range(jmax + 1):
                psS = psSs.pop(j)
                ex = p_ex.tile([128, 512], BF16, tag='ex', name='ex')
                nc.scalar.activation(ex[:], psS[:], AF.Exp,
                                     scale=sclK[:, j, kvh:kvh + 1])
                if g == 0 or j >= 8:  # host mask handles diagonal + overhang
                    nc.vector.tensor_mul(ex[:], ex[:], msk_t[:, j - 8 * g, :])
                if j + ahead <= jmax:
                    do_qk(j + ahead)
                for s in range(4):
                    nc.tensor.matmul(psCs[s][:, 0:129],
                                     ex[:, bass.ts(s, 128)],
                                     VA[:, kvh, j, 0:129],
                                     start=(j == 0), stop=(j == jmax))
            for s in range(4):
                rd = p_rd.tile([128, 1], F32, tag='rd', name='rd')
                nc.vector.reciprocal(rd[:], psCs[s][:, 128:129])
                cn = p_ex.tile([128, 128], BF16, tag='cn', name='cn')
                nc.vector.tensor_scalar_mul(cn[:], psCs[s][:, 0:128], rd[:])
                pst = ps_s.tile([128, 128], BF16, tag='psS', name='psT')
                nc.tensor.transpose(pst[:], cn[:], iden_s[:])
                nc.vector.tensor_copy(ctm[:, h, bass.ts(4 * g + s, 128)],
                                      pst[:])

        # ------- pass 0: K/V proj for global sj blocks 0..7 -------
        for sb in range(8):
            kv_unit(sb, ht_t, sb, do_scl=True)
        msk_s0 = p_msk.tile([128, 8, 512], BF16, tag='msk', name='msk0')
        nc.gpsimd.dma_start(
            msk_s0[:], mskin[0:8].rearrange('t p d -> p t d'))

        # ------- Q proj for local blocks 0..3 (needed by G0) -------
        for l in range(4):
            hq = load_hq(l)
            for qg in range(2):
                ss, qro = q_unit(l, qg, hq)
                q_finish(l, qg, ss, qro)

        # ------- G0 attention interleaved with pass 1 K/V proj ------
        ht_t2 = [p_ht.tile([128, 1024], BF16, tag='ht', name='ht1')
                 for _ in range(16)]
        for ch in range(16):
            nc.gpsimd.dma_start(ht_t2[ch][:], hT[bass.ts(ch, 128), 1024:2048])
        hq_cur = [None]
        for h in range(8):
            attn_group(h, 0, msk_s0)
            sb = 8 + h
            kv_unit(sb, ht_t2, sb - 8, do_scl=True)
            l, qg = 4 + h // 2, h % 2
            if qg == 0:
                hq_cur[0] = load_hq(l)
            ss, qro = q_unit(l, qg, hq_cur[0])
            q_finish(l, qg, ss, qro)

        # ------- wo loads (reuse ht pool) -------
        wo_s = [p_ht.tile([128, 1024], BF16, tag='ht', name='wo')
                for _ in range(16)]
        for ch in range(16):
            nc.gpsimd.dma_start(wo_s[ch][:], owT[bass.ts(ch, 128), :])

        # ------- G1 attention + shared-HBM ctx export -------
        msk_s1 = p_msk.tile([128, 8, 512], BF16, tag='msk', name='msk1')
        nc.gpsimd.dma_start(
            msk_s1[:], mskin[8:16].rearrange('t p d -> p t d'))
        for h in range(8):
            attn_group(h, 1, msk_s1)
            rtmp = nc.gpsimd.alloc_register(f'myrow{h}')
            nc.gpsimd.reg_load(rtmp, rows_s[h:h + 1, 0:1])
            rrow = nc.gpsimd.snap(rtmp, donate=True, min_val=0, max_val=1920)
            nc.gpsimd.dma_start(shctx[bass.ds(rrow, 128), :], ctm[:, h, :])
            if h == 6:  # barrier 1: peer heads 0..6
                nc.gpsimd.dma_start(bar_in[0:1, 0:1], shctx[0:1, 0:1])
                nc.gpsimd.collective_compute(
                    'AllGather', mybir.AluOpType.bypass,
                    replica_groups=[[0, 1], [2, 3], [4, 5], [6, 7]],
                    ins=[bar_in[:].opt()], outs=[bar_out[:].opt()])

        # barrier 2 covers head 7 (barrier 1 was issued inside the G1 loop
        # after head 6's export; pokes read shctx so the RAW deps order each
        # barrier after the exports emitted before it).
        nc.gpsimd.dma_start(bar_in2[0:1, 0:1], shctx[0:1, 0:1])
        nc.gpsimd.collective_compute(
            'AllGather', mybir.AluOpType.bypass,
            replica_groups=[[0, 1], [2, 3], [4, 5], [6, 7]],
            ins=[bar_in2[:].opt()], outs=[bar_out2[:].opt()])
        ctf = [p_wq.tile([128, 1024], BF16, tag='wq', name='ctf')
               for _ in range(8)]
        for i in range(8):
            # corner poke: WAW dep orders the peer read after its barrier
            bo = bar_out if i < 7 else bar_out2
            nc.sync.dma_start(ctf[i][0:1, 0:1], bo[0:1, 0:1])
            ptmp = nc.sync.alloc_register(f'prow{i}')
            nc.sync.reg_load(ptmp, rows_s[i:i + 1, 1:2])
            prow = nc.sync.snap(ptmp, donate=True, min_val=0, max_val=1920)
            nc.sync.dma_start(ctf[i][:], shctx[bass.ds(prow, 128), :])

        # ------- o_proj: local-chunk halves lead, peer halves pipelined ----
        # owT is host-permuted to local-first chunk order, so rows 0..7 pair
        # with ctm heads and 8..15 with peer ctf chunks — uniform program.
        units = [(bi, nt) for bi in range(8) for nt in range(2)]

        oslots = [(ps_a, 'psA'), (ps_a, 'psA'), (ps_s, 'psS'), (ps_s, 'psS')]

        def o_first(u):
            bi, nt = units[u]
            pool_u, tag_u = oslots[u % 4]
            psO = pool_u.tile([128, 512], F32, tag=tag_u, name='psO')
            for i in range(8):
                nc.tensor.matmul(psO[:], ctm[:, i, bass.ts(bi, 128)],
                                 wo_s[i][:, bass.ts(nt, 512)],
                                 start=(i == 0), stop=False)
            return psO

        def o_second(u, psO):
            bi, nt = units[u]
            for i in range(8):
                nc.tensor.matmul(psO[:], ctf[i][:, bass.ts(bi, 128)],
                                 wo_s[8 + i][:, bass.ts(nt, 512)],
                                 start=False, stop=(i == 7))
            ob = p_ob.tile([128, 512], F32, tag='ob', name='ob')
            nc.vector.tensor_copy(ob[:], psO[:])
            nc.gpsimd.dma_start(
                out_e[bass.ts(bi, 128), bass.ts(nt, 512)], ob[:])

        live = []
        for u in range(16):
            live.append((u, o_first(u)))
            if len(live) == 4:
                v, psO = live.pop(0)
                o_second(v, psO)
        for v, psO in live:
            o_second(v, psO)

    split_multi_waits(nc)
    return nc


# ---------------------------------------------------------------------------
_NC_CACHE = None
_LAST_IN_MAPS = None


def _get_nc():
    global _NC_CACHE
    if _NC_CACHE is None:
        _NC_CACHE = build_kernel()
    return _NC_CACHE


def make_in_maps(hidden_states, cos, sin, q_w, k_w, v_w, o_w, q_norm_w, k_norm_w):
    import ml_dtypes
    bf16 = ml_dtypes.bfloat16

    hidden_states = np.asarray(hidden_states, np.float32)
    cos = np.asarray(cos, np.float32)
    sin = np.asarray(sin, np.float32)
    q_w = np.asarray(q_w, np.float32)
    k_w = np.asarray(k_w, np.float32)
    v_w = np.asarray(v_w, np.float32)
    o_w = np.asarray(o_w, np.float32)
    q_norm_w = np.asarray(q_norm_w, np.float32)
    k_norm_w = np.asarray(k_norm_w, np.float32)

    tri_np = np.triu(np.ones((128, 128), np.float32))  # [sj,si]: valid sj<=si
    iden_np = np.eye(128, dtype=np.float32).astype(bf16)

    def rope_tabs(c, s_, w):
        cl, sl = c[:, 0:64], s_[:, 0:64]
        wl, wh = w[0:64], w[64:128]
        return np.stack([cl * wl, sl * wh, cl * wh, sl * wl], axis=1).astype(bf16)

    in_maps = []
    for c in range(8):
        b, sh, hh = c >> 2, (c >> 1) & 1, c & 1
        blks = MYBLKS[sh]
        rows = np.concatenate([np.arange(g * 128, (g + 1) * 128) for g in blks])
        # o_w contraction rows permuted local-first: my hh half then peer half
        operm = np.concatenate([
            np.arange(hh * 1024, (hh + 1) * 1024),
            np.arange((1 - hh) * 1024, (2 - hh) * 1024)])
        myrows = ((hh * 8 + np.arange(8)) * 128).astype(np.uint32)[:, None]
        prows = (((1 - hh) * 8 + np.arange(8)) * 128).astype(np.uint32)[:, None]
        # masks indexed by global sj block j: j<8 -> si group 0 (locals 0..3),
        # j>=8 -> group 1 (locals 4..7). ones below diag, tri on diag, zero
        # above.
        msk = np.zeros((16, 128, 512), np.float32)
        for j in range(16):
            loc = range(4) if j < 8 else range(4, 8)
            for s_i, l in enumerate(loc):
                g_s = blks[l]
                if j < g_s:
                    msk[j, :, s_i * 128:(s_i + 1) * 128] = 1.0
                elif j == g_s:
                    msk[j, :, s_i * 128:(s_i + 1) * 128] = tri_np
        in_maps.append(dict(
            hT=np.ascontiguousarray(hidden_states[b].T).astype(bf16),
            hTq=np.ascontiguousarray(hidden_states[b][rows].T).astype(bf16),
            qwT=np.ascontiguousarray(q_w[hh * 1024:(hh + 1) * 1024].T).astype(bf16),
            kwT=np.ascontiguousarray(k_w[hh * 512:(hh + 1) * 512].T).astype(bf16),
            vwT=np.ascontiguousarray(v_w[hh * 512:(hh + 1) * 512].T).astype(bf16),
            owT=np.ascontiguousarray(
                o_w[hh * 1024:(hh + 1) * 1024].T[operm]).astype(bf16),
            qtab=rope_tabs(cos[b][rows], sin[b][rows], q_norm_w),
            ktab=rope_tabs(cos[b], sin[b], k_norm_w),
            mskin=msk.astype(bf16), iden=iden_np,
            myrows=myrows, prows=prows))
    return in_maps


def gather_out(outs):
    """outs: list of 8 per-core 'out' arrays -> full [B,S,HID]."""
    out = np.zeros((B, S, HID), np.float32)
    for c in range(8):
        b, sh, hh = c >> 2, (c >> 1) & 1, c & 1
        o = np.asarray(outs[c], np.float32)  # [1024, 1024]
        for l, g in enumerate(MYBLKS[sh]):
            out[b, g * 128:(g + 1) * 128, hh * 1024:(hh + 1) * 1024] = \
                o[l * 128:(l + 1) * 128]
    return out


def kernel(hidden_states, cos, sin, q_w, k_w, v_w, o_w, q_norm_w, k_norm_w):
    from concourse.bass_utils import run_bass_kernel_spmd

    in_maps = make_in_maps(hidden_states, cos, sin, q_w, k_w, v_w, o_w,
                           q_norm_w, k_norm_w)
    global _LAST_IN_MAPS
    _LAST_IN_MAPS = in_maps
    nc = _get_nc()
    res = run_bass_kernel_spmd(nc, in_maps, core_ids=list(range(8)))
    return gather_out([res.results[c]['out'] for c in range(8)])


if __name__ == '__main__':
    sys.path.insert(0, '/root/problem')
    import reference
    inputs = {k: np.asarray(v) for k, v in reference.setup_inputs().items()}
    exp = np.asarray(reference.reference(**inputs))
    act = kernel(**inputs)
    rel = np.linalg.norm(act - exp) / np.linalg.norm(exp)
    print('Relative error:', rel)


# revision 24
# speedup vs baseline: 95238.4255x; 1.0014x over previous
"""Trainium2 Bass kernel for Qwen-style GQA attention (B=2,S=2048,H=16,KV=8,D=128).

Sharding (8 cores): batch(2) x si-stripes(2) x head-half(2), uniform SPMD
program (all per-core variation flows through host-prepared inputs).
  core c: b=c>>2, sh=(c>>1)&1, hh=c&1
  stripes: MYBLKS[sh] — causally balanced interleaved si blocks.

vs the original baseline (620us -> 400us on the CoreSim cost model):
  - attention processes si in two groups of 4 local blocks with [128,512]-wide
    exp instructions (uniform j bounds 0..7 / 0..15; host masks zero the
    causal overhang), cutting Act-engine exp time ~30%
  - PSUM->SBUF copies moved off the Act engine to DVE (bf16 2x modes);
    GPSIMD never touches PSUM (hardware BIR constraint)
  - RMSNorm rstd = exp(-ln(ms+eps)/2): ln/exp/square/copy share one act
    table, so there is no act-table thrashing and norms finish inline
  - all big inputs arrive bf16 (half the HBM traffic of f32)
  - ctx^T is exchanged with the hh-partner through pair-shared HBM
    (addr_space='Shared'; per-head row offsets from an input table via
    reg_load + dynamic DMA slices keep the SPMD program uniform), guarded
    by two tiny AllGather barriers (heads 0-6 early, head 7 late)
  - o_proj splits into local-chunk halves (read from SBUF ctm during the
    barrier) and peer halves, with psO tiles parked across idle PSUM banks;
    G0 attention, pass-1 K/V projection and Q projection are interleaved
"""
import sys

sys.path.insert(0, '/opt/trn_rl_repo')

import numpy as np

import concourse.bass as bass
import concourse.tile as tile
from concourse import mybir
from concourse.vector_clock import ScopedClock, VectorClock

B, S, HID = 2, 2048, 2048
H, KV, D = 16, 8, 128
EPS = 1e-6
NBLK = S // 16  # noqa
# causally balanced si-block stripes: sum(blk+1) = 68 for both
MYBLKS = [[0, 2, 4, 6, 9, 11, 13, 15], [1, 3, 5, 7, 8, 10, 12, 14]]
JMAX = [7, 15]  # uniform j bound per si group (max over stripes)

F32 = mybir.dt.float32
BF16 = mybir.dt.bfloat16
AF = mybir.ActivationFunctionType


# ---------------------------------------------------------------------------
# Workarounds: this walrus supports only ONE sync-wait per instruction.
def _patched_drain_and_barrier(self, tick_clock, wait_clock):
    gc = tick_clock.global_clock
    vec = list(gc)
    nz = [i for i, v in enumerate(vec) if v > 0] or [0]
    for i in nz:
        cvec = [vec[j] if j == i else 0 for j in range(len(vec))]
        inst = self.nc.sync.drain()
        wait_clock.add_sem_waits(inst.ins, ScopedClock({None: VectorClock(cvec)}))
    self.nc.all_engine_barrier()
    assert self.sems is not None
    popped = self.nc._tile_sem_poison_stack.pop()
    assert popped is self._sem_poison
    self.nc.clear_and_free_semaphores(list(self.sems.allocated().values()))
    self.nc.all_engine_barrier()


tile.TileContext._drain_and_barrier = _patched_drain_and_barrier


def split_multi_waits(nc):
    for fn in nc.m.functions:
        for blk in fn.blocks:
            insts = list(blk.instructions)
            out = []
            changed = False
            for inst in insts:
                si = inst.sync_info
                if si is not None and len(si.on_wait) > 1:
                    waits = list(si.on_wait)
                    for k, w in enumerate(waits[:-1]):
                        out.append(mybir.InstNoOp(
                            name=f"{inst.name}.w{k}", engine=inst.engine,
                            sync_info=mybir.SyncInfo(on_wait=[w], on_update=[]),
                            text_hint="waitsplit"))
                    si.on_wait = [waits[-1]]
                    changed = True
                out.append(inst)
            if changed:
                blk.instructions[:] = out


def mul_b(eng, out, a, b):
    a2, b2 = bass.broadcast_tensor_aps(a, b)
    eng.tensor_mul(out, a2, b2)


# ---------------------------------------------------------------------------
def build_kernel():
    nc = bass.Bass(trn_type='TRN2')
    hT = nc.dram_tensor('hT', [HID, S], BF16, kind='ExternalInput')
    hTq = nc.dram_tensor('hTq', [HID, 1024], BF16, kind='ExternalInput')
    qwT = nc.dram_tensor('qwT', [HID, 1024], BF16, kind='ExternalInput')
    kwT = nc.dram_tensor('kwT', [HID, 512], BF16, kind='ExternalInput')
    vwT = nc.dram_tensor('vwT', [HID, 512], BF16, kind='ExternalInput')
    owT = nc.dram_tensor('owT', [2048, 1024], BF16, kind='ExternalInput')
    qtab = nc.dram_tensor('qtab', [1024, 4, 64], BF16, kind='ExternalInput')
    ktab = nc.dram_tensor('ktab', [S, 4, 64], BF16, kind='ExternalInput')
    mskin = nc.dram_tensor('mskin', [16, 128, 512], BF16, kind='ExternalInput')
    iden = nc.dram_tensor('iden', [128, 128], BF16, kind='ExternalInput')
    myrows = nc.dram_tensor('myrows', [8, 1], mybir.dt.uint32,
                            kind='ExternalInput')
    prows = nc.dram_tensor('prows', [8, 1], mybir.dt.uint32,
                           kind='ExternalInput')
    out_e = nc.dram_tensor('out', [1024, 1024], F32, kind='ExternalOutput')
    # ctx^T exchange buffer in the pair-shared HBM domain: rows hh*1024+h*128
    shctx = nc.dram_tensor('shctx', [2048, 1024], BF16, kind='Internal',
                           addr_space='Shared')
    bar_in = nc.dram_tensor('bar_in', [1, 1], BF16, kind='Internal')
    bar_out = nc.dram_tensor('bar_out', [2, 1], BF16, kind='Internal')
    bar_in2 = nc.dram_tensor('bar_in2', [1, 1], BF16, kind='Internal')
    bar_out2 = nc.dram_tensor('bar_out2', [2, 1], BF16, kind='Internal')

    from contextlib import ExitStack
    with ExitStack() as ctx:
        tc = ctx.enter_context(tile.TileContext(nc))
        pool = lambda name, bufs, **kw: ctx.enter_context(
            tc.tile_pool(name=name, bufs=bufs, **kw))
        p_wq = pool('wq', 16)     # wq tiles; ctf chunks reuse
        p_wk = pool('wk', 16)
        p_wv = pool('wv', 16)
        p_ht = pool('ht', 16)     # ht pass0 -> hTq -> ht pass1 -> wo
        p_big = pool('big', 1)    # KT/VA/QT/ctm/masks persistent
        p_c = pool('const', 1)
        p_tab = pool('tab', 4)    # streamed rope tables
        p_hq = pool('hq', 2)      # streamed hTq blocks [128,16,128]
        p_ex = pool('expb', 5)
        p_cp = pool('cpb', 4)
        p_ro = pool('rope', 6)    # rope outputs (finished inline now)
        p_t12 = pool('t12', 4)
        p_ob = pool('outb', 2)
        p_s = pool('small', 20)
        p_scl = pool('scl', 1)
        p_rd = pool('rd', 2)
        p_d = pool('dump', 1)
        ps_a = pool('psA', 2, space='PSUM')
        ps_s = pool('psS', 2, space='PSUM')
        ps_c = pool('psC', 4, space='PSUM')
        p_dram = pool('dram', 8, space='DRAM')

        # ---- constants ----
        p_msk = pool('msk', 1)
        iden_s = p_c.tile([128, 128], BF16)
        nc.gpsimd.dma_start(iden_s[:], iden[:])

        # persistent activations
        KT = p_big.tile([128, 4, 2048], BF16, name='KT')     # [d, kvh, sj]
        VA = p_big.tile([128, 4, 16, 132], BF16, name='VA')  # [sj, kvh, sb, d|1]
        QT = p_big.tile([128, 8, 1024], BF16, name='QT')     # [d, h, si-local]
        ctm = p_big.tile([128, 8, 1024], BF16, name='ctm')   # [d, h, si-local]
        sclK = p_scl.tile([128, 16, 4], F32, tag='sclK', name='sclK')
        nc.gpsimd.memset(VA[:, :, :, 128:129], 1.0)
        epsK = p_c.tile([128, 1], F32)
        nc.gpsimd.memset(epsK[:], float(D * EPS))
        epsQ = p_c.tile([128, 1], F32)
        nc.gpsimd.memset(epsQ[:], float(EPS))

        # ---- weights + first ht pass (pass-0 critical loads first) ----
        wq_s = [p_wq.tile([128, 1024], BF16, tag='wq', name='wq') for _ in range(16)]
        wk_s = [p_wk.tile([128, 512], BF16, tag='wk', name='wk') for _ in range(16)]
        wv_s = [p_wv.tile([128, 512], BF16, tag='wv', name='wv') for _ in range(16)]
        ht_t = [p_ht.tile([128, 1024], BF16, tag='ht', name='ht0')
                for _ in range(16)]
        for ch in range(16):
            r = bass.ts(ch, 128)
            nc.gpsimd.dma_start(ht_t[ch][:], hT[r, 0:1024])
            nc.gpsimd.dma_start(wv_s[ch][:], vwT[r, :])
            nc.gpsimd.dma_start(wk_s[ch][:], kwT[r, :])
        for ch in range(16):
            nc.gpsimd.dma_start(wq_s[ch][:], qwT[bass.ts(ch, 128), :])

        dump = p_d.tile([128, 128], F32, name='dump')
        rows_s = p_c.tile([8, 2], mybir.dt.uint32, name='rows_s')
        nc.gpsimd.dma_start(rows_s[:, 0:1], myrows[:])
        nc.gpsimd.dma_start(rows_s[:, 1:2], prows[:])

        # ------- helpers -------
        def rope4(cpb, tab):
            """RoPE for 4 heads packed [128, 4*128] bf16 -> new bf16 tile."""
            lo = cpb[:].rearrange('p (t d) -> p t d', t=4)[:, :, 0:64]
            hi = cpb[:].rearrange('p (t d) -> p t d', t=4)[:, :, 64:128]
            ro = p_ro.tile([128, 512], BF16, tag='ro', name='ro')
            rov = ro[:].rearrange('p (t d) -> p t d', t=4)
            t1 = p_t12.tile([128, 4, 64], BF16, tag='t12', name='t1')
            t2 = p_t12.tile([128, 4, 64], BF16, tag='t12', name='t2')
            mul_b(nc.vector, t1[:], lo, tab[:, 0:1, :])
            mul_b(nc.vector, t2[:], hi, tab[:, 1:2, :])
            nc.vector.tensor_sub(rov[:, :, 0:64], t1[:], t2[:])
            mul_b(nc.vector, t1[:], hi, tab[:, 2:3, :])
            mul_b(nc.vector, t2[:], lo, tab[:, 3:4, :])
            nc.vector.tensor_add(rov[:, :, 64:128], t1[:], t2[:])
            return ro

        def kv_unit(sb, ht_t, col, do_scl):
            """V+K projection for global sj block sb from pass tiles ht_t
            (col = 128-col offset in the pass window). During pass 0 the
            attention pools are idle, so psV borrows psS to decouple the
            V/K PSUM rotation."""
            sslice = bass.ts(col, 128)
            if sb < 8:
                psV = ps_s.tile([128, 512], F32, tag='psS', name='psV')
            else:
                psV = ps_a.tile([128, 512], F32, tag='psA', name='psV')
            for ch in range(16):
                nc.tensor.matmul(psV[:], ht_t[ch][:, sslice], wv_s[ch][:],
                                 start=(ch == 0), stop=(ch == 15))
            nc.scalar.copy(VA[:, :, sb, 0:128],
                           psV[:].rearrange('p (t d) -> p t d', t=4))
            psK = ps_a.tile([128, 512], F32, tag='psA', name='psK')
            for ch in range(16):
                nc.tensor.matmul(psK[:], ht_t[ch][:, sslice], wk_s[ch][:],
                                 start=(ch == 0), stop=(ch == 15))
            kcpb = p_cp.tile([128, 512], BF16, tag='cp', name='kcpb')
            nc.vector.tensor_copy(kcpb[:], psK[:])
            ss = p_s.tile([128, 4], F32, tag='ss', name='ssk')
            for kvh in range(4):
                nc.scalar.activation(dump[:], kcpb[:, bass.ts(kvh, 128)],
                                     AF.Square, accum_out=ss[:, kvh:kvh + 1])
            ktb = p_tab.tile([128, 4, 64], BF16, tag='ktab', name='ktb')
            nc.gpsimd.dma_start(ktb[:], ktab[sb * 128:(sb + 1) * 128])
            kro = rope4(kcpb, ktb)
            krov = kro[:].rearrange('p (t d) -> p t d', t=4)
            for kvh in range(4):
                pst = ps_s.tile([128, 128], BF16, tag='psS', name='psT')
                nc.tensor.transpose(pst[:], krov[:, kvh, :], iden_s[:])
                nc.scalar.copy(KT[:, kvh, bass.ts(sb, 128)], pst[:])
            if do_scl:
                scl_finish(sb, ss)
            return ss

        def scl_finish(sb, ss):
            # SCALE*rstd folded exactly: 1/sqrt(ss + D*eps) = exp(-ln(.)/2);
            # ln/exp/square/copy share one act table -> no table thrash.
            lt = p_s.tile([128, 4], F32, tag='std', name='lt')
            nc.scalar.activation(lt[:], ss[:], AF.Ln, bias=epsK[:])
            nc.scalar.activation(sclK[:, sb, :], lt[:], AF.Exp, scale=-0.5)

        def load_hq(l):
            hq = p_hq.tile([128, 16, 128], BF16, tag='hq', name='hq')
            nc.gpsimd.dma_start(
                hq[:], hTq[:, bass.ts(l, 128)].rearrange('(n p) d -> p n d',
                                                         p=128))
            return hq

        def q_unit(l, qg, hq):
            """Q proj+square+rope for local block l, head group qg."""
            psQ = ps_a.tile([128, 512], F32, tag='psA', name='psQ')
            for ch in range(16):
                nc.tensor.matmul(psQ[:], hq[:, ch, :],
                                 wq_s[ch][:, bass.ts(qg, 512)],
                                 start=(ch == 0), stop=(ch == 15))
            qcpb = p_cp.tile([128, 512], BF16, tag='cp', name='qcpb')
            nc.vector.tensor_copy(qcpb[:], psQ[:])
            ss = p_s.tile([128, 4], F32, tag='ss', name='ssq')
            for hq in range(4):
                nc.scalar.activation(dump[:], qcpb[:, bass.ts(hq, 128)],
                                     AF.Square, accum_out=ss[:, hq:hq + 1])
            qtb = p_tab.tile([128, 4, 64], BF16, tag='qtab', name='qtb')
            nc.gpsimd.dma_start(qtb[:], qtab[l * 128:(l + 1) * 128])
            qro = rope4(qcpb, qtb)
            return ss, qro

        def q_finish(l, qg, ss, qro):
            """rstd -> scale -> transpose into QT."""
            lt = p_s.tile([128, 4], F32, tag='std', name='ltq')
            nc.scalar.activation(lt[:], ss[:], AF.Ln, scale=1.0 / D,
                                 bias=epsQ[:])
            rstd = p_s.tile([128, 4], F32, tag='rstd', name='rstdq')
            nc.scalar.activation(rstd[:], lt[:], AF.Exp, scale=-0.5)
            qrov = qro[:].rearrange('p (t d) -> p t d', t=4)
            qn = p_ro.tile([128, 512], BF16, tag='ro', name='qn')
            qnv = qn[:].rearrange('p (t d) -> p t d', t=4)
            for hq in range(4):
                nc.vector.tensor_scalar_mul(qnv[:, hq, :], qrov[:, hq, :],
                                            rstd[:, hq:hq + 1])
                pst = ps_s.tile([128, 128], BF16, tag='psS', name='psT')
                nc.tensor.transpose(pst[:], qnv[:, hq, :], iden_s[:])
                nc.vector.tensor_copy(QT[:, qg * 4 + hq, bass.ts(l, 128)],
                                      pst[:])

        def attn_group(h, g, msk_t):
            """Attention for head h on si group g (local blocks 4g..4g+3).
            In G1 the proj pools are idle, so QK tiles also rotate through
            psA for a 4-deep pipeline."""
            kvh = h // 2
            jmax = JMAX[g]
            silo = bass.ts(g, 512)
            psCs = [ps_c.tile([128, 132], F32, tag='psC', name='psC')
                    for _ in range(4)]
            # software-pipelined QK: emit QK(j+ahead) before PV(j) so the PE
            # stream never blocks the Act exp stream on a full round trip.
            ahead = 2 if g == 1 else 1
            psSs = {}

            def do_qk(j):
                if g == 1 and j % 2 == 1:
                    psS = ps_a.tile([128, 512], F32, tag='psA', name='psS')
                else:
                    psS = ps_s.tile([128, 512], F32, tag='psS', name='psS')
                nc.tensor.matmul(psS[:], KT[:, kvh, bass.ts(j, 128)],
                                 QT[:, h, silo], start=True, stop=True)
                psSs[j] = psS

            for j in range(min(ahead, jmax + 1)):
                do_qk(j)
            for j in range(jmax + 1):
                psS = psSs.pop(j)
                ex = p_ex.tile([128, 512], BF16, tag='ex', name='ex')
                nc.scalar.activation(ex[:], psS[:], AF.Exp,
                                     scale=sclK[:, j, kvh:kvh + 1])
                if g == 0 or j >= 8:  # host mask handles diagonal + overhang
                    nc.vector.tensor_mul(ex[:], ex[:], msk_t[:, j - 8 * g, :])
                if j + ahead <= jmax:
                    do_qk(j + ahead)
                for s in range(4):
                    nc.tensor.matmul(psCs[s][:, 0:129],
                                     ex[:, bass.ts(s, 128)],
                                     VA[:, kvh, j, 0:129],
                                     start=(j == 0), stop=(j == jmax))
            for s in range(4):
                rd = p_rd.tile([128, 1], F32, tag='rd', name='rd')
                nc.vector.reciprocal(rd[:], psCs[s][:, 128:129])
                cn = p_ex.tile([128, 128], BF16, tag='cn', name='cn')
                nc.vector.tensor_scalar_mul(cn[:], psCs[s][:, 0:128], rd[:])
                pst = ps_s.tile([128, 128], BF16, tag='psS', name='psT')
                nc.tensor.transpose(pst[:], cn[:], iden_s[:])
                nc.vector.tensor_copy(ctm[:, h, bass.ts(4 * g + s, 128)],
                                      pst[:])

        # ------- pass 0: K/V proj for global sj blocks 0..7 -------
        for sb in range(8):
            kv_unit(sb, ht_t, sb, do_scl=True)
        msk_s0 = p_msk.tile([128, 8, 512], BF16, tag='msk', name='msk0')
        nc.gpsimd.dma_start(
            msk_s0[:], mskin[0:8].rearrange('t p d -> p t d'))

        # ------- Q proj for local blocks 0..3 (needed by G0) -------
        for l in range(4):
            hq = load_hq(l)
            for qg in range(2):
                ss, qro = q_unit(l, qg, hq)
                q_finish(l, qg, ss, qro)

        # ------- G0 attention interleaved with pass 1 K/V proj ------
        ht_t2 = [p_ht.tile([128, 1024], BF16, tag='ht', name='ht1')
                 for _ in range(16)]
        for ch in range(16):
            nc.gpsimd.dma_start(ht_t2[ch][:], hT[bass.ts(ch, 128), 1024:2048])
        hq_cur = [None]
        for h in range(8):
            attn_group(h, 0, msk_s0)
            sb = 8 + h
            kv_unit(sb, ht_t2, sb - 8, do_scl=True)
            l, qg = 4 + h // 2, h % 2
            if qg == 0:
                hq_cur[0] = load_hq(l)
            ss, qro = q_unit(l, qg, hq_cur[0])
            q_finish(l, qg, ss, qro)

        # ------- wo loads (reuse ht pool) -------
        wo_s = [p_ht.tile([128, 1024], BF16, tag='ht', name='wo')
                for _ in range(16)]
        for ch in range(16):
            nc.gpsimd.dma_start(wo_s[ch][:], owT[bass.ts(ch, 128), :])

        # ------- G1 attention + shared-HBM ctx export -------
        msk_s1 = p_msk.tile([128, 8, 512], BF16, tag='msk', name='msk1')
        nc.gpsimd.dma_start(
            msk_s1[:], mskin[8:16].rearrange('t p d -> p t d'))
        for h in range(8):
            attn_group(h, 1, msk_s1)
            rtmp = nc.gpsimd.alloc_register(f'myrow{h}')
            nc.gpsimd.reg_load(rtmp, rows_s[h:h + 1, 0:1])
            rrow = nc.gpsimd.snap(rtmp, donate=True, min_val=0, max_val=1920)
            nc.gpsimd.dma_start(shctx[bass.ds(rrow, 128), :], ctm[:, h, :])
            if h == 6:  # barrier 1: peer heads 0..6
                nc.gpsimd.dma_start(bar_in[0:1, 0:1], shctx[0:1, 0:1])
                nc.gpsimd.collective_compute(
                    'AllGather', mybir.AluOpType.bypass,
                    replica_groups=[[0, 1], [2, 3], [4, 5], [6, 7]],
                    ins=[bar_in[:].opt()], outs=[bar_out[:].opt()])

        # barrier 2 covers head 7 (barrier 1 was issued inside the G1 loop
        # after head 6's export; pokes read shctx so the RAW deps order each
        # barrier after the exports emitted before it).
        nc.gpsimd.dma_start(bar_in2[0:1, 0:1], shctx[0:1, 0:1])
        nc.gpsimd.collective_compute(
            'AllGather', mybir.AluOpType.bypass,
            replica_groups=[[0, 1], [2, 3], [4, 5], [6, 7]],
            ins=[bar_in2[:].opt()], outs=[bar_out2[:].opt()])
        ctf = [p_wq.tile([128, 1024], BF16, tag='wq', name='ctf')
               for _ in range(8)]
        for i in range(8):
            # corner poke: WAW dep orders the peer read after its barrier
            bo = bar_out if i < 7 else bar_out2
            nc.sync.dma_start(ctf[i][0:1, 0:1], bo[0:1, 0:1])
            ptmp = nc.sync.alloc_register(f'prow{i}')
            nc.sync.reg_load(ptmp, rows_s[i:i + 1, 1:2])
            prow = nc.sync.snap(ptmp, donate=True, min_val=0, max_val=1920)
            nc.sync.dma_start(ctf[i][:], shctx[bass.ds(prow, 128), :])

        # ------- o_proj: local-chunk halves lead, peer halves pipelined ----
        # owT is host-permuted to local-first chunk order, so rows 0..7 pair
        # with ctm heads and 8..15 with peer ctf chunks — uniform program.
        units = [(bi, nt) for bi in range(8) for nt in range(2)]

        oslots = [(ps_a, 'psA'), (ps_a, 'psA'), (ps_s, 'psS'), (ps_s, 'psS')]

        def o_first(u):
            bi, nt = units[u]
            pool_u, tag_u = oslots[u % 4]
            psO = pool_u.tile([128, 512], F32, tag=tag_u, name='psO')
            for i in range(8):
                nc.tensor.matmul(psO[:], ctm[:, i, bass.ts(bi, 128)],
                                 wo_s[i][:, bass.ts(nt, 512)],
                                 start=(i == 0), stop=False)
            return psO

        def o_second(u, psO):
            bi, nt = units[u]
            for i in range(8):
                nc.tensor.matmul(psO[:], ctf[i][:, bass.ts(bi, 128)],
                                 wo_s[8 + i][:, bass.ts(nt, 512)],
                                 start=False, stop=(i == 7))
            ob = p_ob.tile([128, 512], F32, tag='ob', name='ob')
            nc.vector.tensor_copy(ob[:], psO[:])
            nc.gpsimd.dma_start(
                out_e[bass.ts(bi, 128), bass.ts(nt, 512)], ob[:])

        live = []
        for u in range(16):
            live.append((u, o_first(u)))
            if len(live) == 4:
                v, psO = live.pop(0)
                o_second(v, psO)
        for v, psO in live:
            o_second(v, psO)

    split_multi_waits(nc)
    return nc


# ---------------------------------------------------------------------------
_NC_CACHE = None
_LAST_IN_MAPS = None


def _get_nc():
    global _NC_CACHE
    if _NC_CACHE is None:
        _NC_CACHE = build_kernel()
    return _NC_CACHE


def make_in_maps(hidden_states, cos, sin, q_w, k_w, v_w, o_w, q_norm_w, k_norm_w):
    import ml_dtypes
    bf16 = ml_dtypes.bfloat16

    hidden_states = np.asarray(hidden_states, np.float32)
    cos = np.asarray(cos, np.float32)
    sin = np.asarray(sin, np.float32)
    q_w = np.asarray(q_w, np.float32)
    k_w = np.asarray(k_w, np.float32)
    v_w = np.asarray(v_w, np.float32)
    o_w = np.asarray(o_w, np.float32)
    q_norm_w = np.asarray(q_norm_w, np.float32)
    k_norm_w = np.asarray(k_norm_w, np.float32)

    tri_np = np.triu(np.ones((128, 128), np.float32))  # [sj,si]: valid sj<=si
    iden_np = np.eye(128, dtype=np.float32).astype(bf16)

    def rope_tabs(c, s_, w):
        cl, sl = c[:, 0:64], s_[:, 0:64]
        wl, wh = w[0:64], w[64:128]
        return np.stack([cl * wl, sl * wh, cl * wh, sl * wl], axis=1).astype(bf16)

    in_maps = []
    for c in range(8):
        b, sh, hh = c >> 2, (c >> 1) & 1, c & 1
        blks = MYBLKS[sh]
        rows = np.concatenate([np.arange(g * 128, (g + 1) * 128) for g in blks])
        # o_w contraction rows permuted local-first: my hh half then peer half
        operm = np.concatenate([
            np.arange(hh * 1024, (hh + 1) * 1024),
            np.arange((1 - hh) * 1024, (2 - hh) * 1024)])
        myrows = ((hh * 8 + np.arange(8)) * 128).astype(np.uint32)[:, None]
        prows = (((1 - hh) * 8 + np.arange(8)) * 128).astype(np.uint32)[:, None]
        # masks indexed by global sj block j: j<8 -> si group 0 (locals 0..3),
        # j>=8 -> group 1 (locals 4..7). ones below diag, tri on diag, zero
        # above.
        msk = np.zeros((16, 128, 512), np.float32)
        for j in range(16):
            loc = range(4) if j < 8 else range(4, 8)
            for s_i, l in enumerate(loc):
                g_s = blks[l]
                if j < g_s:
                    msk[j, :, s_i * 128:(s_i + 1) * 128] = 1.0
                elif j == g_s:
                    msk[j, :, s_i * 128:(s_i + 1) * 128] = tri_np
        in_maps.append(dict(
            hT=np.ascontiguousarray(hidden_states[b].T).astype(bf16),
            hTq=np.ascontiguousarray(hidden_states[b][rows].T).astype(bf16),
            qwT=np.ascontiguousarray(q_w[hh * 1024:(hh + 1) * 1024].T).astype(bf16),
            kwT=np.ascontiguousarray(k_w[hh * 512:(hh + 1) * 512].T).astype(bf16),
            vwT=np.ascontiguousarray(v_w[hh * 512:(hh + 1) * 512].T).astype(bf16),
            owT=np.ascontiguousarray(
                o_w[hh * 1024:(hh + 1) * 1024].T[operm]).astype(bf16),
            qtab=rope_tabs(cos[b][rows], sin[b][rows], q_norm_w),
            ktab=rope_tabs(cos[b], sin[b], k_norm_w),
            mskin=msk.astype(bf16), iden=iden_np,
            myrows=myrows, prows=prows))
    return in_maps


def gather_out(outs):
    """outs: list of 8 per-core 'out' arrays -> full [B,S,HID]."""
    out = np.zeros((B, S, HID), np.float32)
    for c in range(8):
        b, sh, hh = c >> 2, (c >> 1) & 1, c & 1
        o = np.asarray(outs[c], np.float32)  # [1024, 1024]
        for l, g in enumerate(MYBLKS[sh]):
            out[b, g * 128:(g + 1) * 128, hh * 1024:(hh + 1) * 1024] = \
                o[l * 128:(l + 1) * 128]
    return out


def kernel(hidden_states, cos, sin, q_w, k_w, v_w, o_w, q_norm_w, k_norm_w):
    from concourse.bass_utils import run_bass_kernel_spmd

    in_maps = make_in_maps(hidden_states, cos, sin, q_w, k_w, v_w, o_w,
                           q_norm_w, k_norm_w)
    global _LAST_IN_MAPS
    _LAST_IN_MAPS = in_maps
    nc = _get_nc()
    res = run_bass_kernel_spmd(nc, in_maps, core_ids=list(range(8)))
    return gather_out([res.results[c]['out'] for c in range(8)])


if __name__ == '__main__':
    sys.path.insert(0, '/root/problem')
    import reference
    inputs = {k: np.asarray(v) for k, v in reference.setup_inputs().items()}
    exp = np.asarray(reference.reference(**inputs))
    act = kernel(**inputs)
    rel = np.linalg.norm(act - exp) / np.linalg.norm(exp)
    print('Relative error:', rel)


# revision 25
# speedup vs baseline: 95799.7724x; 1.0059x over previous
"""Trainium2 Bass kernel for Qwen-style GQA attention (B=2,S=2048,H=16,KV=8,D=128).

Sharding (8 cores): batch(2) x si-stripes(2) x head-half(2), uniform SPMD
program (all per-core variation flows through host-prepared inputs).
  core c: b=c>>2, sh=(c>>1)&1, hh=c&1
  stripes: MYBLKS[sh] — causally balanced interleaved si blocks.

vs the original baseline (620us -> 400us on the CoreSim cost model):
  - attention processes si in two groups of 4 local blocks with [128,512]-wide
    exp instructions (uniform j bounds 0..7 / 0..15; host masks zero the
    causal overhang), cutting Act-engine exp time ~30%
  - PSUM->SBUF copies moved off the Act engine to DVE (bf16 2x modes);
    GPSIMD never touches PSUM (hardware BIR constraint)
  - RMSNorm rstd = exp(-ln(ms+eps)/2): ln/exp/square/copy share one act
    table, so there is no act-table thrashing and norms finish inline
  - all big inputs arrive bf16 (half the HBM traffic of f32)
  - ctx^T is exchanged with the hh-partner through pair-shared HBM
    (addr_space='Shared'; per-head row offsets from an input table via
    reg_load + dynamic DMA slices keep the SPMD program uniform), guarded
    by two tiny AllGather barriers (heads 0-6 early, head 7 late)
  - o_proj splits into local-chunk halves (read from SBUF ctm during the
    barrier) and peer halves, with psO tiles parked across idle PSUM banks;
    G0 attention, pass-1 K/V projection and Q projection are interleaved
"""
import sys

sys.path.insert(0, '/opt/trn_rl_repo')

import numpy as np

import concourse.bass as bass
import concourse.tile as tile
from concourse import mybir
from concourse.vector_clock import ScopedClock, VectorClock

B, S, HID = 2, 2048, 2048
H, KV, D = 16, 8, 128
EPS = 1e-6
NBLK = S // 16  # noqa
# causally balanced si-block stripes: sum(blk+1) = 68 for both
MYBLKS = [[0, 2, 4, 6, 9, 11, 13, 15], [1, 3, 5, 7, 8, 10, 12, 14]]
JMAX = [7, 15]  # uniform j bound per si group (max over stripes)

F32 = mybir.dt.float32
BF16 = mybir.dt.bfloat16
AF = mybir.ActivationFunctionType


# ---------------------------------------------------------------------------
# Workarounds: this walrus supports only ONE sync-wait per instruction.
def _patched_drain_and_barrier(self, tick_clock, wait_clock):
    gc = tick_clock.global_clock
    vec = list(gc)
    nz = [i for i, v in enumerate(vec) if v > 0] or [0]
    for i in nz:
        cvec = [vec[j] if j == i else 0 for j in range(len(vec))]
        inst = self.nc.sync.drain()
        wait_clock.add_sem_waits(inst.ins, ScopedClock({None: VectorClock(cvec)}))
    self.nc.all_engine_barrier()
    assert self.sems is not None
    popped = self.nc._tile_sem_poison_stack.pop()
    assert popped is self._sem_poison
    self.nc.clear_and_free_semaphores(list(self.sems.allocated().values()))
    self.nc.all_engine_barrier()


tile.TileContext._drain_and_barrier = _patched_drain_and_barrier


def split_multi_waits(nc):
    for fn in nc.m.functions:
        for blk in fn.blocks:
            insts = list(blk.instructions)
            out = []
            changed = False
            for inst in insts:
                si = inst.sync_info
                if si is not None and len(si.on_wait) > 1:
                    waits = list(si.on_wait)
                    for k, w in enumerate(waits[:-1]):
                        out.append(mybir.InstNoOp(
                            name=f"{inst.name}.w{k}", engine=inst.engine,
                            sync_info=mybir.SyncInfo(on_wait=[w], on_update=[]),
                            text_hint="waitsplit"))
                    si.on_wait = [waits[-1]]
                    changed = True
                out.append(inst)
            if changed:
                blk.instructions[:] = out


def mul_b(eng, out, a, b):
    a2, b2 = bass.broadcast_tensor_aps(a, b)
    eng.tensor_mul(out, a2, b2)


# ---------------------------------------------------------------------------
def build_kernel():
    nc = bass.Bass(trn_type='TRN2')
    hT = nc.dram_tensor('hT', [HID, S], BF16, kind='ExternalInput')
    hTq = nc.dram_tensor('hTq', [HID, 1024], BF16, kind='ExternalInput')
    qwT = nc.dram_tensor('qwT', [HID, 1024], BF16, kind='ExternalInput')
    kwT = nc.dram_tensor('kwT', [HID, 512], BF16, kind='ExternalInput')
    vwT = nc.dram_tensor('vwT', [HID, 512], BF16, kind='ExternalInput')
    owT = nc.dram_tensor('owT', [2048, 1024], BF16, kind='ExternalInput')
    qtab = nc.dram_tensor('qtab', [1024, 4, 64], BF16, kind='ExternalInput')
    ktab = nc.dram_tensor('ktab', [S, 4, 64], BF16, kind='ExternalInput')
    mskin = nc.dram_tensor('mskin', [16, 128, 512], BF16, kind='ExternalInput')
    iden = nc.dram_tensor('iden', [128, 128], BF16, kind='ExternalInput')
    myrows = nc.dram_tensor('myrows', [8, 1], mybir.dt.uint32,
                            kind='ExternalInput')
    prows = nc.dram_tensor('prows', [8, 1], mybir.dt.uint32,
                           kind='ExternalInput')
    out_e = nc.dram_tensor('out', [1024, 1024], F32, kind='ExternalOutput')
    # ctx^T exchange buffer in the pair-shared HBM domain: rows hh*1024+h*128
    shctx = nc.dram_tensor('shctx', [2048, 1024], BF16, kind='Internal',
                           addr_space='Shared')
    bar_in = nc.dram_tensor('bar_in', [1, 1], BF16, kind='Internal')
    bar_out = nc.dram_tensor('bar_out', [2, 1], BF16, kind='Internal')
    bar_in2 = nc.dram_tensor('bar_in2', [1, 1], BF16, kind='Internal')
    bar_out2 = nc.dram_tensor('bar_out2', [2, 1], BF16, kind='Internal')

    from contextlib import ExitStack
    with ExitStack() as ctx:
        tc = ctx.enter_context(tile.TileContext(nc))
        pool = lambda name, bufs, **kw: ctx.enter_context(
            tc.tile_pool(name=name, bufs=bufs, **kw))
        p_wq = pool('wq', 16)     # wq tiles; ctf chunks reuse
        p_wk = pool('wk', 16)
        p_wv = pool('wv', 16)
        p_ht = pool('ht', 16)     # ht pass0 -> hTq -> ht pass1 -> wo
        p_big = pool('big', 1)    # KT/VA/QT/ctm/masks persistent
        p_c = pool('const', 1)
        p_tab = pool('tab', 4)    # streamed rope tables
        p_hq = pool('hq', 2)      # streamed hTq blocks [128,16,128]
        p_ex = pool('expb', 5)
        p_cp = pool('cpb', 6)
        p_ro = pool('rope', 6)    # rope outputs (finished inline now)
        p_t12 = pool('t12', 4)
        p_ob = pool('outb', 2)
        p_s = pool('small', 20)
        p_scl = pool('scl', 1)
        p_rd = pool('rd', 2)
        p_d = pool('dump', 1)
        ps_a = pool('psA', 2, space='PSUM')
        ps_s = pool('psS', 2, space='PSUM')
        ps_c = pool('psC', 4, space='PSUM')
        p_dram = pool('dram', 8, space='DRAM')

        # ---- constants ----
        p_msk = pool('msk', 1)
        iden_s = p_c.tile([128, 128], BF16)
        nc.gpsimd.dma_start(iden_s[:], iden[:])

        # persistent activations
        KT = p_big.tile([128, 4, 2048], BF16, name='KT')     # [d, kvh, sj]
        VA = p_big.tile([128, 4, 16, 132], BF16, name='VA')  # [sj, kvh, sb, d|1]
        QT = p_big.tile([128, 8, 1024], BF16, name='QT')     # [d, h, si-local]
        ctm = p_big.tile([128, 8, 1024], BF16, name='ctm')   # [d, h, si-local]
        sclK = p_scl.tile([128, 16, 4], F32, tag='sclK', name='sclK')
        nc.gpsimd.memset(VA[:, :, :, 128:129], 1.0)
        epsK = p_c.tile([128, 1], F32)
        nc.gpsimd.memset(epsK[:], float(D * EPS))
        epsQ = p_c.tile([128, 1], F32)
        nc.gpsimd.memset(epsQ[:], float(EPS))

        # ---- weights + first ht pass (pass-0 critical loads first) ----
        wq_s = [p_wq.tile([128, 1024], BF16, tag='wq', name='wq') for _ in range(16)]
        wk_s = [p_wk.tile([128, 512], BF16, tag='wk', name='wk') for _ in range(16)]
        wv_s = [p_wv.tile([128, 512], BF16, tag='wv', name='wv') for _ in range(16)]
        ht_t = [p_ht.tile([128, 1024], BF16, tag='ht', name='ht0')
                for _ in range(16)]
        for ch in range(16):
            r = bass.ts(ch, 128)
            nc.gpsimd.dma_start(ht_t[ch][:], hT[r, 0:1024])
            nc.gpsimd.dma_start(wv_s[ch][:], vwT[r, :])
            nc.gpsimd.dma_start(wk_s[ch][:], kwT[r, :])
        for ch in range(16):
            nc.gpsimd.dma_start(wq_s[ch][:], qwT[bass.ts(ch, 128), :])

        dump = p_d.tile([128, 128], F32, name='dump')
        rows_s = p_c.tile([8, 2], mybir.dt.uint32, name='rows_s')
        nc.gpsimd.dma_start(rows_s[:, 0:1], myrows[:])
        nc.gpsimd.dma_start(rows_s[:, 1:2], prows[:])

        # ------- helpers -------
        def rope4(cpb, tab):
            """RoPE for 4 heads packed [128, 4*128] bf16 -> new bf16 tile."""
            lo = cpb[:].rearrange('p (t d) -> p t d', t=4)[:, :, 0:64]
            hi = cpb[:].rearrange('p (t d) -> p t d', t=4)[:, :, 64:128]
            ro = p_ro.tile([128, 512], BF16, tag='ro', name='ro')
            rov = ro[:].rearrange('p (t d) -> p t d', t=4)
            t1 = p_t12.tile([128, 4, 64], BF16, tag='t12', name='t1')
            t2 = p_t12.tile([128, 4, 64], BF16, tag='t12', name='t2')
            mul_b(nc.vector, t1[:], lo, tab[:, 0:1, :])
            mul_b(nc.vector, t2[:], hi, tab[:, 1:2, :])
            nc.vector.tensor_sub(rov[:, :, 0:64], t1[:], t2[:])
            mul_b(nc.vector, t1[:], hi, tab[:, 2:3, :])
            mul_b(nc.vector, t2[:], lo, tab[:, 3:4, :])
            nc.vector.tensor_add(rov[:, :, 64:128], t1[:], t2[:])
            return ro

        def kv_unit(sb, ht_t, col, do_scl):
            """V+K projection for global sj block sb from pass tiles ht_t
            (col = 128-col offset in the pass window). During pass 0 the
            attention pools are idle, so psV borrows psS to decouple the
            V/K PSUM rotation."""
            sslice = bass.ts(col, 128)
            if sb < 8:
                psV = ps_s.tile([128, 512], F32, tag='psS', name='psV')
            else:
                psV = ps_a.tile([128, 512], F32, tag='psA', name='psV')
            for ch in range(16):
                nc.tensor.matmul(psV[:], ht_t[ch][:, sslice], wv_s[ch][:],
                                 start=(ch == 0), stop=(ch == 15))
            nc.scalar.copy(VA[:, :, sb, 0:128],
                           psV[:].rearrange('p (t d) -> p t d', t=4))
            psK = ps_a.tile([128, 512], F32, tag='psA', name='psK')
            for ch in range(16):
                nc.tensor.matmul(psK[:], ht_t[ch][:, sslice], wk_s[ch][:],
                                 start=(ch == 0), stop=(ch == 15))
            kcpb = p_cp.tile([128, 512], BF16, tag='cp', name='kcpb')
            nc.vector.tensor_copy(kcpb[:], psK[:])
            ss = p_s.tile([128, 4], F32, tag='ss', name='ssk')
            for kvh in range(4):
                nc.scalar.activation(dump[:], kcpb[:, bass.ts(kvh, 128)],
                                     AF.Square, accum_out=ss[:, kvh:kvh + 1])
            ktb = p_tab.tile([128, 4, 64], BF16, tag='ktab', name='ktb')
            nc.gpsimd.dma_start(ktb[:], ktab[sb * 128:(sb + 1) * 128])
            kro = rope4(kcpb, ktb)
            krov = kro[:].rearrange('p (t d) -> p t d', t=4)
            for kvh in range(4):
                pst = ps_s.tile([128, 128], BF16, tag='psS', name='psT')
                nc.tensor.transpose(pst[:], krov[:, kvh, :], iden_s[:])
                nc.scalar.copy(KT[:, kvh, bass.ts(sb, 128)], pst[:])
            if do_scl:
                scl_finish(sb, ss)
            return ss

        def scl_finish(sb, ss):
            # SCALE*rstd folded exactly: 1/sqrt(ss + D*eps) = exp(-ln(.)/2);
            # ln/exp/square/copy share one act table -> no table thrash.
            lt = p_s.tile([128, 4], F32, tag='std', name='lt')
            nc.scalar.activation(lt[:], ss[:], AF.Ln, bias=epsK[:])
            nc.scalar.activation(sclK[:, sb, :], lt[:], AF.Exp, scale=-0.5)

        def load_hq(l):
            hq = p_hq.tile([128, 16, 128], BF16, tag='hq', name='hq')
            nc.gpsimd.dma_start(
                hq[:], hTq[:, bass.ts(l, 128)].rearrange('(n p) d -> p n d',
                                                         p=128))
            return hq

        def q_unit(l, qg, hq):
            """Q proj+square+rope for local block l, head group qg."""
            psQ = ps_a.tile([128, 512], F32, tag='psA', name='psQ')
            for ch in range(16):
                nc.tensor.matmul(psQ[:], hq[:, ch, :],
                                 wq_s[ch][:, bass.ts(qg, 512)],
                                 start=(ch == 0), stop=(ch == 15))
            qcpb = p_cp.tile([128, 512], BF16, tag='cp', name='qcpb')
            nc.vector.tensor_copy(qcpb[:], psQ[:])
            ss = p_s.tile([128, 4], F32, tag='ss', name='ssq')
            for hq in range(4):
                nc.scalar.activation(dump[:], qcpb[:, bass.ts(hq, 128)],
                                     AF.Square, accum_out=ss[:, hq:hq + 1])
            qtb = p_tab.tile([128, 4, 64], BF16, tag='qtab', name='qtb')
            nc.gpsimd.dma_start(qtb[:], qtab[l * 128:(l + 1) * 128])
            qro = rope4(qcpb, qtb)
            return ss, qro

        def q_finish(l, qg, ss, qro):
            """rstd -> scale -> transpose into QT."""
            lt = p_s.tile([128, 4], F32, tag='std', name='ltq')
            nc.scalar.activation(lt[:], ss[:], AF.Ln, scale=1.0 / D,
                                 bias=epsQ[:])
            rstd = p_s.tile([128, 4], F32, tag='rstd', name='rstdq')
            nc.scalar.activation(rstd[:], lt[:], AF.Exp, scale=-0.5)
            qrov = qro[:].rearrange('p (t d) -> p t d', t=4)
            qn = p_ro.tile([128, 512], BF16, tag='ro', name='qn')
            qnv = qn[:].rearrange('p (t d) -> p t d', t=4)
            for hq in range(4):
                nc.vector.tensor_scalar_mul(qnv[:, hq, :], qrov[:, hq, :],
                                            rstd[:, hq:hq + 1])
                pst = ps_s.tile([128, 128], BF16, tag='psS', name='psT')
                nc.tensor.transpose(pst[:], qnv[:, hq, :], iden_s[:])
                nc.vector.tensor_copy(QT[:, qg * 4 + hq, bass.ts(l, 128)],
                                      pst[:])

        def attn_group(h, g, msk_t):
            """Attention for head h on si group g (local blocks 4g..4g+3).
            In G1 the proj pools are idle, so QK tiles also rotate through
            psA for a 4-deep pipeline."""
            kvh = h // 2
            jmax = JMAX[g]
            silo = bass.ts(g, 512)
            psCs = [ps_c.tile([128, 132], F32, tag='psC', name='psC')
                    for _ in range(4)]
            # software-pipelined QK: emit QK(j+ahead) before PV(j) so the PE
            # stream never blocks the Act exp stream on a full round trip.
            ahead = 2 if g == 1 else 1
            psSs = {}

            def do_qk(j):
                if g == 1 and j % 2 == 1:
                    psS = ps_a.tile([128, 512], F32, tag='psA', name='psS')
                else:
                    psS = ps_s.tile([128, 512], F32, tag='psS', name='psS')
                nc.tensor.matmul(psS[:], KT[:, kvh, bass.ts(j, 128)],
                                 QT[:, h, silo], start=True, stop=True)
                psSs[j] = psS

            for j in range(min(ahead, jmax + 1)):
                do_qk(j)
            for j in range(jmax + 1):
                psS = psSs.pop(j)
                ex = p_ex.tile([128, 512], BF16, tag='ex', name='ex')
                nc.scalar.activation(ex[:], psS[:], AF.Exp,
                                     scale=sclK[:, j, kvh:kvh + 1])
                if g == 0 or j >= 8:  # host mask handles diagonal + overhang
                    nc.vector.tensor_mul(ex[:], ex[:], msk_t[:, j - 8 * g, :])
                if j + ahead <= jmax:
                    do_qk(j + ahead)
                for s in range(4):
                    nc.tensor.matmul(psCs[s][:, 0:129],
                                     ex[:, bass.ts(s, 128)],
                                     VA[:, kvh, j, 0:129],
                                     start=(j == 0), stop=(j == jmax))
            for s in range(4):
                rd = p_rd.tile([128, 1], F32, tag='rd', name='rd')
                nc.vector.reciprocal(rd[:], psCs[s][:, 128:129])
                cn = p_ex.tile([128, 128], BF16, tag='cn', name='cn')
                nc.vector.tensor_scalar_mul(cn[:], psCs[s][:, 0:128], rd[:])
                pst = ps_s.tile([128, 128], BF16, tag='psS', name='psT')
                nc.tensor.transpose(pst[:], cn[:], iden_s[:])
                nc.vector.tensor_copy(ctm[:, h, bass.ts(4 * g + s, 128)],
                                      pst[:])

        # ------- pass 0: K/V proj for global sj blocks 0..7 -------
        for sb in range(8):
            kv_unit(sb, ht_t, sb, do_scl=True)
        msk_s0 = p_msk.tile([128, 8, 512], BF16, tag='msk', name='msk0')
        nc.gpsimd.dma_start(
            msk_s0[:], mskin[0:8].rearrange('t p d -> p t d'))

        # ------- Q proj for local blocks 0..3 (needed by G0) -------
        for l in range(4):
            hq = load_hq(l)
            for qg in range(2):
                ss, qro = q_unit(l, qg, hq)
                q_finish(l, qg, ss, qro)

        # ------- G0 attention interleaved with pass 1 K/V proj ------
        ht_t2 = [p_ht.tile([128, 1024], BF16, tag='ht', name='ht1')
                 for _ in range(16)]
        for ch in range(16):
            nc.gpsimd.dma_start(ht_t2[ch][:], hT[bass.ts(ch, 128), 1024:2048])
        hq_cur = [None]
        for h in range(8):
            attn_group(h, 0, msk_s0)
            sb = 8 + h
            kv_unit(sb, ht_t2, sb - 8, do_scl=True)
            l, qg = 4 + h // 2, h % 2
            if qg == 0:
                hq_cur[0] = load_hq(l)
            ss, qro = q_unit(l, qg, hq_cur[0])
            q_finish(l, qg, ss, qro)

        # ------- wo loads (reuse ht pool) -------
        wo_s = [p_ht.tile([128, 1024], BF16, tag='ht', name='wo')
                for _ in range(16)]
        for ch in range(16):
            nc.gpsimd.dma_start(wo_s[ch][:], owT[bass.ts(ch, 128), :])

        # ------- G1 attention + shared-HBM ctx export -------
        msk_s1 = p_msk.tile([128, 8, 512], BF16, tag='msk', name='msk1')
        nc.gpsimd.dma_start(
            msk_s1[:], mskin[8:16].rearrange('t p d -> p t d'))
        for h in range(8):
            attn_group(h, 1, msk_s1)
            rtmp = nc.gpsimd.alloc_register(f'myrow{h}')
            nc.gpsimd.reg_load(rtmp, rows_s[h:h + 1, 0:1])
            rrow = nc.gpsimd.snap(rtmp, donate=True, min_val=0, max_val=1920)
            nc.gpsimd.dma_start(shctx[bass.ds(rrow, 128), :], ctm[:, h, :])
            if h == 6:  # barrier 1: peer heads 0..6
                nc.gpsimd.dma_start(bar_in[0:1, 0:1], shctx[0:1, 0:1])
                nc.gpsimd.collective_compute(
                    'AllGather', mybir.AluOpType.bypass,
                    replica_groups=[[0, 1], [2, 3], [4, 5], [6, 7]],
                    ins=[bar_in[:].opt()], outs=[bar_out[:].opt()])

        # barrier 2 covers head 7 (barrier 1 was issued inside the G1 loop
        # after head 6's export; pokes read shctx so the RAW deps order each
        # barrier after the exports emitted before it).
        nc.gpsimd.dma_start(bar_in2[0:1, 0:1], shctx[0:1, 0:1])
        nc.gpsimd.collective_compute(
            'AllGather', mybir.AluOpType.bypass,
            replica_groups=[[0, 1], [2, 3], [4, 5], [6, 7]],
            ins=[bar_in2[:].opt()], outs=[bar_out2[:].opt()])
        ctf = [p_wq.tile([128, 1024], BF16, tag='wq', name='ctf')
               for _ in range(8)]
        for i in range(8):
            # corner poke: WAW dep orders the peer read after its barrier
            bo = bar_out if i < 7 else bar_out2
            nc.sync.dma_start(ctf[i][0:1, 0:1], bo[0:1, 0:1])
            ptmp = nc.sync.alloc_register(f'prow{i}')
            nc.sync.reg_load(ptmp, rows_s[i:i + 1, 1:2])
            prow = nc.sync.snap(ptmp, donate=True, min_val=0, max_val=1920)
            nc.sync.dma_start(ctf[i][:], shctx[bass.ds(prow, 128), :])

        # ------- o_proj: local-chunk halves lead, peer halves pipelined ----
        # owT is host-permuted to local-first chunk order, so rows 0..7 pair
        # with ctm heads and 8..15 with peer ctf chunks — uniform program.
        units = [(bi, nt) for bi in range(8) for nt in range(2)]

        oslots = [(ps_a, 'psA'), (ps_a, 'psA'), (ps_s, 'psS'), (ps_s, 'psS')]

        def o_first(u):
            bi, nt = units[u]
            pool_u, tag_u = oslots[u % 4]
            psO = pool_u.tile([128, 512], F32, tag=tag_u, name='psO')
            for i in range(8):
                nc.tensor.matmul(psO[:], ctm[:, i, bass.ts(bi, 128)],
                                 wo_s[i][:, bass.ts(nt, 512)],
                                 start=(i == 0), stop=False)
            return psO

        def o_second(u, psO):
            bi, nt = units[u]
            for i in range(8):
                nc.tensor.matmul(psO[:], ctf[i][:, bass.ts(bi, 128)],
                                 wo_s[8 + i][:, bass.ts(nt, 512)],
                                 start=False, stop=(i == 7))
            ob = p_ob.tile([128, 512], F32, tag='ob', name='ob')
            if u % 2 == 0:
                nc.vector.tensor_copy(ob[:], psO[:])
            else:
                nc.scalar.copy(ob[:], psO[:])
            nc.gpsimd.dma_start(
                out_e[bass.ts(bi, 128), bass.ts(nt, 512)], ob[:])

        live = []
        for u in range(16):
            live.append((u, o_first(u)))
            if len(live) == 4:
                v, psO = live.pop(0)
                o_second(v, psO)
        for v, psO in live:
            o_second(v, psO)

    split_multi_waits(nc)
    return nc


# ---------------------------------------------------------------------------
_NC_CACHE = None
_LAST_IN_MAPS = None


def _get_nc():
    global _NC_CACHE
    if _NC_CACHE is None:
        _NC_CACHE = build_kernel()
    return _NC_CACHE


def make_in_maps(hidden_states, cos, sin, q_w, k_w, v_w, o_w, q_norm_w, k_norm_w):
    import ml_dtypes
    bf16 = ml_dtypes.bfloat16

    hidden_states = np.asarray(hidden_states, np.float32)
    cos = np.asarray(cos, np.float32)
    sin = np.asarray(sin, np.float32)
    q_w = np.asarray(q_w, np.float32)
    k_w = np.asarray(k_w, np.float32)
    v_w = np.asarray(v_w, np.float32)
    o_w = np.asarray(o_w, np.float32)
    q_norm_w = np.asarray(q_norm_w, np.float32)
    k_norm_w = np.asarray(k_norm_w, np.float32)

    tri_np = np.triu(np.ones((128, 128), np.float32))  # [sj,si]: valid sj<=si
    iden_np = np.eye(128, dtype=np.float32).astype(bf16)

    def rope_tabs(c, s_, w):
        cl, sl = c[:, 0:64], s_[:, 0:64]
        wl, wh = w[0:64], w[64:128]
        return np.stack([cl * wl, sl * wh, cl * wh, sl * wl], axis=1).astype(bf16)

    in_maps = []
    for c in range(8):
        b, sh, hh = c >> 2, (c >> 1) & 1, c & 1
        blks = MYBLKS[sh]
        rows = np.concatenate([np.arange(g * 128, (g + 1) * 128) for g in blks])
        # o_w contraction rows permuted local-first: my hh half then peer half
        operm = np.concatenate([
            np.arange(hh * 1024, (hh + 1) * 1024),
            np.arange((1 - hh) * 1024, (2 - hh) * 1024)])
        myrows = ((hh * 8 + np.arange(8)) * 128).astype(np.uint32)[:, None]
        prows = (((1 - hh) * 8 + np.arange(8)) * 128).astype(np.uint32)[:, None]
        # masks indexed by global sj block j: j<8 -> si group 0 (locals 0..3),
        # j>=8 -> group 1 (locals 4..7). ones below diag, tri on diag, zero
        # above.
        msk = np.zeros((16, 128, 512), np.float32)
        for j in range(16):
            loc = range(4) if j < 8 else range(4, 8)
            for s_i, l in enumerate(loc):
                g_s = blks[l]
                if j < g_s:
                    msk[j, :, s_i * 128:(s_i + 1) * 128] = 1.0
                elif j == g_s:
                    msk[j, :, s_i * 128:(s_i + 1) * 128] = tri_np
        in_maps.append(dict(
            hT=np.ascontiguousarray(hidden_states[b].T).astype(bf16),
            hTq=np.ascontiguousarray(hidden_states[b][rows].T).astype(bf16),
            qwT=np.ascontiguousarray(q_w[hh * 1024:(hh + 1) * 1024].T).astype(bf16),
            kwT=np.ascontiguousarray(k_w[hh * 512:(hh + 1) * 512].T).astype(bf16),
            vwT=np.ascontiguousarray(v_w[hh * 512:(hh + 1) * 512].T).astype(bf16),
            owT=np.ascontiguousarray(
                o_w[hh * 1024:(hh + 1) * 1024].T[operm]).astype(bf16),
            qtab=rope_tabs(cos[b][rows], sin[b][rows], q_norm_w),
            ktab=rope_tabs(cos[b], sin[b], k_norm_w),
            mskin=msk.astype(bf16), iden=iden_np,
            myrows=myrows, prows=prows))
    return in_maps


def gather_out(outs):
    """outs: list of 8 per-core 'out' arrays -> full [B,S,HID]."""
    out = np.zeros((B, S, HID), np.float32)
    for c in range(8):
        b, sh, hh = c >> 2, (c >> 1) & 1, c & 1
        o = np.asarray(outs[c], np.float32)  # [1024, 1024]
        for l, g in enumerate(MYBLKS[sh]):
            out[b, g * 128:(g + 1) * 128, hh * 1024:(hh + 1) * 1024] = \
                o[l * 128:(l + 1) * 128]
    return out


def kernel(hidden_states, cos, sin, q_w, k_w, v_w, o_w, q_norm_w, k_norm_w):
    from concourse.bass_utils import run_bass_kernel_spmd

    in_maps = make_in_maps(hidden_states, cos, sin, q_w, k_w, v_w, o_w,
                           q_norm_w, k_norm_w)
    global _LAST_IN_MAPS
    _LAST_IN_MAPS = in_maps
    nc = _get_nc()
    res = run_bass_kernel_spmd(nc, in_maps, core_ids=list(range(8)))
    return gather_out([res.results[c]['out'] for c in range(8)])


if __name__ == '__main__':
    sys.path.insert(0, '/root/problem')
    import reference
    inputs = {k: np.asarray(v) for k, v in reference.setup_inputs().items()}
    exp = np.asarray(reference.reference(**inputs))
    act = kernel(**inputs)
    rel = np.linalg.norm(act - exp) / np.linalg.norm(exp)
    print('Relative error:', rel)
